# revision 44
# baseline (speedup 1.0000x reference)
"""DiT-SiTo block kernel builder for one NeuronCore (2 samples per core).

Index conventions (per sample):
  tokens t in [0,1024); window w in [0,256); slot s4 in {0..3}
  src index s in [0,768): s = 3*w + j (reference order)
  window-chunk layout: w = 128*c + p  (c in {0,1}, p = partition)
  gathered src rows: (p, cc) with cc = 3*c + j  ->  s = 3*(128*c+p) + j
  keep positions r in [0,512): r < 256 -> dst of window w=r; else kept src
  block token chunks: chunk c holds positions r in [128c, 128c+128), p = r%128
"""

from contextlib import ExitStack

import numpy as np

import concourse.bass as bass
import concourse.mybir as mybir
import concourse.tile as tile
from concourse.bass import IndirectOffsetOnAxis
from concourse import library_config
from concourse.masks import make_identity

I16 = mybir.dt.int16
F32 = mybir.dt.float32
F32R = mybir.dt.float32r
BF16 = mybir.dt.bfloat16
I32 = mybir.dt.int32
AF = mybir.ActivationFunctionType
ALU = mybir.AluOpType
AX = mybir.AxisListType

B2 = 2
N = 1024
D = 1152
DC = D // 128          # 9
H = 16
DH = 72
NW = 256
NS = 768
T = 512
TC = T // 128          # 4
D4 = 4608
BIG = 1.0e4
RSQ_DH = float(1.0 / np.sqrt(DH))


def host_constants():
    w = np.arange(NW)
    winbase = (64 * (w >> 4) + 2 * (w & 15)).astype(np.float32)
    iota256 = np.arange(NW, dtype=np.float32)
    jrow = np.arange(3, dtype=np.float32)
    # ltmask[cc, p, jj] = 1.0 iff jj < s(p, cc);  cc = 3*c + j
    cc = np.arange(6)
    c, j = cc // 3, cc % 3
    s = 3 * (128 * c[:, None] + np.arange(128)[None, :]) + j[:, None]   # [6,128]
    ltm = (np.arange(NS)[None, None, :] < s[:, :, None]).astype(np.float32)
    return winbase, iota256, jrow, np.ascontiguousarray(ltm)


def to_bf16(a):
    import ml_dtypes
    return np.ascontiguousarray(np.asarray(a, np.float32).astype(
        ml_dtypes.bfloat16))


def retile_weights(inp):
    """Host-side: fold LN affine into the following matmul, retile weights.

    All block weights go out partition-major and bf16 so every DMA load is
    [128, big-contiguous] (fat descriptors):
      wqk3  [128, 18*DC*128]  (p, mc, dc, col)  stationary chunks
      wv3   [128, DC*1152]    (p, dc, col)      moving rows
      wp3   [128, DC*1152]    (p, dc, col)      moving rows
      wf13  [36, 128, DC*128] (mf, p, dc, col)  stationary chunks
      wf23  [36, 128, 1152]   (kk, p, col)      moving rows
    """
    f32 = np.float32
    g1, b1 = np.asarray(inp["ln1_g"], f32), np.asarray(inp["ln1_b"], f32)
    g2, b2 = np.asarray(inp["ln2_g"], f32), np.asarray(inp["ln2_b"], f32)
    wqkv = np.asarray(inp["w_qkv"], f32); bqkv = np.asarray(inp["b_qkv"], f32)
    wfc1 = np.asarray(inp["w_fc1"], f32); bfc1 = np.asarray(inp["b_fc1"], f32)
    wqkv_f = g1[:, None] * wqkv
    bqkv_f = bqkv + b1 @ wqkv
    wfc1_f = g2[:, None] * wfc1
    bfc1_f = bfc1 + b2 @ wfc1

    # column order: [q heads 0-7 | k heads 0-7 | q heads 8-15 | k heads 8-15]
    perm = np.concatenate([
        np.arange(576), D + np.arange(576),
        576 + np.arange(576), D + 576 + np.arange(576)])
    wqk = wqkv_f[:, perm]                                      # [1152, 2304]
    wqk3 = wqk.reshape(DC, 128, 18, 128).transpose(1, 2, 0, 3).reshape(
        128, 18 * DC * 128)
    wv = wqkv_f[:, 2 * D:]
    wv3 = wv.reshape(DC, 128, D).transpose(1, 0, 2).reshape(128, DC * D)
    wp = np.asarray(inp["w_proj"], f32)
    wp3 = wp.reshape(DC, 128, D).transpose(1, 0, 2).reshape(128, DC * D)
    wf13 = wfc1_f.reshape(DC, 128, 36, 128).transpose(2, 1, 0, 3).reshape(
        36, 128, DC * 128)
    wf23 = np.asarray(inp["w_fc2"], f32).reshape(36, 128, D)
    bqk = np.ascontiguousarray(bqkv_f[perm].reshape(18, 128).T)     # [128, 18]
    return dict(
        wqk3=to_bf16(wqk3), wv3=to_bf16(wv3), wp3=to_bf16(wp3),
        wf13=to_bf16(wf13), wf23=to_bf16(wf23),
        bqk=bqk.astype(f32),
        bv_row=to_bf16(bqkv_f[None, 2 * D:]),
        bproj_row=to_bf16(np.asarray(inp["b_proj"], f32)[None, :]),
        bfc1=np.ascontiguousarray(bfc1_f.reshape(36, 128).T).astype(f32),
        bfc2_row=to_bf16(np.asarray(inp["b_fc2"], f32)[None, :]),
    )


def make_in_map(x_pair, noise_pair, weights):
    m = dict(x=np.ascontiguousarray(x_pair, np.float32),
             noise=np.ascontiguousarray(noise_pair, np.float32))
    m.update(weights)
    return m


def newton_recip(nc, pool, x, tag, iters=2):
    """r ~= 1/x to fp32 accuracy. x: [p,1] tile slice."""
    p = x.shape[0]
    r = pool.tile([p, 1], F32, tag=tag + "_r")
    t = pool.tile([p, 1], F32, tag=tag + "_t")
    nc.vector.reciprocal(r[:], x[:])
    for _ in range(iters):
        nc.vector.scalar_tensor_tensor(
            t[:], x[:], -1.0, r[:], op0=ALU.mult, op1=ALU.mult)
        nc.vector.tensor_scalar_add(t[:], t[:], 2.0)
        nc.vector.tensor_mul(r[:], r[:], t[:])
    return r


def build(nc, cfg=None):
    cfg = dict(cfg or {})
    BD = BF16
    dbg = cfg.get("debug", False)
    stop_after = cfg.get("stop_after", None)   # "index" to skip the block

    x_in = nc.dram_tensor("x", (B2, N, D), F32, kind="ExternalInput")
    noise_in = nc.dram_tensor("noise", (B2, NW, 4), F32, kind="ExternalInput")
    wqk3 = nc.dram_tensor("wqk3", (128, 18 * DC * 128), BD, kind="ExternalInput")
    wv3 = nc.dram_tensor("wv3", (128, DC * D), BD, kind="ExternalInput")
    wp3 = nc.dram_tensor("wp3", (128, DC * D), BD, kind="ExternalInput")
    wf13 = nc.dram_tensor("wf13", (36, 128, DC * 128), BD, kind="ExternalInput")
    wf23 = nc.dram_tensor("wf23", (36, 128, D), BD, kind="ExternalInput")
    bqk = nc.dram_tensor("bqk", (128, 18), F32, kind="ExternalInput")
    bv_row = nc.dram_tensor("bv_row", (1, D), BD, kind="ExternalInput")
    bproj_row = nc.dram_tensor("bproj_row", (1, D), BD, kind="ExternalInput")
    bfc1 = nc.dram_tensor("bfc1", (128, 36), F32, kind="ExternalInput")
    bfc2_row = nc.dram_tensor("bfc2_row", (1, D), BD, kind="ExternalInput")

    out = nc.dram_tensor("out", (B2, N, D), F32, kind="ExternalOutput")
    aden_d = nc.dram_tensor("aden_d", (B2, H, T), BD, kind="Internal")
    aden_raw = nc.dram_tensor("aden_raw", (B2, H, T), F32, kind="Internal")

    wb_np, iota_np, jrow_np, ltm_np = host_constants()
    winbase = nc.inline_tensor(wb_np, name="winbase")
    iota128 = nc.inline_tensor(np.arange(128, dtype=np.float32), name="iota128")
    iota256 = nc.inline_tensor(iota_np, name="iota256")
    jrow = nc.inline_tensor(jrow_np, name="jrow")
    ltmask = nc.inline_tensor(ltm_np, name="ltmask")

    okind = "ExternalOutput" if dbg else "Internal"
    xn_d = [nc.dram_tensor(f"xn_d{b}", (N, D), F32, kind=okind) for b in range(B2)]
    ktmp_d = [nc.dram_tensor(f"ktmp_d{b}", (NS,), I32, kind="Internal")
              for b in range(B2)]
    g_d = [nc.dram_tensor(f"g_d{b}", (N,), I32, kind=okind) for b in range(B2)]
    keep_d = [nc.dram_tensor(f"keep_d{b}", (T,), I32, kind=okind)
              for b in range(B2)]
    bo_d = [nc.dram_tensor(f"bo_d{b}", (T, D), BD, kind=okind)
            for b in range(B2)]
    mrow_d = [nc.dram_tensor(f"mrow_d{b}", (NS,), F32, kind="Internal")
              for b in range(B2)]
    sidx_d = [nc.dram_tensor(f"sidx_d{b}", (NS,), I16, kind="Internal")
              for b in range(B2)]
    didx_d = [nc.dram_tensor(f"didx_d{b}", (NW,), I16, kind="Internal")
              for b in range(B2)]
    kidx_d = [nc.dram_tensor(f"kidx_d{b}", (T,), I16, kind="Internal")
              for b in range(B2)]
    gidx_d = [nc.dram_tensor(f"gidx_d{b}", (N,), I16, kind="Internal")
              for b in range(B2)]
    ninv_d = [nc.dram_tensor(f"ninv_d{b}", (N,), F32, kind="Internal")
              for b in range(B2)]
    mk_d = [nc.dram_tensor(f"mk_d{b}", (NS,), F32, kind="Internal")
            for b in range(B2)]
    tok_d = [nc.dram_tensor(f"tok_d{b}", (N,), F32, kind="Internal")
             for b in range(B2)]
    val_d = [nc.dram_tensor(f"val_d{b}", (N,), F32, kind="Internal")
             for b in range(B2)]
    kperm_d = [nc.dram_tensor(f"kperm_d{b}", (T,), I32, kind="Internal")
               for b in range(B2)]
    if dbg:
        dbg_sc = nc.dram_tensor("dbg_sc", (B2, 128, 2, 4), F32,
                                kind="ExternalOutput")
        dbg_ms = nc.dram_tensor("dbg_ms", (B2, NS), F32, kind="ExternalOutput")
        dbg_best = nc.dram_tensor("dbg_best", (B2, NS), F32, kind="ExternalOutput")
        dbg_rank = nc.dram_tensor("dbg_rank", (B2, NS), F32, kind="ExternalOutput")

    ctx = ExitStack()
    tc = ctx.enter_context(tile.TileContext(nc))

    consts = ctx.enter_context(tc.tile_pool(name="consts", bufs=1))
    nc.gpsimd.load_library(library_config.mlp)
    ident = consts.tile([128, 128], F32)
    io128 = consts.tile([128, 1], F32)
    nc.sync.dma_start(io128[:], bass.AP(
        tensor=iota128, offset=0, ap=[[1, 128], [1, 1]]))
    make_identity(nc, ident)
    if BD != F32:
        identb = consts.tile([128, 128], BD)
        nc.vector.tensor_copy(identb[:], ident[:])
    else:
        identb = ident
    iotaBIG = consts.tile([128, NW], F32)
    nc.sync.dma_start(iotaBIG[:], bass.AP(
        tensor=iota256, offset=0, ap=[[0, 128], [1, NW]]))
    nc.vector.tensor_scalar_add(iotaBIG[:], iotaBIG[:], BIG)
    jb = consts.tile([128, 3], F32)
    nc.sync.dma_start(jb[:], bass.AP(tensor=jrow, offset=0, ap=[[0, 128], [1, 3]]))
    wbt = consts.tile([128, 2], F32)
    for c in range(2):
        nc.sync.dma_start(
            wbt[:, c:c + 1],
            bass.AP(tensor=winbase, offset=128 * c, ap=[[1, 128], [1, 1]]))
    witer = consts.tile([1, NW], F32)
    nc.sync.dma_start(witer[:], iota256[None, :])

    # Preload the V weight during the (latency-bound) index phase; wqk is
    # too big to coexist with the index pools and loads at block start.
    wv_pool = ctx.enter_context(tc.tile_pool(name="wv_pool", bufs=1))
    wvt = wv_pool.tile([128, DC, D], BD)

    ipools = ExitStack()
    psumS = ipools.enter_context(tc.tile_pool(name="psumS", bufs=3, space="PSUM"))
    psumT = ipools.enter_context(tc.tile_pool(name="psumT", bufs=3, space="PSUM"))

    # ============ index pipeline (phase-major, samples interleaved) ============
    # cross-phase per-sample tiles live in pX; phase-local scratch rotates
    # via bufs=2 pools so sample 1's work overlaps sample 0's DMA waits.
    pX = ipools.enter_context(tc.tile_pool(name="pX", bufs=1))
    st = {}; dt = {}; msc = {}; bst = {}; rnk = {}
    sidx_w = {}; didx_w = {}; srow_s = {}; drow_w = {}; brow = {}; rrow = {}

    # ---- A+B fused: norms, window scores, and xn all from slot tiles ----
    # Window-layout slot rows are exactly token rows of x, so sumsq over a
    # slot row reproduces the token norm bit-for-bit; xn is then written via
    # the inverse slot AP. One read of x, no chunk-layout pass at all.
    def slot_ap(b, s4, c):
        sy, sx = s4 >> 1, s4 & 1
        return bass.AP(
            tensor=x_in, offset=(b * N + 64 * 8 * c + 32 * sy + sx) * D,
            ap=[[64 * D, 8], [2 * D, 16], [1, D]])

    def slot_ap_xn(b, s4, c, hh):
        sy, sx = s4 >> 1, s4 & 1
        return bass.AP(
            tensor=xn_d[b],
            offset=(64 * 8 * c + 256 * hh + 32 * sy + sx) * D,
            ap=[[64 * D, 4], [2 * D, 16], [1, D]])

    PAIRS = [(0, 1), (0, 2), (0, 3), (1, 2), (1, 3), (2, 3)]
    PIDX = {p: i for i, p in enumerate(PAIRS)}

    with (
        tc.tile_pool(name="pB", bufs=1) as pB,
        tc.tile_pool(name="pBs", bufs=2) as pBs,
        tc.tile_pool(name="pR", bufs=2) as pR,
        tc.tile_pool(name="pXN", bufs=4) as pXN,
    ):
        slt = {}
        for b in range(B2):
            for s4 in range(4):
                for c in range(2):
                    sl = pB.tile([128, D], F32, tag=f"sl{b}_{s4}_{c}",
                                 name=f"sl{b}_{s4}_{c}")
                    nc.sync.dma_start(sl[:], slot_ap(b, s4, c))
                    slt[b, s4, c] = sl
        for dc in range(DC):
            nc.sync.dma_start(wvt[:, dc, :], wv3[:, dc * D:(dc + 1) * D])

        rslt = {}
        for b in range(B2):
            # per-token sumsq in window layout (scalar engine)
            nsq = pB.tile([128, 2, 4], F32, tag=f"nsq{b}", name=f"nsq{b}")
            for s4 in range(4):
                for c in range(2):
                    sq = pBs.tile([128, D], BF16, tag="sqscr")
                    nc.scalar.activation(
                        sq[:], slt[b, s4, c][:], AF.Square,
                        accum_out=nsq[:, c, s4:s4 + 1])
            # rsl = 1/(norm + 1e-6), same Newton chain as the reference path
            s08 = pB.tile([128, 2, 4], F32, tag=f"s08{b}", name=f"s08{b}")
            nc.scalar.activation(s08[:], nsq[:], AF.Sqrt)
            rs8 = pB.tile([128, 2, 4], F32, tag=f"rs8{b}", name=f"rs8{b}")
            t8 = pB.tile([128, 2, 4], F32, tag=f"t8{b}", name=f"t8{b}")
            nc.vector.reciprocal(rs8[:], s08[:])
            for _ in range(2):
                nc.vector.scalar_tensor_tensor(
                    t8[:], s08[:], -1.0, rs8[:], op0=ALU.mult, op1=ALU.mult)
                nc.vector.tensor_scalar_add(t8[:], t8[:], 2.0)
                nc.vector.tensor_mul(rs8[:], rs8[:], t8[:])
            q8 = pB.tile([128, 2, 4], F32, tag=f"q8{b}", name=f"q8{b}")
            nc.vector.tensor_mul(q8[:], nsq[:], rs8[:])
            nc.vector.tensor_add(q8[:], q8[:], s08[:])
            nc.vector.tensor_scalar(
                q8[:], q8[:], 0.5, 1e-6, op0=ALU.mult, op1=ALU.add)
            iv = pB.tile([128, 2, 4], F32, tag=f"inv{b}", name=f"inv{b}")
            nc.vector.reciprocal(iv[:], q8[:])
            for _ in range(2):
                nc.vector.scalar_tensor_tensor(
                    t8[:], q8[:], -1.0, iv[:], op0=ALU.mult, op1=ALU.mult)
                nc.vector.tensor_scalar_add(t8[:], t8[:], 2.0)
                nc.vector.tensor_mul(iv[:], iv[:], t8[:])
            rslt[b] = iv

        for b in range(B2):
            dd = pB.tile([128, 2, 6], F32, tag=f"dots{b}", name=f"dots{b}")
            for c in range(2):
                for pi, (sa, sb) in enumerate(PAIRS):
                    scr = pBs.tile([128, D], F32, tag="scr")
                    nc.vector.scalar_tensor_tensor(
                        scr[:], slt[b, sa, c][:], 1.0, slt[b, sb, c][:],
                        op0=ALU.bypass, op1=ALU.mult,
                        accum_out=dd[:, c, pi:pi + 1])
            nt = pB.tile([128, 2, 4], F32, tag=f"nt{b}", name=f"nt{b}")
            for c in range(2):
                nc.sync.dma_start(
                    nt[:, c, :],
                    bass.AP(tensor=noise_in, offset=(b * NW + 128 * c) * 4,
                            ap=[[4, 128], [1, 4]]))
            nc.vector.tensor_scalar_mul(nt[:], nt[:], 0.1)
            sc = pB.tile([128, 2, 4], F32, tag=f"scsc{b}", name=f"scsc{b}")
            rsl = rslt[b]
            for s4 in range(4):
                others = [s for s in range(4) if s != s4]
                acc = pBs.tile([128, 2], F32, tag="acc")
                t2 = pBs.tile([128, 2], F32, tag="t2")
                o0 = others[0]
                nc.vector.tensor_mul(
                    acc[:], dd[:, :, PIDX[min(s4, o0), max(s4, o0)]],
                    rsl[:, :, o0])
                for o in others[1:]:
                    nc.vector.tensor_mul(
                        t2[:], dd[:, :, PIDX[min(s4, o), max(s4, o)]],
                        rsl[:, :, o])
                    nc.vector.tensor_add(acc[:], acc[:], t2[:])
                nc.vector.tensor_mul(acc[:], acc[:], rsl[:, :, s4])
                nc.vector.scalar_tensor_tensor(
                    sc[:, :, s4], acc[:], 0.25, nt[:, :, s4],
                    op0=ALU.mult, op1=ALU.add)
            if dbg:
                nc.sync.dma_start(dbg_sc[b], sc[:])

            # argmax over the 4 slots, first max wins:
            # dl = (1-e0) * (1 + (1-e1) * (2 - e2))
            mm = pBs.tile([128, 2], F32, tag="mm")
            m23 = pBs.tile([128, 2], F32, tag="m23")
            nc.vector.tensor_tensor(mm[:], sc[:, :, 0], sc[:, :, 1], op=ALU.max)
            nc.vector.tensor_tensor(m23[:], sc[:, :, 2], sc[:, :, 3], op=ALU.max)
            nc.vector.tensor_tensor(mm[:], mm[:], m23[:], op=ALU.max)
            e0 = pBs.tile([128, 2], F32, tag="e0")
            e1 = pBs.tile([128, 2], F32, tag="e1")
            e2 = pBs.tile([128, 2], F32, tag="e2")
            nc.vector.tensor_tensor(e0[:], sc[:, :, 0], mm[:], op=ALU.is_equal)
            nc.vector.tensor_tensor(e1[:], sc[:, :, 1], mm[:], op=ALU.is_equal)
            nc.vector.tensor_tensor(e2[:], sc[:, :, 2], mm[:], op=ALU.is_equal)
            u2 = pBs.tile([128, 2], F32, tag="u2")
            nc.vector.tensor_scalar(
                u2[:], e2[:], -1.0, 2.0, op0=ALU.mult, op1=ALU.add)
            v1 = pBs.tile([128, 2], F32, tag="v1")
            nc.vector.scalar_tensor_tensor(
                v1[:], e1[:], -1.0, u2[:], op0=ALU.mult, op1=ALU.mult)
            u1 = pBs.tile([128, 2], F32, tag="u1")
            nc.vector.tensor_add(u1[:], v1[:], u2[:])
            nc.vector.tensor_scalar_add(u1[:], u1[:], 1.0)
            v0 = pBs.tile([128, 2], F32, tag="v0")
            nc.vector.scalar_tensor_tensor(
                v0[:], e0[:], -1.0, u1[:], op0=ALU.mult, op1=ALU.mult)
            dl = pBs.tile([128, 2], F32, tag="dl")
            nc.vector.tensor_add(dl[:], v0[:], u1[:])

            # dst token = winbase + 32*(dl>>1) + (dl&1)
            syt = pBs.tile([128, 2], F32, tag="syt")
            nc.vector.tensor_scalar(syt[:], dl[:], 2.0, None, op0=ALU.is_ge)
            sxt = pBs.tile([128, 2], F32, tag="sxt")
            nc.vector.scalar_tensor_tensor(
                sxt[:], syt[:], -2.0, dl[:], op0=ALU.mult, op1=ALU.add)
            dtb = pX.tile([128, 2], F32, tag=f"dt{b}")
            nc.vector.scalar_tensor_tensor(
                dtb[:], syt[:], 32.0, sxt[:], op0=ALU.mult, op1=ALU.add)
            nc.vector.tensor_add(dtb[:], dtb[:], wbt[:])
            dt[b] = dtb

            # src tokens [128, 2, 3]
            stb_ = pX.tile([128, 2, 3], F32, tag=f"st{b}")
            for c in range(2):
                ge = pBs.tile([128, 3], F32, tag="ge")
                nc.vector.tensor_scalar(
                    ge[:], jb[:], dl[:, c:c + 1], None, op0=ALU.is_ge)
                sl3 = pBs.tile([128, 3], F32, tag="sl3")
                nc.vector.tensor_add(sl3[:], ge[:], jb[:])
                sy2 = pBs.tile([128, 3], F32, tag="sy2")
                nc.vector.tensor_scalar(
                    sy2[:], sl3[:], 2.0, None, op0=ALU.is_ge)
                sx2 = pBs.tile([128, 3], F32, tag="sx2")
                nc.vector.scalar_tensor_tensor(
                    sx2[:], sy2[:], -2.0, sl3[:], op0=ALU.mult, op1=ALU.add)
                nc.vector.scalar_tensor_tensor(
                    stb_[:, c, :], sy2[:], 32.0, sx2[:],
                    op0=ALU.mult, op1=ALU.add)
                nc.vector.tensor_scalar_add(
                    stb_[:, c, :], stb_[:, c, :], wbt[:, c:c + 1])
            st[b] = stb_

            # ---- R: token rows + idx staging (interleaved with B) ----
            srow = pX.tile([1, NS], F32, tag=f"srow_s{b}")
            for c in range(2):
                nc.sync.dma_start(
                    srow[:, 384 * c:384 * (c + 1)].rearrange(
                        "a (p j) -> a p j", p=128),
                    st[b][:, c, :])
            srow_s[b] = srow
            drow = pX.tile([1, NW], F32, tag=f"drow_w{b}")
            for c in range(2):
                nc.sync.dma_start(
                    drow[:, 128 * c:128 * (c + 1)], dt[b][:, c:c + 1])
            drow_w[b] = drow
            # i-order idx rows for dma_gather (i = 128*chunk + p)
            sgrow = pR.tile([1, NS], F32, tag="sgrow")
            for h2 in range(2):
                for j3 in range(3):
                    nc.sync.dma_start(
                        sgrow[:, 384 * h2 + 128 * j3:384 * h2 + 128 * (j3 + 1)]
                        .rearrange("a (k o) -> a k o", o=1),
                        st[b][:, h2, j3:j3 + 1])
            sgi = pR.tile([1, NS], I16, tag="sgi")
            nc.vector.tensor_copy(sgi[:], sgrow[:])
            nc.sync.dma_start(sidx_d[b][None, :], sgi[:])
            sw = pX.tile([128, 48], I16, tag=f"sidx_w{b}")
            for g8 in range(8):
                nc.sync.dma_start(sw[16 * g8:16 * (g8 + 1), :], bass.AP(
                    tensor=sidx_d[b], offset=0, ap=[[1, 16], [16, 48]]))
            sidx_w[b] = sw
            dgi = pR.tile([1, NW], I16, tag="dgi")
            nc.vector.tensor_copy(dgi[:], drow[:])
            nc.sync.dma_start(didx_d[b][None, :], dgi[:])
            dw = pX.tile([128, 16], I16, tag=f"didx_w{b}")
            for g8 in range(8):
                nc.sync.dma_start(dw[16 * g8:16 * (g8 + 1), :], bass.AP(
                    tensor=didx_d[b], offset=0, ap=[[1, 16], [16, 16]]))
            didx_w[b] = dw

            # xn_d rows written straight from normalized slot tiles;
            # each write split across two queues, 4 bufs so the muls are
            # not serialized behind the 295KB write DMAs
            for s4 in range(4):
                for c in range(2):
                    sln = pXN.tile([128, D], F32, tag="sln")
                    nc.vector.tensor_scalar_mul(
                        sln[:], slt[b, s4, c][:], rslt[b][:, c, s4:s4 + 1])
                    for hh in range(2):
                        nc.sync.dma_start(slot_ap_xn(b, s4, c, hh),
                                          sln[64 * hh:64 * (hh + 1), :])

    # ---- C: gather xn rows, transpose, sim, best + exact stable rank ----
    with (
        tc.tile_pool(name="pC1", bufs=1) as pC,
        tc.tile_pool(name="pC2", bufs=2) as pC2,
    ):
        for b in range(B2):
            mscb = pX.tile([128, 6], F32, tag=f"msc{b}")
            bstb = pX.tile([128, 6], F32, tag=f"bst{b}")
            rnkb = pX.tile([128, 6], F32, tag=f"rnk{b}")
            xnd = pC.tile([128, 2, D], F32, tag=f"xnd{b}")
            nc.gpsimd.dma_gather(
                out_ap=xnd[:], in_ap=xn_d[b][:], idxs_ap=didx_w[b][:],
                num_idxs=NW, num_idxs_reg=NW, elem_size=D)
            xndT = pC.tile([128, DC, NW], F32, tag=f"xndT{b}")
            for c in range(2):
                for dc in range(DC):
                    pt = psumT.tile([128, 128], F32, tag="pt")
                    nc.tensor.transpose(
                        pt[:], xnd[:, c, 128 * dc:128 * (dc + 1)], ident[:])
                    if dc % 2 == 0:
                        nc.scalar.copy(
                            xndT[:, dc, 128 * c:128 * (c + 1)], pt[:])
                    else:
                        nc.vector.tensor_copy(
                            xndT[:, dc, 128 * c:128 * (c + 1)], pt[:])
            for half in range(2):
                xns = pC.tile([128, 3, D], F32, tag=f"xns{half}")
                nc.gpsimd.dma_gather(
                    out_ap=xns[:], in_ap=xn_d[b][:],
                    idxs_ap=sidx_w[b][:, 24 * half:24 * (half + 1)],
                    num_idxs=384, num_idxs_reg=384, elem_size=D)
                xnsT = pC.tile([128, 3, DC, 128], F32, tag=f"xnsT{half}")
                for c3 in range(3):
                    for dc in range(DC):
                        pt = psumT.tile([128, 128], F32, tag="pt")
                        nc.tensor.transpose(
                            pt[:], xns[:, c3, 128 * dc:128 * (dc + 1)],
                            ident[:])
                        if dc % 2 == 0:
                            nc.scalar.copy(xnsT[:, c3, dc, :], pt[:])
                        else:
                            nc.vector.tensor_copy(xnsT[:, c3, dc, :], pt[:])
                for c3 in range(3):
                    cc6 = 3 * half + c3
                    ps = psumS.tile([128, NW], F32, tag="ps")
                    for dc in range(DC):
                        nc.tensor.matmul(
                            ps[:], xnsT[:, c3, dc, :], xndT[:, dc, :],
                            start=(dc == 0), stop=(dc == DC - 1))
                    nc.vector.reduce_max(
                        mscb[:, cc6:cc6 + 1], ps[:], axis=AX.X)
                    eqt = pC.tile([128, NW], F32, tag="eqt")
                    nc.vector.tensor_scalar(
                        eqt[:], ps[:], mscb[:, cc6:cc6 + 1], None,
                        op0=ALU.is_equal)
                    mskt = pC.tile([128, NW], F32, tag="mskt")
                    nc.vector.scalar_tensor_tensor(
                        mskt[:], eqt[:], -BIG, iotaBIG[:],
                        op0=ALU.mult, op1=ALU.add)
                    nc.vector.tensor_reduce(
                        bstb[:, cc6:cc6 + 1], mskt[:], axis=AX.X, op=ALU.min)
            msc[b], bst[b], rnk[b] = mscb, bstb, rnkb

        for b in range(B2):
            # maxsim broadcast via DRAM bounce (s order)
            for cc6 in range(6):
                c, j = cc6 // 3, cc6 % 3
                nc.sync.dma_start(
                    bass.AP(tensor=mrow_d[b], offset=384 * c + j,
                            ap=[[3, 128], [1, 1]]),
                    msc[b][:, cc6:cc6 + 1])
            mbc = pC2.tile([128, NS], F32, tag="mbc")
            nc.sync.dma_start(
                mbc[:],
                bass.AP(tensor=mrow_d[b], offset=0, ap=[[0, 128], [1, NS]]))

            # exact stable rank
            gcnt = pC2.tile([128, 1], F32, tag="gcnt")
            ecnt = pC2.tile([128, 1], F32, tag="ecnt")
            for lh in range(2):
                ltm = pC2.tile([128, 3, NS], F32, tag="ltm")
                for c3 in range(3):
                    nc.sync.dma_start(ltm[:, c3, :], ltmask[3 * lh + c3])
                for c3 in range(3):
                    cc6 = 3 * lh + c3
                    sc1 = pC2.tile([128, NS], F32, tag="sc1")
                    nc.vector.scalar_tensor_tensor(
                        sc1[:], mbc[:], msc[b][:, cc6:cc6 + 1], mbc[:],
                        op0=ALU.is_gt, op1=ALU.bypass, accum_out=gcnt[:])
                    sc2 = pC2.tile([128, NS], F32, tag="sc2")
                    nc.vector.scalar_tensor_tensor(
                        sc2[:], mbc[:], msc[b][:, cc6:cc6 + 1], ltm[:, c3, :],
                        op0=ALU.is_equal, op1=ALU.mult, accum_out=ecnt[:])
                    nc.vector.tensor_add(
                        rnk[b][:, cc6:cc6 + 1], gcnt[:], ecnt[:])

            # best + rank rows in s order
            browb = pX.tile([1, NS], F32, tag=f"brow{b}")
            rrowb = pX.tile([1, NS], F32, tag=f"rrow{b}")
            for cc6 in range(6):
                c, j = cc6 // 3, cc6 % 3
                dst_b = browb[:].rearrange(
                    "a (c p j) -> a p c j", c=2, p=128)[:, :, c, j]
                nc.sync.dma_start(dst_b, bst[b][:, cc6:cc6 + 1])
                dst_r = rrowb[:].rearrange(
                    "a (c p j) -> a p c j", c=2, p=128)[:, :, c, j]
                nc.sync.dma_start(dst_r, rnk[b][:, cc6:cc6 + 1])
            brow[b], rrow[b] = browb, rrowb
            if dbg:
                nc.sync.dma_start(dbg_ms[b][None, :], mbc[0:1, :])
                nc.sync.dma_start(dbg_best[b][None, :], browb[:])
                nc.sync.dma_start(dbg_rank[b][None, :], rrowb[:])

    # ---- K: masks, prefix sum, one-hot compaction, g construction ----
    with tc.tile_pool(name="pK", bufs=2) as pK:
        for b in range(B2):
            kpm = pK.tile([1, NS], F32, tag="kpm")
            nc.vector.tensor_scalar(
                kpm[:], rrow[b][:], 512.0, None, op0=ALU.is_ge)
            kex = pK.tile([1, NS], F32, tag="kex")
            nc.vector.tensor_tensor_scan(
                kex[:], kpm[:], kpm[:], 0.0, op0=ALU.add, op1=ALU.bypass)
            nc.vector.tensor_sub(kex[:], kex[:], kpm[:])
            # v_src = best + kpm*(256 + kex - best)
            tq = pK.tile([1, NS], F32, tag="tmp768")
            nc.vector.tensor_sub(tq[:], kex[:], brow[b][:])
            nc.vector.scalar_tensor_tensor(
                tq[:], tq[:], 256.0, kpm[:], op0=ALU.add, op1=ALU.mult)
            vsr = pK.tile([1, NS], F32, tag="vsr")
            nc.vector.tensor_add(vsr[:], tq[:], brow[b][:])

            # f32 token row [dst_w | src_s] and value row [witer | vsr]
            trow = pK.tile([1, N], F32, tag="trow")
            nc.scalar.copy(trow[:, :NW], drow_w[b][:])
            nc.scalar.copy(trow[:, NW:], srow_s[b][:])
            vrow = pK.tile([1, N], F32, tag="vrow")
            nc.scalar.copy(vrow[:, :NW], witer[:])
            nc.scalar.copy(vrow[:, NW:], vsr[:])
            # masked keep-rank row: kpm*(kex+1) - 1  (pruned -> -1)
            mk = pK.tile([1, NS], F32, tag="mk")
            nc.vector.scalar_tensor_tensor(
                mk[:], kex[:], 1.0, kpm[:], op0=ALU.add, op1=ALU.mult)
            nc.vector.tensor_scalar_add(mk[:], mk[:], -1.0)
            nc.sync.dma_start(mk_d[b][None, :], mk[:])
            nc.sync.dma_start(tok_d[b][None, :], trow[:])
            nc.sync.dma_start(val_d[b][None, :], vrow[:])
            mkb = pK.tile([128, NS], F32, tag="mkb")
            nc.sync.dma_start(mkb[:], bass.AP(
                tensor=mk_d[b], offset=0, ap=[[0, 128], [1, NS]]))
            stb = pK.tile([128, NS], F32, tag="stb")
            nc.sync.dma_start(stb[:], bass.AP(
                tensor=tok_d[b], offset=NW, ap=[[0, 128], [1, NS]]))
            tkb = pK.tile([128, N], F32, tag="tkb")
            nc.sync.dma_start(tkb[:], bass.AP(
                tensor=tok_d[b], offset=0, ap=[[0, 128], [1, N]]))
            vlb = pK.tile([128, N], F32, tag="vlb")
            nc.sync.dma_start(vlb[:], bass.AP(
                tensor=val_d[b], offset=0, ap=[[0, 128], [1, N]]))

            krow = pK.tile([1, T], F32, tag="krow")
            nc.scalar.copy(krow[:, :NW], drow_w[b][:])
            eqk = pK.tile([128, NS], F32, tag="eqk")
            for c2 in range(2):
                rtg = pK.tile([128, 1], F32, tag="rtg")
                nc.vector.tensor_scalar_add(rtg[:], io128[:], float(128 * c2))
                kv = pK.tile([128, 1], F32, tag="kv")
                nc.vector.scalar_tensor_tensor(
                    eqk[:], mkb[:], rtg[:], stb[:],
                    op0=ALU.is_equal, op1=ALU.mult, accum_out=kv[:])
                seg = krow[:, NW + 128 * c2:NW + 128 * (c2 + 1)]
                nc.sync.dma_start(seg.rearrange("a (k o) -> a k o", o=1), kv[:])

            grow = pK.tile([1, N], F32, tag="grow")
            eqg = pK.tile([128, N], F32, tag="eqg")
            for c8 in range(8):
                ttg = pK.tile([128, 1], F32, tag="ttg")
                nc.vector.tensor_scalar_add(ttg[:], io128[:], float(128 * c8))
                gv = pK.tile([128, 1], F32, tag="gv")
                nc.vector.scalar_tensor_tensor(
                    eqg[:], tkb[:], ttg[:], vlb[:],
                    op0=ALU.is_equal, op1=ALU.mult, accum_out=gv[:])
                seg = grow[:, 128 * c8:128 * (c8 + 1)]
                nc.sync.dma_start(seg.rearrange("a (k o) -> a k o", o=1), gv[:])

            # int16 gather-idx staging
            ki16 = pK.tile([1, T], I16, tag="ki16")
            nc.vector.tensor_copy(ki16[:], krow[:])
            nc.sync.dma_start(kidx_d[b][None, :], ki16[:])
            gi16 = pK.tile([1, N], I16, tag="gi16")
            nc.vector.tensor_copy(gi16[:], grow[:])
            nc.sync.dma_start(gidx_d[b][None, :], gi16[:])
            if dbg:
                ki32 = pK.tile([1, T], I32, tag="ki32")
                nc.vector.tensor_copy(ki32[:], krow[:])
                nc.sync.dma_start(keep_d[b][None, :], ki32[:])
                gi32 = pK.tile([1, N], I32, tag="gi32")
                nc.vector.tensor_copy(gi32[:], grow[:])
                nc.sync.dma_start(g_d[b][None, :], gi32[:])
    ipools.close()

    if stop_after == "index":
        ctx.close()
        return dict(nc=nc)

    # =================== DiT block ===================
    build_block(nc, tc, ctx, cfg, dict(
        identb=identb, ident=ident, x_in=x_in,
        wqk3=wqk3, wvt=wvt, wp3=wp3, wf13=wf13, wf23=wf23,
        bqk=bqk, bv_row=bv_row, bproj_row=bproj_row, bfc1=bfc1,
        bfc2_row=bfc2_row, bo_d=bo_d, kidx_d=kidx_d, aden_d=aden_d,
        aden_raw=aden_raw, gidx_d=gidx_d, out=out, dbg=dbg))

    ctx.close()
    return dict(nc=nc)


def layer_norm(nc, pool, small, xin, yout, eps=1e-6):
    """Row LN: yout = (x - mu) * rsqrt(var + eps). xin fp32 [128, D]."""
    mu = small.tile([128, 1], F32, tag="ln_mu")
    nc.vector.tensor_reduce(mu[:], xin, axis=AX.X, op=ALU.add)
    nc.vector.tensor_scalar_mul(mu[:], mu[:], 1.0 / D)
    xc = pool.tile([128, D], F32, tag="ln_xc")
    nc.vector.tensor_scalar(xc[:], xin, mu[:], None, op0=ALU.subtract)
    sq = pool.tile([128, D], F32, tag="ln_sq")
    var = small.tile([128, 1], F32, tag="ln_var")
    nc.vector.scalar_tensor_tensor(
        sq[:], xc[:], 1.0, xc[:], op0=ALU.bypass, op1=ALU.mult, accum_out=var[:])
    nc.vector.tensor_scalar(
        var[:], var[:], 1.0 / D, eps, op0=ALU.mult, op1=ALU.add)
    sd = small.tile([128, 1], F32, tag="ln_sd")
    nc.scalar.activation(sd[:], var[:], AF.Sqrt)
    rstd = small.tile([128, 1], F32, tag="ln_rstd")
    nc.vector.reciprocal(rstd[:], sd[:])
    nc.vector.tensor_scalar_mul(yout, xc[:], rstd[:])


def build_block(nc, tc, ctx, cfg, env):
    BD = BF16
    stop_after = cfg.get("stop_after", None)
    identb = env["identb"]
    x_in = env["x_in"]
    wqk3, wvt = env["wqk3"], env["wvt"]
    wp3, wf13, wf23 = env["wp3"], env["wf13"], env["wf23"]
    bo_d, kidx_d = env["bo_d"], env["kidx_d"]
    aden_d, aden_raw = env["aden_d"], env["aden_raw"]
    gidx_d, out = env["gidx_d"], env["out"]

    # PSUM banks: psA 2 + psB 2 on ctx; psT (LN phases) and psPO/psD
    # (attention) are scoped so attention gets 2 bufs each: max 8 banks
    bcp = ctx.enter_context(tc.tile_pool(name="bcp", bufs=1))
    psA = ctx.enter_context(tc.tile_pool(name="psA", bufs=2, space="PSUM"))
    psB = ctx.enter_context(tc.tile_pool(name="psB", bufs=2, space="PSUM"))
    small = ctx.enter_context(tc.tile_pool(name="bsmall", bufs=4))

    bvb = bcp.tile([128, D], BD)
    nc.sync.dma_start(bvb[:], bass.AP(
        tensor=env["bv_row"], offset=0, ap=[[0, 128], [1, D]]))
    bpb = bcp.tile([128, D], BD)
    nc.sync.dma_start(bpb[:], bass.AP(
        tensor=env["bproj_row"], offset=0, ap=[[0, 128], [1, D]]))
    bf2b = bcp.tile([128, D], BD)
    nc.sync.dma_start(bf2b[:], bass.AP(
        tensor=env["bfc2_row"], offset=0, ap=[[0, 128], [1, D]]))
    bqkt = bcp.tile([128, 18], F32)
    nc.sync.dma_start(bqkt[:], env["bqk"][:, :])
    bf1t = bcp.tile([128, 36], F32)
    nc.sync.dma_start(bf1t[:], env["bfc1"][:, :])
    # all-ones stationary column (softmax denominators via matmul)
    onesb = bcp.tile([128, 1], BD)
    nc.vector.tensor_scalar(onesb[:], bqkt[:, 0:1], 0.0, 1.0,
                            op0=ALU.mult, op1=ALU.add)
    kidx_t = [None, None]
    for b in range(B2):
        kpt = bcp.tile([128, 32], I16, tag=f"kidx2_{b}", name=f"kidx2_{b}")
        for g8 in range(8):
            nc.sync.dma_start(kpt[16 * g8:16 * (g8 + 1), :], bass.AP(
                tensor=kidx_d[b], offset=0, ap=[[1, 16], [16, 32]]))
        kidx_t[b] = kpt

    def transpose_to(psT, dst_sl, y, dc):
        pt = psT.tile([128, 128], BD, tag="bt")
        nc.tensor.transpose(pt[:], y[:, 128 * dc:128 * (dc + 1)], identb[:])
        if dc % 2 == 0:
            nc.scalar.copy(dst_sl, pt[:])
        else:
            nc.vector.tensor_copy(dst_sl, pt[:])

    # x1 accumulator f32; LN1 gathers write straight into it (x rows),
    # proj then adds b_proj + attention delta, MLP adds the rest.
    p_x1 = ctx.enter_context(tc.tile_pool(name="p_x1", bufs=1))
    x1 = p_x1.tile([128, B2, TC, D], F32)

    with tc.tile_pool(name="p_ot", bufs=1) as p_ot:
        OT = [p_ot.tile([128, DC, T], BD, tag=f"OT{b}", name=f"OT{b}")
              for b in range(B2)]
        with tc.tile_pool(name="p_yt", bufs=1) as p_yt:
            YT = p_yt.tile([128, DC, 2 * T], BD)
            # ---- LN1 -> YT ----
            with (
                tc.tile_pool(name="p_ln1", bufs=2) as p_ln,
                tc.tile_pool(name="psT1", bufs=2, space="PSUM") as psT1,
            ):
                for ct in range(8):
                    b, c4 = divmod(ct, TC)
                    nc.gpsimd.dma_gather(
                        out_ap=x1[:, b, c4:c4 + 1, :], in_ap=x_in[b],
                        idxs_ap=kidx_t[b][:, 8 * c4:8 * (c4 + 1)],
                        num_idxs=128, num_idxs_reg=128, elem_size=D)
                    y = p_ln.tile([128, D], BD, tag="y")
                    layer_norm(nc, p_ln, small, x1[:, b, c4, :], y[:])
                    for dc in range(DC):
                        transpose_to(psT1, YT[:, dc, 128 * ct:128 * (ct + 1)],
                                     y, dc)

            with tc.tile_pool(name="p_v", bufs=1) as p_v:
                V = p_v.tile([128, 2 * TC, D], BD)
                # ---- V (token-major) ----
                for ct in range(8):
                    for ns in range(3):
                        pv = psB.tile([128, 384], F32, tag="b")
                        for dc in range(DC):
                            nc.tensor.matmul(
                                pv[:], YT[:, dc, 128 * ct:128 * (ct + 1)],
                                wvt[:, dc, 384 * ns:384 * (ns + 1)],
                                start=(dc == 0), stop=(dc == DC - 1))
                        nc.vector.scalar_tensor_tensor(
                            V[:, ct, 384 * ns:384 * (ns + 1)], pv[:], 1.0,
                            bvb[:, 384 * ns:384 * (ns + 1)],
                            op0=ALU.bypass, op1=ALU.add)

                if stop_after == "v":
                    return
                # ---- attention: transposed scores, no max-subtract,
                #      denominators via ones-matmul, normalize at oh ----
                with (
                    tc.tile_pool(name="p_wqk2", bufs=1) as p_wqk2,
                    tc.tile_pool(name="psPO", bufs=2, space="PSUM") as psPO,
                    tc.tile_pool(name="psD", bufs=2, space="PSUM") as psD,
                ):
                    for hg in range(2):
                        wqh = p_wqk2.tile([128, DC, DC, 128], BD, tag="wqh")
                        for mcl in range(DC):
                            mc = DC * hg + mcl
                            nc.sync.dma_start(
                                wqh[:, mcl, :, :],
                                wqk3[:, mc * D:(mc + 1) * D])
                        for b in range(B2):
                            with (
                                tc.tile_pool(name="p_qk", bufs=1) as p_qk,
                                tc.tile_pool(name="p_att", bufs=3) as p_att,
                                tc.tile_pool(name="p_et", bufs=2) as p_et,
                            ):
                                QKT = p_qk.tile([128, DC, T], BD,
                                                name=f"QKT{b}{hg}")
                                for mcl in range(DC):
                                    mc = DC * hg + mcl
                                    pq = psA.tile([128, T], F32, tag="a")
                                    for dc in range(DC):
                                        nc.tensor.matmul(
                                            pq[:], wqh[:, mcl, dc, :],
                                            YT[:, dc, T * b:T * (b + 1)],
                                            start=(dc == 0),
                                            stop=(dc == DC - 1))
                                    if mcl % 2 == 0:
                                        nc.scalar.activation(
                                            QKT[:, mcl, :], pq[:],
                                            AF.Identity,
                                            bias=bqkt[:, mc:mc + 1])
                                    else:
                                        nc.vector.tensor_scalar(
                                            QKT[:, mcl, :], pq[:],
                                            bqkt[:, mc:mc + 1], None,
                                            op0=ALU.add)
                                for hl in range(8):
                                    h = 8 * hg + hl
                                    qh = p_att.tile([DH, T], BD, tag="qh")
                                    kh = p_att.tile([DH, T], BD, tag="kh")
                                    for (dst, base) in ((qh, DH * hl),
                                                        (kh, 576 + DH * hl)):
                                        r0 = base
                                        while r0 < base + DH:
                                            mcl, p0 = divmod(r0, 128)
                                            take = min(128 - p0,
                                                       base + DH - r0)
                                            nc.sync.dma_start(
                                                dst[r0 - base:
                                                    r0 - base + take, :],
                                                QKT[p0:p0 + take, mcl, :])
                                            r0 += take
                                    # E^T chunks from transposed scores
                                    ET = p_et.tile([128, TC, T], BD, tag="ET")
                                    for kc in range(TC):
                                        ps = psA.tile([128, T], F32, tag="a")
                                        nc.tensor.matmul(
                                            ps[:],
                                            kh[:, 128 * kc:128 * (kc + 1)],
                                            qh[:], start=True, stop=True)
                                        nc.scalar.activation(
                                            ET[:, kc, :], ps[:], AF.Exp,
                                            scale=RSQ_DH)
                                    # o^T (unnormalized) and denominators
                                    po = psPO.tile([DH, T], F32, tag="po")
                                    pden = psD.tile([1, T], F32, tag="pden")
                                    for kc in range(TC):
                                        nc.tensor.matmul(
                                            po[:],
                                            V[:, TC * b + kc,
                                              DH * h:DH * (h + 1)],
                                            ET[:, kc, :],
                                            start=(kc == 0),
                                            stop=(kc == TC - 1))
                                        nc.tensor.matmul(
                                            pden[:], onesb[:], ET[:, kc, :],
                                            start=(kc == 0),
                                            stop=(kc == TC - 1))
                                    # reciprocal via [32,16] DRAM reshape
                                    # ([1,512] DVE recip is ~4cyc/elem serial)
                                    rraw = small.tile([1, T], F32, tag="rraw")
                                    nc.scalar.copy(rraw[:], pden[:])
                                    nc.sync.dma_start(
                                        aden_raw[b, h][None, :], rraw[:])
                                    r32 = small.tile([32, 16], F32, tag="r32")
                                    nc.sync.dma_start(r32[:], bass.AP(
                                        tensor=aden_raw,
                                        offset=(b * H + h) * T,
                                        ap=[[16, 32], [1, 16]]))
                                    rd32 = small.tile([32, 16], F32,
                                                      tag="rd32")
                                    nc.vector.reciprocal(rd32[:], r32[:])
                                    rdb = small.tile([32, 16], BD, tag="rdb")
                                    nc.vector.tensor_copy(rdb[:], rd32[:])
                                    nc.sync.dma_start(
                                        bass.AP(tensor=aden_d,
                                                offset=(b * H + h) * T,
                                                ap=[[16, 32], [1, 16]]),
                                        rdb[:])
                                    rbc = p_att.tile([DH, T], BD, tag="rbc")
                                    nc.sync.dma_start(rbc[:], bass.AP(
                                        tensor=aden_d, offset=(b * H + h) * T,
                                        ap=[[0, DH], [1, T]]))
                                    oh = p_att.tile([DH, T], BD, tag="oh")
                                    nc.vector.tensor_tensor(
                                        oh[:], po[:], rbc[:], op=ALU.mult)
                                    r0 = DH * h
                                    while r0 < DH * (h + 1):
                                        dc, p0 = divmod(r0, 128)
                                        take = min(128 - p0, DH * (h + 1) - r0)
                                        nc.sync.dma_start(
                                            OT[b][p0:p0 + take, dc, :],
                                            oh[r0 - DH * h:
                                               r0 - DH * h + take, :])
                                        r0 += take

        # ---- proj + residual -> x1, full 9-dc PSUM accumulation ----
        with tc.tile_pool(name="p_wp", bufs=1) as p_wp:
            wpt = p_wp.tile([128, DC, D], BD)
            for dc in range(DC):
                nc.sync.dma_start(wpt[:, dc, :], wp3[:, dc * D:(dc + 1) * D])
            for b in range(B2):
                for c4 in range(TC):
                    nc.vector.tensor_add(
                        x1[:, b, c4, :], x1[:, b, c4, :], bpb[:])
                    for ns in range(3):
                        pp = psB.tile([128, 384], F32, tag="b")
                        for dc in range(DC):
                            nc.tensor.matmul(
                                pp[:], OT[b][:, dc, 128 * c4:128 * (c4 + 1)],
                                wpt[:, dc, 384 * ns:384 * (ns + 1)],
                                start=(dc == 0), stop=(dc == DC - 1))
                        sl = x1[:, b, c4, 384 * ns:384 * (ns + 1)]
                        nc.vector.scalar_tensor_tensor(
                            sl, pp[:], 1.0, sl, op0=ALU.bypass, op1=ALU.add)

    if stop_after == "attn":
        return
    # ---- LN2 + MLP (per-sample supergroups of 18) + inline recover ----
    with tc.tile_pool(name="p_y2", bufs=1) as p_y2:
        Y2T = p_y2.tile([128, DC, 2 * T], BD)
        with (
            tc.tile_pool(name="p_ln2", bufs=2) as p_ln,
            tc.tile_pool(name="psT2", bufs=2, space="PSUM") as psT2,
        ):
            for ct in range(8):
                b, c4 = divmod(ct, TC)
                y = p_ln.tile([128, D], BD, tag="y")
                layer_norm(nc, p_ln, small, x1[:, b, c4, :], y[:])
                for dc in range(DC):
                    transpose_to(psT2, Y2T[:, dc, 128 * ct:128 * (ct + 1)],
                                 y, dc)
                nc.vector.tensor_add(
                    x1[:, b, c4, :], x1[:, b, c4, :], bf2b[:])

        def emit_recover(rb):
            # recover sample rb: gather block rows back to full token order,
            # split in halves so the out-write of half 0 overlaps half 1
            with tc.tile_pool(name="recp", bufs=2) as recp:
                gw = recp.tile([128, 64], I16, tag="gw")
                for g8 in range(8):
                    nc.sync.dma_start(gw[16 * g8:16 * (g8 + 1), :], bass.AP(
                        tensor=gidx_d[rb], offset=0, ap=[[1, 16], [16, 64]]))
                for hf in range(2):
                    og = recp.tile([128, 4, D], BD, tag="og")
                    nc.gpsimd.dma_gather(
                        out_ap=og[:], in_ap=bo_d[rb][:],
                        idxs_ap=gw[:, 32 * hf:32 * (hf + 1)],
                        num_idxs=T, num_idxs_reg=T, elem_size=D)
                    ogf = recp.tile([128, 4, D], F32, tag="ogf")
                    nc.vector.tensor_copy(ogf[:], og[:])
                    nc.sync.dma_start(
                        bass.AP(tensor=out, offset=(rb * N + hf * T) * D,
                                ap=[[D, 128], [128 * D, 4], [1, D]]),
                        ogf[:])

        for b in range(B2):
            for sg in range(2):
                with tc.tile_pool(name="p_ht", bufs=1) as p_ht:
                    HT = p_ht.tile([128, 18, T], BD, name=f"HT{b}{sg}")
                    with tc.tile_pool(name="p_wf1", bufs=3) as p_wf1:
                        for k18 in range(18):
                            mf = 18 * sg + k18
                            wt = p_wf1.tile([128, DC, 128], BD, tag="wf1")
                            nc.sync.dma_start(wt[:], wf13[mf])
                            pf = psA.tile([128, T], F32, tag="a")
                            for dc in range(DC):
                                nc.tensor.matmul(
                                    pf[:], wt[:, dc, :],
                                    Y2T[:, dc, T * b:T * (b + 1)],
                                    start=(dc == 0), stop=(dc == DC - 1))
                            nc.scalar.activation(
                                HT[:, k18, :], pf[:],
                                AF.Gelu_apprx_tanh, bias=bf1t[:, mf:mf + 1])
                    with tc.tile_pool(name="p_wf2", bufs=1) as p_wf2:
                        wf2 = [p_wf2.tile([128, D], BD, tag=f"wf2_{i}",
                                          name=f"wf2t{b}{sg}{i}")
                               for i in range(18)]
                        for i in range(18):
                            nc.sync.dma_start(wf2[i][:], wf23[18 * sg + i])
                        for c4 in range(TC):
                            for ns in range(3):
                                pg = psB.tile([128, 384], F32, tag="b")
                                for i in range(18):
                                    nc.tensor.matmul(
                                        pg[:],
                                        HT[:, i, 128 * c4:128 * (c4 + 1)],
                                        wf2[i][:, 384 * ns:384 * (ns + 1)],
                                        start=(i == 0), stop=(i == 17))
                                sl = x1[:, b, c4, 384 * ns:384 * (ns + 1)]
                                nc.vector.scalar_tensor_tensor(
                                    sl, pg[:], 1.0, sl,
                                    op0=ALU.bypass, op1=ALU.add)

            # ---- write block output rows (DRAM row = 128*c4 + p) ----
            for c4 in range(TC):
                xob = small.tile([128, D], BD, tag="xob")
                nc.vector.tensor_copy(xob[:], x1[:, b, c4, :])
                nc.sync.dma_start(
                    bass.AP(tensor=bo_d[b], offset=c4 * 128 * D,
                            ap=[[D, 128], [1, D]]),
                    xob[:])
            emit_recover(b)


# ======================================================================
# kernel() entry point: full inputs -> full output on 8 NeuronCores
# ======================================================================

_MODULE_CACHE = {}


def _get_module(block_dtype_name):
    # block_dtype_name kept for interface compat; the block is always bf16
    if "bf16" not in _MODULE_CACHE:
        from concourse import bacc
        nc = bacc.Bacc(None, target_bir_lowering=False)
        build(nc, {})
        nc.compile()
        _MODULE_CACHE["bf16"] = nc
    return _MODULE_CACHE["bf16"]


def kernel(x, noise, ln1_g, ln1_b, ln2_g, ln2_b, w_qkv, b_qkv, w_proj, b_proj,
           w_fc1, b_fc1, w_fc2, b_fc2, block_dtype="f32r", **run_kw):
    from concourse import bass_utils

    x = np.ascontiguousarray(np.asarray(x, np.float32))
    noise = np.ascontiguousarray(np.asarray(noise, np.float32))
    B = x.shape[0]
    n_cores = B // B2
    wt = retile_weights(
        dict(ln1_g=ln1_g, ln1_b=ln1_b, ln2_g=ln2_g, ln2_b=ln2_b,
             w_qkv=w_qkv, b_qkv=b_qkv, w_proj=w_proj, b_proj=b_proj,
             w_fc1=w_fc1, b_fc1=b_fc1, w_fc2=w_fc2, b_fc2=b_fc2))

    nc = _get_module(block_dtype)
    in_maps = []
    for c in range(n_cores):
        m = dict(x=x[B2 * c:B2 * (c + 1)], noise=noise[B2 * c:B2 * (c + 1)])
        m.update(wt)
        in_maps.append(m)
    res = bass_utils.run_bass_kernel_spmd(
        nc, in_maps, core_ids=list(range(n_cores)), **run_kw)
    out = np.concatenate([res.results[c]["out"] for c in range(n_cores)], axis=0)
    if run_kw.get("trace"):
        return out, res
    return out



# revision 45
# speedup vs baseline: 1.0024x; 1.0024x over previous
"""DiT-SiTo block kernel builder for one NeuronCore (2 samples per core).

Index conventions (per sample):
  tokens t in [0,1024); window w in [0,256); slot s4 in {0..3}
  src index s in [0,768): s = 3*w + j (reference order)
  window-chunk layout: w = 128*c + p  (c in {0,1}, p = partition)
  gathered src rows: (p, cc) with cc = 3*c + j  ->  s = 3*(128*c+p) + j
  keep positions r in [0,512): r < 256 -> dst of window w=r; else kept src
  block token chunks: chunk c holds positions r in [128c, 128c+128), p = r%128
"""

from contextlib import ExitStack

import numpy as np

import concourse.bass as bass
import concourse.mybir as mybir
import concourse.tile as tile
from concourse.bass import IndirectOffsetOnAxis
from concourse import library_config
from concourse.masks import make_identity

I16 = mybir.dt.int16
F32 = mybir.dt.float32
F32R = mybir.dt.float32r
BF16 = mybir.dt.bfloat16
I32 = mybir.dt.int32
AF = mybir.ActivationFunctionType
ALU = mybir.AluOpType
AX = mybir.AxisListType

B2 = 2
N = 1024
D = 1152
DC = D // 128          # 9
H = 16
DH = 72
NW = 256
NS = 768
T = 512
TC = T // 128          # 4
D4 = 4608
BIG = 1.0e4
RSQ_DH = float(1.0 / np.sqrt(DH))


def host_constants():
    w = np.arange(NW)
    winbase = (64 * (w >> 4) + 2 * (w & 15)).astype(np.float32)
    iota256 = np.arange(NW, dtype=np.float32)
    jrow = np.arange(3, dtype=np.float32)
    # ltmask[cc, p, jj] = 1.0 iff jj < s(p, cc);  cc = 3*c + j
    cc = np.arange(6)
    c, j = cc // 3, cc % 3
    s = 3 * (128 * c[:, None] + np.arange(128)[None, :]) + j[:, None]   # [6,128]
    ltm = (np.arange(NS)[None, None, :] < s[:, :, None]).astype(np.float32)
    return winbase, iota256, jrow, np.ascontiguousarray(ltm)


def to_bf16(a):
    import ml_dtypes
    return np.ascontiguousarray(np.asarray(a, np.float32).astype(
        ml_dtypes.bfloat16))


def retile_weights(inp):
    """Host-side: fold LN affine into the following matmul, retile weights.

    All block weights go out partition-major and bf16 so every DMA load is
    [128, big-contiguous] (fat descriptors):
      wqk3  [128, 18*DC*128]  (p, mc, dc, col)  stationary chunks
      wv3   [128, DC*1152]    (p, dc, col)      moving rows
      wp3   [128, DC*1152]    (p, dc, col)      moving rows
      wf13  [36, 128, DC*128] (mf, p, dc, col)  stationary chunks
      wf23  [36, 128, 1152]   (kk, p, col)      moving rows
    """
    f32 = np.float32
    g1, b1 = np.asarray(inp["ln1_g"], f32), np.asarray(inp["ln1_b"], f32)
    g2, b2 = np.asarray(inp["ln2_g"], f32), np.asarray(inp["ln2_b"], f32)
    wqkv = np.asarray(inp["w_qkv"], f32); bqkv = np.asarray(inp["b_qkv"], f32)
    wfc1 = np.asarray(inp["w_fc1"], f32); bfc1 = np.asarray(inp["b_fc1"], f32)
    wqkv_f = g1[:, None] * wqkv
    bqkv_f = bqkv + b1 @ wqkv
    wfc1_f = g2[:, None] * wfc1
    bfc1_f = bfc1 + b2 @ wfc1

    # column order: [q heads 0-7 | k heads 0-7 | q heads 8-15 | k heads 8-15]
    perm = np.concatenate([
        np.arange(576), D + np.arange(576),
        576 + np.arange(576), D + 576 + np.arange(576)])
    wqk = wqkv_f[:, perm]                                      # [1152, 2304]
    wqk3 = wqk.reshape(DC, 128, 18, 128).transpose(1, 2, 0, 3).reshape(
        128, 18 * DC * 128)
    wv = wqkv_f[:, 2 * D:]
    wv3 = wv.reshape(DC, 128, D).transpose(1, 0, 2).reshape(128, DC * D)
    wp = np.asarray(inp["w_proj"], f32)
    wp3 = wp.reshape(DC, 128, D).transpose(1, 0, 2).reshape(128, DC * D)
    wf13 = wfc1_f.reshape(DC, 128, 36, 128).transpose(2, 1, 0, 3).reshape(
        36, 128, DC * 128)
    wf23 = np.asarray(inp["w_fc2"], f32).reshape(36, 128, D)
    bqk = np.ascontiguousarray(bqkv_f[perm].reshape(18, 128).T)     # [128, 18]
    return dict(
        wqk3=to_bf16(wqk3), wv3=to_bf16(wv3), wp3=to_bf16(wp3),
        wf13=to_bf16(wf13), wf23=to_bf16(wf23),
        bqk=bqk.astype(f32),
        bv_row=to_bf16(bqkv_f[None, 2 * D:]),
        bproj_row=to_bf16(np.asarray(inp["b_proj"], f32)[None, :]),
        bfc1=np.ascontiguousarray(bfc1_f.reshape(36, 128).T).astype(f32),
        bfc2_row=to_bf16(np.asarray(inp["b_fc2"], f32)[None, :]),
    )


def make_in_map(x_pair, noise_pair, weights):
    m = dict(x=np.ascontiguousarray(x_pair, np.float32),
             noise=np.ascontiguousarray(noise_pair, np.float32))
    m.update(weights)
    return m


def newton_recip(nc, pool, x, tag, iters=2):
    """r ~= 1/x to fp32 accuracy. x: [p,1] tile slice."""
    p = x.shape[0]
    r = pool.tile([p, 1], F32, tag=tag + "_r")
    t = pool.tile([p, 1], F32, tag=tag + "_t")
    nc.vector.reciprocal(r[:], x[:])
    for _ in range(iters):
        nc.vector.scalar_tensor_tensor(
            t[:], x[:], -1.0, r[:], op0=ALU.mult, op1=ALU.mult)
        nc.vector.tensor_scalar_add(t[:], t[:], 2.0)
        nc.vector.tensor_mul(r[:], r[:], t[:])
    return r


def build(nc, cfg=None):
    cfg = dict(cfg or {})
    BD = BF16
    dbg = cfg.get("debug", False)
    stop_after = cfg.get("stop_after", None)   # "index" to skip the block

    x_in = nc.dram_tensor("x", (B2, N, D), F32, kind="ExternalInput")
    noise_in = nc.dram_tensor("noise", (B2, NW, 4), F32, kind="ExternalInput")
    wqk3 = nc.dram_tensor("wqk3", (128, 18 * DC * 128), BD, kind="ExternalInput")
    wv3 = nc.dram_tensor("wv3", (128, DC * D), BD, kind="ExternalInput")
    wp3 = nc.dram_tensor("wp3", (128, DC * D), BD, kind="ExternalInput")
    wf13 = nc.dram_tensor("wf13", (36, 128, DC * 128), BD, kind="ExternalInput")
    wf23 = nc.dram_tensor("wf23", (36, 128, D), BD, kind="ExternalInput")
    bqk = nc.dram_tensor("bqk", (128, 18), F32, kind="ExternalInput")
    bv_row = nc.dram_tensor("bv_row", (1, D), BD, kind="ExternalInput")
    bproj_row = nc.dram_tensor("bproj_row", (1, D), BD, kind="ExternalInput")
    bfc1 = nc.dram_tensor("bfc1", (128, 36), F32, kind="ExternalInput")
    bfc2_row = nc.dram_tensor("bfc2_row", (1, D), BD, kind="ExternalInput")

    out = nc.dram_tensor("out", (B2, N, D), F32, kind="ExternalOutput")
    aden_d = nc.dram_tensor("aden_d", (B2, H, T), BD, kind="Internal")
    aden_raw = nc.dram_tensor("aden_raw", (B2, H, T), F32, kind="Internal")

    wb_np, iota_np, jrow_np, ltm_np = host_constants()
    winbase = nc.inline_tensor(wb_np, name="winbase")
    iota128 = nc.inline_tensor(np.arange(128, dtype=np.float32), name="iota128")
    iota256 = nc.inline_tensor(iota_np, name="iota256")
    jrow = nc.inline_tensor(jrow_np, name="jrow")
    ltmask = nc.inline_tensor(ltm_np, name="ltmask")

    okind = "ExternalOutput" if dbg else "Internal"
    xn_d = [nc.dram_tensor(f"xn_d{b}", (N, D), F32, kind=okind) for b in range(B2)]
    ktmp_d = [nc.dram_tensor(f"ktmp_d{b}", (NS,), I32, kind="Internal")
              for b in range(B2)]
    g_d = [nc.dram_tensor(f"g_d{b}", (N,), I32, kind=okind) for b in range(B2)]
    keep_d = [nc.dram_tensor(f"keep_d{b}", (T,), I32, kind=okind)
              for b in range(B2)]
    bo_d = [nc.dram_tensor(f"bo_d{b}", (T, D), BD, kind=okind)
            for b in range(B2)]
    mrow_d = [nc.dram_tensor(f"mrow_d{b}", (NS,), F32, kind="Internal")
              for b in range(B2)]
    sidx_d = [nc.dram_tensor(f"sidx_d{b}", (NS,), I16, kind="Internal")
              for b in range(B2)]
    didx_d = [nc.dram_tensor(f"didx_d{b}", (NW,), I16, kind="Internal")
              for b in range(B2)]
    kidx_d = [nc.dram_tensor(f"kidx_d{b}", (T,), I16, kind="Internal")
              for b in range(B2)]
    gidx_d = [nc.dram_tensor(f"gidx_d{b}", (N,), I16, kind="Internal")
              for b in range(B2)]
    ninv_d = [nc.dram_tensor(f"ninv_d{b}", (N,), F32, kind="Internal")
              for b in range(B2)]
    mk_d = [nc.dram_tensor(f"mk_d{b}", (NS,), F32, kind="Internal")
            for b in range(B2)]
    tok_d = [nc.dram_tensor(f"tok_d{b}", (N,), F32, kind="Internal")
             for b in range(B2)]
    val_d = [nc.dram_tensor(f"val_d{b}", (N,), F32, kind="Internal")
             for b in range(B2)]
    kperm_d = [nc.dram_tensor(f"kperm_d{b}", (T,), I32, kind="Internal")
               for b in range(B2)]
    if dbg:
        dbg_sc = nc.dram_tensor("dbg_sc", (B2, 128, 2, 4), F32,
                                kind="ExternalOutput")
        dbg_ms = nc.dram_tensor("dbg_ms", (B2, NS), F32, kind="ExternalOutput")
        dbg_best = nc.dram_tensor("dbg_best", (B2, NS), F32, kind="ExternalOutput")
        dbg_rank = nc.dram_tensor("dbg_rank", (B2, NS), F32, kind="ExternalOutput")

    ctx = ExitStack()
    tc = ctx.enter_context(tile.TileContext(nc))

    consts = ctx.enter_context(tc.tile_pool(name="consts", bufs=1))
    nc.gpsimd.load_library(library_config.mlp)
    ident = consts.tile([128, 128], F32)
    io128 = consts.tile([128, 1], F32)
    nc.sync.dma_start(io128[:], bass.AP(
        tensor=iota128, offset=0, ap=[[1, 128], [1, 1]]))
    make_identity(nc, ident)
    if BD != F32:
        identb = consts.tile([128, 128], BD)
        nc.vector.tensor_copy(identb[:], ident[:])
    else:
        identb = ident
    iotaBIG = consts.tile([128, NW], F32)
    nc.sync.dma_start(iotaBIG[:], bass.AP(
        tensor=iota256, offset=0, ap=[[0, 128], [1, NW]]))
    nc.vector.tensor_scalar_add(iotaBIG[:], iotaBIG[:], BIG)
    jb = consts.tile([128, 3], F32)
    nc.sync.dma_start(jb[:], bass.AP(tensor=jrow, offset=0, ap=[[0, 128], [1, 3]]))
    wbt = consts.tile([128, 2], F32)
    for c in range(2):
        nc.sync.dma_start(
            wbt[:, c:c + 1],
            bass.AP(tensor=winbase, offset=128 * c, ap=[[1, 128], [1, 1]]))
    witer = consts.tile([1, NW], F32)
    nc.sync.dma_start(witer[:], iota256[None, :])

    # Preload the V weight during the (latency-bound) index phase; wqk is
    # too big to coexist with the index pools and loads at block start.
    wv_pool = ctx.enter_context(tc.tile_pool(name="wv_pool", bufs=1))
    wvt = wv_pool.tile([128, DC, D], BD)

    ipools = ExitStack()
    psumS = ipools.enter_context(tc.tile_pool(name="psumS", bufs=3, space="PSUM"))
    psumT = ipools.enter_context(tc.tile_pool(name="psumT", bufs=3, space="PSUM"))

    # ============ index pipeline (phase-major, samples interleaved) ============
    # cross-phase per-sample tiles live in pX; phase-local scratch rotates
    # via bufs=2 pools so sample 1's work overlaps sample 0's DMA waits.
    pX = ipools.enter_context(tc.tile_pool(name="pX", bufs=1))
    st = {}; dt = {}; msc = {}; bst = {}; rnk = {}
    sidx_w = {}; didx_w = {}; srow_s = {}; drow_w = {}; brow = {}; rrow = {}

    # ---- A+B fused: norms, window scores, and xn all from slot tiles ----
    # Window-layout slot rows are exactly token rows of x, so sumsq over a
    # slot row reproduces the token norm bit-for-bit; xn is then written via
    # the inverse slot AP. One read of x, no chunk-layout pass at all.
    def slot_ap(b, s4, c):
        sy, sx = s4 >> 1, s4 & 1
        return bass.AP(
            tensor=x_in, offset=(b * N + 64 * 8 * c + 32 * sy + sx) * D,
            ap=[[64 * D, 8], [2 * D, 16], [1, D]])

    def slot_ap_xn(b, s4, c):
        sy, sx = s4 >> 1, s4 & 1
        return bass.AP(
            tensor=xn_d[b], offset=(64 * 8 * c + 32 * sy + sx) * D,
            ap=[[64 * D, 8], [2 * D, 16], [1, D]])

    PAIRS = [(0, 1), (0, 2), (0, 3), (1, 2), (1, 3), (2, 3)]
    PIDX = {p: i for i, p in enumerate(PAIRS)}

    with (
        tc.tile_pool(name="pB", bufs=1) as pB,
        tc.tile_pool(name="pBs", bufs=2) as pBs,
        tc.tile_pool(name="pR", bufs=2) as pR,
    ):
        slt = {}
        for b in range(B2):
            for s4 in range(4):
                for c in range(2):
                    sl = pB.tile([128, D], F32, tag=f"sl{b}_{s4}_{c}",
                                 name=f"sl{b}_{s4}_{c}")
                    nc.sync.dma_start(sl[:], slot_ap(b, s4, c))
                    slt[b, s4, c] = sl
        for dc in range(DC):
            nc.sync.dma_start(wvt[:, dc, :], wv3[:, dc * D:(dc + 1) * D])

        rslt = {}
        for b in range(B2):
            # per-token sumsq in window layout (scalar engine)
            nsq = pB.tile([128, 2, 4], F32, tag=f"nsq{b}", name=f"nsq{b}")
            for s4 in range(4):
                for c in range(2):
                    sq = pBs.tile([128, D], BF16, tag="sqscr")
                    nc.scalar.activation(
                        sq[:], slt[b, s4, c][:], AF.Square,
                        accum_out=nsq[:, c, s4:s4 + 1])
            # rsl = 1/(norm + 1e-6), same Newton chain as the reference path
            s08 = pB.tile([128, 2, 4], F32, tag=f"s08{b}", name=f"s08{b}")
            nc.scalar.activation(s08[:], nsq[:], AF.Sqrt)
            rs8 = pB.tile([128, 2, 4], F32, tag=f"rs8{b}", name=f"rs8{b}")
            t8 = pB.tile([128, 2, 4], F32, tag=f"t8{b}", name=f"t8{b}")
            nc.vector.reciprocal(rs8[:], s08[:])
            for _ in range(2):
                nc.vector.scalar_tensor_tensor(
                    t8[:], s08[:], -1.0, rs8[:], op0=ALU.mult, op1=ALU.mult)
                nc.vector.tensor_scalar_add(t8[:], t8[:], 2.0)
                nc.vector.tensor_mul(rs8[:], rs8[:], t8[:])
            q8 = pB.tile([128, 2, 4], F32, tag=f"q8{b}", name=f"q8{b}")
            nc.vector.tensor_mul(q8[:], nsq[:], rs8[:])
            nc.vector.tensor_add(q8[:], q8[:], s08[:])
            nc.vector.tensor_scalar(
                q8[:], q8[:], 0.5, 1e-6, op0=ALU.mult, op1=ALU.add)
            iv = pB.tile([128, 2, 4], F32, tag=f"inv{b}", name=f"inv{b}")
            nc.vector.reciprocal(iv[:], q8[:])
            for _ in range(2):
                nc.vector.scalar_tensor_tensor(
                    t8[:], q8[:], -1.0, iv[:], op0=ALU.mult, op1=ALU.mult)
                nc.vector.tensor_scalar_add(t8[:], t8[:], 2.0)
                nc.vector.tensor_mul(iv[:], iv[:], t8[:])
            rslt[b] = iv

        for b in range(B2):
            dd = pB.tile([128, 2, 6], F32, tag=f"dots{b}", name=f"dots{b}")
            for c in range(2):
                for pi, (sa, sb) in enumerate(PAIRS):
                    scr = pBs.tile([128, D], F32, tag="scr")
                    nc.vector.scalar_tensor_tensor(
                        scr[:], slt[b, sa, c][:], 1.0, slt[b, sb, c][:],
                        op0=ALU.bypass, op1=ALU.mult,
                        accum_out=dd[:, c, pi:pi + 1])
            nt = pB.tile([128, 2, 4], F32, tag=f"nt{b}", name=f"nt{b}")
            for c in range(2):
                nc.sync.dma_start(
                    nt[:, c, :],
                    bass.AP(tensor=noise_in, offset=(b * NW + 128 * c) * 4,
                            ap=[[4, 128], [1, 4]]))
            nc.vector.tensor_scalar_mul(nt[:], nt[:], 0.1)
            sc = pB.tile([128, 2, 4], F32, tag=f"scsc{b}", name=f"scsc{b}")
            rsl = rslt[b]
            for s4 in range(4):
                others = [s for s in range(4) if s != s4]
                acc = pBs.tile([128, 2], F32, tag="acc")
                t2 = pBs.tile([128, 2], F32, tag="t2")
                o0 = others[0]
                nc.vector.tensor_mul(
                    acc[:], dd[:, :, PIDX[min(s4, o0), max(s4, o0)]],
                    rsl[:, :, o0])
                for o in others[1:]:
                    nc.vector.tensor_mul(
                        t2[:], dd[:, :, PIDX[min(s4, o), max(s4, o)]],
                        rsl[:, :, o])
                    nc.vector.tensor_add(acc[:], acc[:], t2[:])
                nc.vector.tensor_mul(acc[:], acc[:], rsl[:, :, s4])
                nc.vector.scalar_tensor_tensor(
                    sc[:, :, s4], acc[:], 0.25, nt[:, :, s4],
                    op0=ALU.mult, op1=ALU.add)
            if dbg:
                nc.sync.dma_start(dbg_sc[b], sc[:])

            # argmax over the 4 slots, first max wins:
            # dl = (1-e0) * (1 + (1-e1) * (2 - e2))
            mm = pBs.tile([128, 2], F32, tag="mm")
            m23 = pBs.tile([128, 2], F32, tag="m23")
            nc.vector.tensor_tensor(mm[:], sc[:, :, 0], sc[:, :, 1], op=ALU.max)
            nc.vector.tensor_tensor(m23[:], sc[:, :, 2], sc[:, :, 3], op=ALU.max)
            nc.vector.tensor_tensor(mm[:], mm[:], m23[:], op=ALU.max)
            e0 = pBs.tile([128, 2], F32, tag="e0")
            e1 = pBs.tile([128, 2], F32, tag="e1")
            e2 = pBs.tile([128, 2], F32, tag="e2")
            nc.vector.tensor_tensor(e0[:], sc[:, :, 0], mm[:], op=ALU.is_equal)
            nc.vector.tensor_tensor(e1[:], sc[:, :, 1], mm[:], op=ALU.is_equal)
            nc.vector.tensor_tensor(e2[:], sc[:, :, 2], mm[:], op=ALU.is_equal)
            u2 = pBs.tile([128, 2], F32, tag="u2")
            nc.vector.tensor_scalar(
                u2[:], e2[:], -1.0, 2.0, op0=ALU.mult, op1=ALU.add)
            v1 = pBs.tile([128, 2], F32, tag="v1")
            nc.vector.scalar_tensor_tensor(
                v1[:], e1[:], -1.0, u2[:], op0=ALU.mult, op1=ALU.mult)
            u1 = pBs.tile([128, 2], F32, tag="u1")
            nc.vector.tensor_add(u1[:], v1[:], u2[:])
            nc.vector.tensor_scalar_add(u1[:], u1[:], 1.0)
            v0 = pBs.tile([128, 2], F32, tag="v0")
            nc.vector.scalar_tensor_tensor(
                v0[:], e0[:], -1.0, u1[:], op0=ALU.mult, op1=ALU.mult)
            dl = pBs.tile([128, 2], F32, tag="dl")
            nc.vector.tensor_add(dl[:], v0[:], u1[:])

            # dst token = winbase + 32*(dl>>1) + (dl&1)
            syt = pBs.tile([128, 2], F32, tag="syt")
            nc.vector.tensor_scalar(syt[:], dl[:], 2.0, None, op0=ALU.is_ge)
            sxt = pBs.tile([128, 2], F32, tag="sxt")
            nc.vector.scalar_tensor_tensor(
                sxt[:], syt[:], -2.0, dl[:], op0=ALU.mult, op1=ALU.add)
            dtb = pX.tile([128, 2], F32, tag=f"dt{b}")
            nc.vector.scalar_tensor_tensor(
                dtb[:], syt[:], 32.0, sxt[:], op0=ALU.mult, op1=ALU.add)
            nc.vector.tensor_add(dtb[:], dtb[:], wbt[:])
            dt[b] = dtb

            # src tokens [128, 2, 3]
            stb_ = pX.tile([128, 2, 3], F32, tag=f"st{b}")
            for c in range(2):
                ge = pBs.tile([128, 3], F32, tag="ge")
                nc.vector.tensor_scalar(
                    ge[:], jb[:], dl[:, c:c + 1], None, op0=ALU.is_ge)
                sl3 = pBs.tile([128, 3], F32, tag="sl3")
                nc.vector.tensor_add(sl3[:], ge[:], jb[:])
                sy2 = pBs.tile([128, 3], F32, tag="sy2")
                nc.vector.tensor_scalar(
                    sy2[:], sl3[:], 2.0, None, op0=ALU.is_ge)
                sx2 = pBs.tile([128, 3], F32, tag="sx2")
                nc.vector.scalar_tensor_tensor(
                    sx2[:], sy2[:], -2.0, sl3[:], op0=ALU.mult, op1=ALU.add)
                nc.vector.scalar_tensor_tensor(
                    stb_[:, c, :], sy2[:], 32.0, sx2[:],
                    op0=ALU.mult, op1=ALU.add)
                nc.vector.tensor_scalar_add(
                    stb_[:, c, :], stb_[:, c, :], wbt[:, c:c + 1])
            st[b] = stb_

            # ---- R: token rows + idx staging (interleaved with B) ----
            srow = pX.tile([1, NS], F32, tag=f"srow_s{b}")
            for c in range(2):
                nc.sync.dma_start(
                    srow[:, 384 * c:384 * (c + 1)].rearrange(
                        "a (p j) -> a p j", p=128),
                    st[b][:, c, :])
            srow_s[b] = srow
            drow = pX.tile([1, NW], F32, tag=f"drow_w{b}")
            for c in range(2):
                nc.sync.dma_start(
                    drow[:, 128 * c:128 * (c + 1)], dt[b][:, c:c + 1])
            drow_w[b] = drow
            # i-order idx rows for dma_gather (i = 128*chunk + p)
            sgrow = pR.tile([1, NS], F32, tag="sgrow")
            for h2 in range(2):
                for j3 in range(3):
                    nc.sync.dma_start(
                        sgrow[:, 384 * h2 + 128 * j3:384 * h2 + 128 * (j3 + 1)]
                        .rearrange("a (k o) -> a k o", o=1),
                        st[b][:, h2, j3:j3 + 1])
            sgi = pR.tile([1, NS], I16, tag="sgi")
            nc.vector.tensor_copy(sgi[:], sgrow[:])
            nc.sync.dma_start(sidx_d[b][None, :], sgi[:])
            sw = pX.tile([128, 48], I16, tag=f"sidx_w{b}")
            for g8 in range(8):
                nc.sync.dma_start(sw[16 * g8:16 * (g8 + 1), :], bass.AP(
                    tensor=sidx_d[b], offset=0, ap=[[1, 16], [16, 48]]))
            sidx_w[b] = sw
            dgi = pR.tile([1, NW], I16, tag="dgi")
            nc.vector.tensor_copy(dgi[:], drow[:])
            nc.sync.dma_start(didx_d[b][None, :], dgi[:])
            dw = pX.tile([128, 16], I16, tag=f"didx_w{b}")
            for g8 in range(8):
                nc.sync.dma_start(dw[16 * g8:16 * (g8 + 1), :], bass.AP(
                    tensor=didx_d[b], offset=0, ap=[[1, 16], [16, 16]]))
            didx_w[b] = dw

            # xn_d rows written straight from normalized slot tiles
            for s4 in range(4):
                for c in range(2):
                    sln = pBs.tile([128, D], F32, tag="sln")
                    nc.vector.tensor_scalar_mul(
                        sln[:], slt[b, s4, c][:], rslt[b][:, c, s4:s4 + 1])
                    nc.sync.dma_start(slot_ap_xn(b, s4, c), sln[:])

    # ---- C: gather xn rows, transpose, sim, best + exact stable rank ----
    with (
        tc.tile_pool(name="pC1", bufs=1) as pC,
        tc.tile_pool(name="pC2", bufs=2) as pC2,
    ):
        for b in range(B2):
            mscb = pX.tile([128, 6], F32, tag=f"msc{b}")
            bstb = pX.tile([128, 6], F32, tag=f"bst{b}")
            rnkb = pX.tile([128, 6], F32, tag=f"rnk{b}")
            xnd = pC.tile([128, 2, D], F32, tag=f"xnd{b}")
            nc.gpsimd.dma_gather(
                out_ap=xnd[:], in_ap=xn_d[b][:], idxs_ap=didx_w[b][:],
                num_idxs=NW, num_idxs_reg=NW, elem_size=D)
            xndT = pC.tile([128, DC, NW], F32, tag=f"xndT{b}")
            for c in range(2):
                for dc in range(DC):
                    pt = psumT.tile([128, 128], F32, tag="pt")
                    nc.tensor.transpose(
                        pt[:], xnd[:, c, 128 * dc:128 * (dc + 1)], ident[:])
                    if dc % 2 == 0:
                        nc.scalar.copy(
                            xndT[:, dc, 128 * c:128 * (c + 1)], pt[:])
                    else:
                        nc.vector.tensor_copy(
                            xndT[:, dc, 128 * c:128 * (c + 1)], pt[:])
            for half in range(2):
                xns = pC.tile([128, 3, D], F32, tag=f"xns{half}")
                nc.gpsimd.dma_gather(
                    out_ap=xns[:], in_ap=xn_d[b][:],
                    idxs_ap=sidx_w[b][:, 24 * half:24 * (half + 1)],
                    num_idxs=384, num_idxs_reg=384, elem_size=D)
                xnsT = pC.tile([128, 3, DC, 128], F32, tag=f"xnsT{half}")
                for c3 in range(3):
                    for dc in range(DC):
                        pt = psumT.tile([128, 128], F32, tag="pt")
                        nc.tensor.transpose(
                            pt[:], xns[:, c3, 128 * dc:128 * (dc + 1)],
                            ident[:])
                        if dc % 2 == 0:
                            nc.scalar.copy(xnsT[:, c3, dc, :], pt[:])
                        else:
                            nc.vector.tensor_copy(xnsT[:, c3, dc, :], pt[:])
                for c3 in range(3):
                    cc6 = 3 * half + c3
                    ps = psumS.tile([128, NW], F32, tag="ps")
                    for dc in range(DC):
                        nc.tensor.matmul(
                            ps[:], xnsT[:, c3, dc, :], xndT[:, dc, :],
                            start=(dc == 0), stop=(dc == DC - 1))
                    nc.vector.reduce_max(
                        mscb[:, cc6:cc6 + 1], ps[:], axis=AX.X)
                    eqt = pC.tile([128, NW], F32, tag="eqt")
                    nc.vector.tensor_scalar(
                        eqt[:], ps[:], mscb[:, cc6:cc6 + 1], None,
                        op0=ALU.is_equal)
                    mskt = pC.tile([128, NW], F32, tag="mskt")
                    nc.vector.scalar_tensor_tensor(
                        mskt[:], eqt[:], -BIG, iotaBIG[:],
                        op0=ALU.mult, op1=ALU.add)
                    nc.vector.tensor_reduce(
                        bstb[:, cc6:cc6 + 1], mskt[:], axis=AX.X, op=ALU.min)
            msc[b], bst[b], rnk[b] = mscb, bstb, rnkb

        for b in range(B2):
            # maxsim broadcast via DRAM bounce (s order)
            for cc6 in range(6):
                c, j = cc6 // 3, cc6 % 3
                nc.sync.dma_start(
                    bass.AP(tensor=mrow_d[b], offset=384 * c + j,
                            ap=[[3, 128], [1, 1]]),
                    msc[b][:, cc6:cc6 + 1])
            mbc = pC2.tile([128, NS], F32, tag="mbc")
            nc.sync.dma_start(
                mbc[:],
                bass.AP(tensor=mrow_d[b], offset=0, ap=[[0, 128], [1, NS]]))

            # exact stable rank
            gcnt = pC2.tile([128, 1], F32, tag="gcnt")
            ecnt = pC2.tile([128, 1], F32, tag="ecnt")
            for lh in range(2):
                ltm = pC2.tile([128, 3, NS], F32, tag="ltm")
                for c3 in range(3):
                    nc.sync.dma_start(ltm[:, c3, :], ltmask[3 * lh + c3])
                for c3 in range(3):
                    cc6 = 3 * lh + c3
                    sc1 = pC2.tile([128, NS], F32, tag="sc1")
                    nc.vector.scalar_tensor_tensor(
                        sc1[:], mbc[:], msc[b][:, cc6:cc6 + 1], mbc[:],
                        op0=ALU.is_gt, op1=ALU.bypass, accum_out=gcnt[:])
                    sc2 = pC2.tile([128, NS], F32, tag="sc2")
                    nc.vector.scalar_tensor_tensor(
                        sc2[:], mbc[:], msc[b][:, cc6:cc6 + 1], ltm[:, c3, :],
                        op0=ALU.is_equal, op1=ALU.mult, accum_out=ecnt[:])
                    nc.vector.tensor_add(
                        rnk[b][:, cc6:cc6 + 1], gcnt[:], ecnt[:])

            # best + rank rows in s order
            browb = pX.tile([1, NS], F32, tag=f"brow{b}")
            rrowb = pX.tile([1, NS], F32, tag=f"rrow{b}")
            for cc6 in range(6):
                c, j = cc6 // 3, cc6 % 3
                dst_b = browb[:].rearrange(
                    "a (c p j) -> a p c j", c=2, p=128)[:, :, c, j]
                nc.sync.dma_start(dst_b, bst[b][:, cc6:cc6 + 1])
                dst_r = rrowb[:].rearrange(
                    "a (c p j) -> a p c j", c=2, p=128)[:, :, c, j]
                nc.sync.dma_start(dst_r, rnk[b][:, cc6:cc6 + 1])
            brow[b], rrow[b] = browb, rrowb
            if dbg:
                nc.sync.dma_start(dbg_ms[b][None, :], mbc[0:1, :])
                nc.sync.dma_start(dbg_best[b][None, :], browb[:])
                nc.sync.dma_start(dbg_rank[b][None, :], rrowb[:])

    # ---- K: masks, prefix sum, one-hot compaction, g construction ----
    with tc.tile_pool(name="pK", bufs=2) as pK:
        for b in range(B2):
            kpm = pK.tile([1, NS], F32, tag="kpm")
            nc.vector.tensor_scalar(
                kpm[:], rrow[b][:], 512.0, None, op0=ALU.is_ge)
            kex = pK.tile([1, NS], F32, tag="kex")
            nc.vector.tensor_tensor_scan(
                kex[:], kpm[:], kpm[:], 0.0, op0=ALU.add, op1=ALU.bypass)
            nc.vector.tensor_sub(kex[:], kex[:], kpm[:])
            # v_src = best + kpm*(256 + kex - best)
            tq = pK.tile([1, NS], F32, tag="tmp768")
            nc.vector.tensor_sub(tq[:], kex[:], brow[b][:])
            nc.vector.scalar_tensor_tensor(
                tq[:], tq[:], 256.0, kpm[:], op0=ALU.add, op1=ALU.mult)
            vsr = pK.tile([1, NS], F32, tag="vsr")
            nc.vector.tensor_add(vsr[:], tq[:], brow[b][:])

            # f32 token row [dst_w | src_s] and value row [witer | vsr]
            trow = pK.tile([1, N], F32, tag="trow")
            nc.scalar.copy(trow[:, :NW], drow_w[b][:])
            nc.scalar.copy(trow[:, NW:], srow_s[b][:])
            vrow = pK.tile([1, N], F32, tag="vrow")
            nc.scalar.copy(vrow[:, :NW], witer[:])
            nc.scalar.copy(vrow[:, NW:], vsr[:])
            # masked keep-rank row: kpm*(kex+1) - 1  (pruned -> -1)
            mk = pK.tile([1, NS], F32, tag="mk")
            nc.vector.scalar_tensor_tensor(
                mk[:], kex[:], 1.0, kpm[:], op0=ALU.add, op1=ALU.mult)
            nc.vector.tensor_scalar_add(mk[:], mk[:], -1.0)
            nc.sync.dma_start(mk_d[b][None, :], mk[:])
            nc.sync.dma_start(tok_d[b][None, :], trow[:])
            nc.sync.dma_start(val_d[b][None, :], vrow[:])
            mkb = pK.tile([128, NS], F32, tag="mkb")
            nc.sync.dma_start(mkb[:], bass.AP(
                tensor=mk_d[b], offset=0, ap=[[0, 128], [1, NS]]))
            stb = pK.tile([128, NS], F32, tag="stb")
            nc.sync.dma_start(stb[:], bass.AP(
                tensor=tok_d[b], offset=NW, ap=[[0, 128], [1, NS]]))
            tkb = pK.tile([128, N], F32, tag="tkb")
            nc.sync.dma_start(tkb[:], bass.AP(
                tensor=tok_d[b], offset=0, ap=[[0, 128], [1, N]]))
            vlb = pK.tile([128, N], F32, tag="vlb")
            nc.sync.dma_start(vlb[:], bass.AP(
                tensor=val_d[b], offset=0, ap=[[0, 128], [1, N]]))

            krow = pK.tile([1, T], F32, tag="krow")
            nc.scalar.copy(krow[:, :NW], drow_w[b][:])
            eqk = pK.tile([128, NS], F32, tag="eqk")
            for c2 in range(2):
                rtg = pK.tile([128, 1], F32, tag="rtg")
                nc.vector.tensor_scalar_add(rtg[:], io128[:], float(128 * c2))
                kv = pK.tile([128, 1], F32, tag="kv")
                nc.vector.scalar_tensor_tensor(
                    eqk[:], mkb[:], rtg[:], stb[:],
                    op0=ALU.is_equal, op1=ALU.mult, accum_out=kv[:])
                seg = krow[:, NW + 128 * c2:NW + 128 * (c2 + 1)]
                nc.sync.dma_start(seg.rearrange("a (k o) -> a k o", o=1), kv[:])

            grow = pK.tile([1, N], F32, tag="grow")
            eqg = pK.tile([128, N], F32, tag="eqg")
            for c8 in range(8):
                ttg = pK.tile([128, 1], F32, tag="ttg")
                nc.vector.tensor_scalar_add(ttg[:], io128[:], float(128 * c8))
                gv = pK.tile([128, 1], F32, tag="gv")
                nc.vector.scalar_tensor_tensor(
                    eqg[:], tkb[:], ttg[:], vlb[:],
                    op0=ALU.is_equal, op1=ALU.mult, accum_out=gv[:])
                seg = grow[:, 128 * c8:128 * (c8 + 1)]
                nc.sync.dma_start(seg.rearrange("a (k o) -> a k o", o=1), gv[:])

            # int16 gather-idx staging
            ki16 = pK.tile([1, T], I16, tag="ki16")
            nc.vector.tensor_copy(ki16[:], krow[:])
            nc.sync.dma_start(kidx_d[b][None, :], ki16[:])
            gi16 = pK.tile([1, N], I16, tag="gi16")
            nc.vector.tensor_copy(gi16[:], grow[:])
            nc.sync.dma_start(gidx_d[b][None, :], gi16[:])
            if dbg:
                ki32 = pK.tile([1, T], I32, tag="ki32")
                nc.vector.tensor_copy(ki32[:], krow[:])
                nc.sync.dma_start(keep_d[b][None, :], ki32[:])
                gi32 = pK.tile([1, N], I32, tag="gi32")
                nc.vector.tensor_copy(gi32[:], grow[:])
                nc.sync.dma_start(g_d[b][None, :], gi32[:])
    ipools.close()

    if stop_after == "index":
        ctx.close()
        return dict(nc=nc)

    # =================== DiT block ===================
    build_block(nc, tc, ctx, cfg, dict(
        identb=identb, ident=ident, x_in=x_in,
        wqk3=wqk3, wvt=wvt, wp3=wp3, wf13=wf13, wf23=wf23,
        bqk=bqk, bv_row=bv_row, bproj_row=bproj_row, bfc1=bfc1,
        bfc2_row=bfc2_row, bo_d=bo_d, kidx_d=kidx_d, aden_d=aden_d,
        aden_raw=aden_raw, gidx_d=gidx_d, out=out, dbg=dbg))

    ctx.close()
    return dict(nc=nc)


def layer_norm(nc, pool, small, xin, yout, eps=1e-6):
    """Row LN: yout = (x - mu) * rsqrt(var + eps). xin fp32 [128, D]."""
    mu = small.tile([128, 1], F32, tag="ln_mu")
    nc.vector.tensor_reduce(mu[:], xin, axis=AX.X, op=ALU.add)
    nc.vector.tensor_scalar_mul(mu[:], mu[:], 1.0 / D)
    xc = pool.tile([128, D], F32, tag="ln_xc")
    nc.vector.tensor_scalar(xc[:], xin, mu[:], None, op0=ALU.subtract)
    sq = pool.tile([128, D], F32, tag="ln_sq")
    var = small.tile([128, 1], F32, tag="ln_var")
    nc.vector.scalar_tensor_tensor(
        sq[:], xc[:], 1.0, xc[:], op0=ALU.bypass, op1=ALU.mult, accum_out=var[:])
    nc.vector.tensor_scalar(
        var[:], var[:], 1.0 / D, eps, op0=ALU.mult, op1=ALU.add)
    sd = small.tile([128, 1], F32, tag="ln_sd")
    nc.scalar.activation(sd[:], var[:], AF.Sqrt)
    rstd = small.tile([128, 1], F32, tag="ln_rstd")
    nc.vector.reciprocal(rstd[:], sd[:])
    nc.vector.tensor_scalar_mul(yout, xc[:], rstd[:])


def build_block(nc, tc, ctx, cfg, env):
    BD = BF16
    stop_after = cfg.get("stop_after", None)
    identb = env["identb"]
    x_in = env["x_in"]
    wqk3, wvt = env["wqk3"], env["wvt"]
    wp3, wf13, wf23 = env["wp3"], env["wf13"], env["wf23"]
    bo_d, kidx_d = env["bo_d"], env["kidx_d"]
    aden_d, aden_raw = env["aden_d"], env["aden_raw"]
    gidx_d, out = env["gidx_d"], env["out"]

    # PSUM banks: psA 2 + psB 2 on ctx; psT (LN phases) and psPO/psD
    # (attention) are scoped so attention gets 2 bufs each: max 8 banks
    bcp = ctx.enter_context(tc.tile_pool(name="bcp", bufs=1))
    psA = ctx.enter_context(tc.tile_pool(name="psA", bufs=2, space="PSUM"))
    psB = ctx.enter_context(tc.tile_pool(name="psB", bufs=2, space="PSUM"))
    small = ctx.enter_context(tc.tile_pool(name="bsmall", bufs=4))

    bvb = bcp.tile([128, D], BD)
    nc.sync.dma_start(bvb[:], bass.AP(
        tensor=env["bv_row"], offset=0, ap=[[0, 128], [1, D]]))
    bpb = bcp.tile([128, D], BD)
    nc.sync.dma_start(bpb[:], bass.AP(
        tensor=env["bproj_row"], offset=0, ap=[[0, 128], [1, D]]))
    bf2b = bcp.tile([128, D], BD)
    nc.sync.dma_start(bf2b[:], bass.AP(
        tensor=env["bfc2_row"], offset=0, ap=[[0, 128], [1, D]]))
    bqkt = bcp.tile([128, 18], F32)
    nc.sync.dma_start(bqkt[:], env["bqk"][:, :])
    bf1t = bcp.tile([128, 36], F32)
    nc.sync.dma_start(bf1t[:], env["bfc1"][:, :])
    # all-ones stationary column (softmax denominators via matmul)
    onesb = bcp.tile([128, 1], BD)
    nc.vector.tensor_scalar(onesb[:], bqkt[:, 0:1], 0.0, 1.0,
                            op0=ALU.mult, op1=ALU.add)
    kidx_t = [None, None]
    for b in range(B2):
        kpt = bcp.tile([128, 32], I16, tag=f"kidx2_{b}", name=f"kidx2_{b}")
        for g8 in range(8):
            nc.sync.dma_start(kpt[16 * g8:16 * (g8 + 1), :], bass.AP(
                tensor=kidx_d[b], offset=0, ap=[[1, 16], [16, 32]]))
        kidx_t[b] = kpt

    def transpose_to(psT, dst_sl, y, dc):
        pt = psT.tile([128, 128], BD, tag="bt")
        nc.tensor.transpose(pt[:], y[:, 128 * dc:128 * (dc + 1)], identb[:])
        if dc % 2 == 0:
            nc.scalar.copy(dst_sl, pt[:])
        else:
            nc.vector.tensor_copy(dst_sl, pt[:])

    # x1 accumulator f32; LN1 gathers write straight into it (x rows),
    # proj then adds b_proj + attention delta, MLP adds the rest.
    p_x1 = ctx.enter_context(tc.tile_pool(name="p_x1", bufs=1))
    x1 = p_x1.tile([128, B2, TC, D], F32)

    with tc.tile_pool(name="p_ot", bufs=1) as p_ot:
        OT = [p_ot.tile([128, DC, T], BD, tag=f"OT{b}", name=f"OT{b}")
              for b in range(B2)]
        with tc.tile_pool(name="p_yt", bufs=1) as p_yt:
            YT = p_yt.tile([128, DC, 2 * T], BD)
            # ---- LN1 -> YT ----
            with (
                tc.tile_pool(name="p_ln1", bufs=2) as p_ln,
                tc.tile_pool(name="psT1", bufs=2, space="PSUM") as psT1,
            ):
                for ct in range(8):
                    b, c4 = divmod(ct, TC)
                    nc.gpsimd.dma_gather(
                        out_ap=x1[:, b, c4:c4 + 1, :], in_ap=x_in[b],
                        idxs_ap=kidx_t[b][:, 8 * c4:8 * (c4 + 1)],
                        num_idxs=128, num_idxs_reg=128, elem_size=D)
                    y = p_ln.tile([128, D], BD, tag="y")
                    layer_norm(nc, p_ln, small, x1[:, b, c4, :], y[:])
                    for dc in range(DC):
                        transpose_to(psT1, YT[:, dc, 128 * ct:128 * (ct + 1)],
                                     y, dc)

            with tc.tile_pool(name="p_v", bufs=1) as p_v:
                V = p_v.tile([128, 2 * TC, D], BD)
                # ---- V (token-major) ----
                for ct in range(8):
                    for ns in range(3):
                        pv = psB.tile([128, 384], F32, tag="b")
                        for dc in range(DC):
                            nc.tensor.matmul(
                                pv[:], YT[:, dc, 128 * ct:128 * (ct + 1)],
                                wvt[:, dc, 384 * ns:384 * (ns + 1)],
                                start=(dc == 0), stop=(dc == DC - 1))
                        nc.vector.scalar_tensor_tensor(
                            V[:, ct, 384 * ns:384 * (ns + 1)], pv[:], 1.0,
                            bvb[:, 384 * ns:384 * (ns + 1)],
                            op0=ALU.bypass, op1=ALU.add)

                if stop_after == "v":
                    return
                # ---- attention: transposed scores, no max-subtract,
                #      denominators via ones-matmul, normalize at oh ----
                with (
                    tc.tile_pool(name="p_wqk2", bufs=1) as p_wqk2,
                    tc.tile_pool(name="psPO", bufs=2, space="PSUM") as psPO,
                    tc.tile_pool(name="psD", bufs=2, space="PSUM") as psD,
                ):
                    for hg in range(2):
                        wqh = p_wqk2.tile([128, DC, DC, 128], BD, tag="wqh")
                        for mcl in range(DC):
                            mc = DC * hg + mcl
                            nc.sync.dma_start(
                                wqh[:, mcl, :, :],
                                wqk3[:, mc * D:(mc + 1) * D])
                        for b in range(B2):
                            with (
                                tc.tile_pool(name="p_qk", bufs=1) as p_qk,
                                tc.tile_pool(name="p_att", bufs=3) as p_att,
                                tc.tile_pool(name="p_et", bufs=2) as p_et,
                            ):
                                QKT = p_qk.tile([128, DC, T], BD,
                                                name=f"QKT{b}{hg}")
                                for mcl in range(DC):
                                    mc = DC * hg + mcl
                                    pq = psA.tile([128, T], F32, tag="a")
                                    for dc in range(DC):
                                        nc.tensor.matmul(
                                            pq[:], wqh[:, mcl, dc, :],
                                            YT[:, dc, T * b:T * (b + 1)],
                                            start=(dc == 0),
                                            stop=(dc == DC - 1))
                                    if mcl % 2 == 0:
                                        nc.scalar.activation(
                                            QKT[:, mcl, :], pq[:],
                                            AF.Identity,
                                            bias=bqkt[:, mc:mc + 1])
                                    else:
                                        nc.vector.tensor_scalar(
                                            QKT[:, mcl, :], pq[:],
                                            bqkt[:, mc:mc + 1], None,
                                            op0=ALU.add)
                                for hl in range(8):
                                    h = 8 * hg + hl
                                    qh = p_att.tile([DH, T], BD, tag="qh")
                                    kh = p_att.tile([DH, T], BD, tag="kh")
                                    for (dst, base) in ((qh, DH * hl),
                                                        (kh, 576 + DH * hl)):
                                        r0 = base
                                        while r0 < base + DH:
                                            mcl, p0 = divmod(r0, 128)
                                            take = min(128 - p0,
                                                       base + DH - r0)
                                            nc.sync.dma_start(
                                                dst[r0 - base:
                                                    r0 - base + take, :],
                                                QKT[p0:p0 + take, mcl, :])
                                            r0 += take
                                    # E^T chunks from transposed scores
                                    ET = p_et.tile([128, TC, T], BD, tag="ET")
                                    for kc in range(TC):
                                        ps = psA.tile([128, T], F32, tag="a")
                                        nc.tensor.matmul(
                                            ps[:],
                                            kh[:, 128 * kc:128 * (kc + 1)],
                                            qh[:], start=True, stop=True)
                                        nc.scalar.activation(
                                            ET[:, kc, :], ps[:], AF.Exp,
                                            scale=RSQ_DH)
                                    # o^T (unnormalized) and denominators
                                    po = psPO.tile([DH, T], F32, tag="po")
                                    pden = psD.tile([1, T], F32, tag="pden")
                                    for kc in range(TC):
                                        nc.tensor.matmul(
                                            po[:],
                                            V[:, TC * b + kc,
                                              DH * h:DH * (h + 1)],
                                            ET[:, kc, :],
                                            start=(kc == 0),
                                            stop=(kc == TC - 1))
                                        nc.tensor.matmul(
                                            pden[:], onesb[:], ET[:, kc, :],
                                            start=(kc == 0),
                                            stop=(kc == TC - 1))
                                    # reciprocal via [32,16] DRAM reshape
                                    # ([1,512] DVE recip is ~4cyc/elem serial)
                                    rraw = small.tile([1, T], F32, tag="rraw")
                                    nc.scalar.copy(rraw[:], pden[:])
                                    nc.sync.dma_start(
                                        aden_raw[b, h][None, :], rraw[:])
                                    r32 = small.tile([32, 16], F32, tag="r32")
                                    nc.sync.dma_start(r32[:], bass.AP(
                                        tensor=aden_raw,
                                        offset=(b * H + h) * T,
                                        ap=[[16, 32], [1, 16]]))
                                    rd32 = small.tile([32, 16], F32,
                                                      tag="rd32")
                                    nc.vector.reciprocal(rd32[:], r32[:])
                                    rdb = small.tile([32, 16], BD, tag="rdb")
                                    nc.vector.tensor_copy(rdb[:], rd32[:])
                                    nc.sync.dma_start(
                                        bass.AP(tensor=aden_d,
                                                offset=(b * H + h) * T,
                                                ap=[[16, 32], [1, 16]]),
                                        rdb[:])
                                    rbc = p_att.tile([DH, T], BD, tag="rbc")
                                    nc.sync.dma_start(rbc[:], bass.AP(
                                        tensor=aden_d, offset=(b * H + h) * T,
                                        ap=[[0, DH], [1, T]]))
                                    oh = p_att.tile([DH, T], BD, tag="oh")
                                    nc.vector.tensor_tensor(
                                        oh[:], po[:], rbc[:], op=ALU.mult)
                                    r0 = DH * h
                                    while r0 < DH * (h + 1):
                                        dc, p0 = divmod(r0, 128)
                                        take = min(128 - p0, DH * (h + 1) - r0)
                                        nc.sync.dma_start(
                                            OT[b][p0:p0 + take, dc, :],
                                            oh[r0 - DH * h:
                                               r0 - DH * h + take, :])
                                        r0 += take

        # ---- proj + residual -> x1, full 9-dc PSUM accumulation ----
        with tc.tile_pool(name="p_wp", bufs=1) as p_wp:
            wpt = p_wp.tile([128, DC, D], BD)
            for dc in range(DC):
                nc.sync.dma_start(wpt[:, dc, :], wp3[:, dc * D:(dc + 1) * D])
            for b in range(B2):
                for c4 in range(TC):
                    nc.vector.tensor_add(
                        x1[:, b, c4, :], x1[:, b, c4, :], bpb[:])
                    for ns in range(3):
                        pp = psB.tile([128, 384], F32, tag="b")
                        for dc in range(DC):
                            nc.tensor.matmul(
                                pp[:], OT[b][:, dc, 128 * c4:128 * (c4 + 1)],
                                wpt[:, dc, 384 * ns:384 * (ns + 1)],
                                start=(dc == 0), stop=(dc == DC - 1))
                        sl = x1[:, b, c4, 384 * ns:384 * (ns + 1)]
                        nc.vector.scalar_tensor_tensor(
                            sl, pp[:], 1.0, sl, op0=ALU.bypass, op1=ALU.add)

    if stop_after == "attn":
        return
    # ---- LN2 + MLP (per-sample supergroups of 18) + inline recover ----
    with tc.tile_pool(name="p_y2", bufs=1) as p_y2:
        Y2T = p_y2.tile([128, DC, 2 * T], BD)
        with (
            tc.tile_pool(name="p_ln2", bufs=2) as p_ln,
            tc.tile_pool(name="psT2", bufs=2, space="PSUM") as psT2,
        ):
            for ct in range(8):
                b, c4 = divmod(ct, TC)
                y = p_ln.tile([128, D], BD, tag="y")
                layer_norm(nc, p_ln, small, x1[:, b, c4, :], y[:])
                for dc in range(DC):
                    transpose_to(psT2, Y2T[:, dc, 128 * ct:128 * (ct + 1)],
                                 y, dc)
                nc.vector.tensor_add(
                    x1[:, b, c4, :], x1[:, b, c4, :], bf2b[:])

        def emit_recover(rb):
            # recover sample rb: gather block rows back to full token order,
            # split in halves so the out-write of half 0 overlaps half 1
            with tc.tile_pool(name="recp", bufs=2) as recp:
                gw = recp.tile([128, 64], I16, tag="gw")
                for g8 in range(8):
                    nc.sync.dma_start(gw[16 * g8:16 * (g8 + 1), :], bass.AP(
                        tensor=gidx_d[rb], offset=0, ap=[[1, 16], [16, 64]]))
                for hf in range(2):
                    og = recp.tile([128, 4, D], BD, tag="og")
                    nc.gpsimd.dma_gather(
                        out_ap=og[:], in_ap=bo_d[rb][:],
                        idxs_ap=gw[:, 32 * hf:32 * (hf + 1)],
                        num_idxs=T, num_idxs_reg=T, elem_size=D)
                    ogf = recp.tile([128, 4, D], F32, tag="ogf")
                    nc.vector.tensor_copy(ogf[:], og[:])
                    nc.sync.dma_start(
                        bass.AP(tensor=out, offset=(rb * N + hf * T) * D,
                                ap=[[D, 128], [128 * D, 4], [1, D]]),
                        ogf[:])

        for b in range(B2):
            for sg in range(2):
                with tc.tile_pool(name="p_ht", bufs=1) as p_ht:
                    HT = p_ht.tile([128, 18, T], BD, name=f"HT{b}{sg}")
                    with tc.tile_pool(name="p_wf1", bufs=3) as p_wf1:
                        for k18 in range(18):
                            mf = 18 * sg + k18
                            wt = p_wf1.tile([128, DC, 128], BD, tag="wf1")
                            nc.sync.dma_start(wt[:], wf13[mf])
                            pf = psA.tile([128, T], F32, tag="a")
                            for dc in range(DC):
                                nc.tensor.matmul(
                                    pf[:], wt[:, dc, :],
                                    Y2T[:, dc, T * b:T * (b + 1)],
                                    start=(dc == 0), stop=(dc == DC - 1))
                            nc.scalar.activation(
                                HT[:, k18, :], pf[:],
                                AF.Gelu_apprx_tanh, bias=bf1t[:, mf:mf + 1])
                    with tc.tile_pool(name="p_wf2", bufs=1) as p_wf2:
                        wf2 = [p_wf2.tile([128, D], BD, tag=f"wf2_{i}",
                                          name=f"wf2t{b}{sg}{i}")
                               for i in range(18)]
                        for i in range(18):
                            nc.sync.dma_start(wf2[i][:], wf23[18 * sg + i])
                        for c4 in range(TC):
                            for ns in range(3):
                                pg = psB.tile([128, 384], F32, tag="b")
                                for i in range(18):
                                    nc.tensor.matmul(
                                        pg[:],
                                        HT[:, i, 128 * c4:128 * (c4 + 1)],
                                        wf2[i][:, 384 * ns:384 * (ns + 1)],
                                        start=(i == 0), stop=(i == 17))
                                sl = x1[:, b, c4, 384 * ns:384 * (ns + 1)]
                                nc.vector.scalar_tensor_tensor(
                                    sl, pg[:], 1.0, sl,
                                    op0=ALU.bypass, op1=ALU.add)

            # ---- write block output rows (DRAM row = 128*c4 + p) ----
            for c4 in range(TC):
                xob = small.tile([128, D], BD, tag="xob")
                nc.vector.tensor_copy(xob[:], x1[:, b, c4, :])
                nc.sync.dma_start(
                    bass.AP(tensor=bo_d[b], offset=c4 * 128 * D,
                            ap=[[D, 128], [1, D]]),
                    xob[:])
            emit_recover(b)


# ======================================================================
# kernel() entry point: full inputs -> full output on 8 NeuronCores
# ======================================================================

_MODULE_CACHE = {}


def _get_module(block_dtype_name):
    # block_dtype_name kept for interface compat; the block is always bf16
    if "bf16" not in _MODULE_CACHE:
        from concourse import bacc
        nc = bacc.Bacc(None, target_bir_lowering=False)
        build(nc, {})
        nc.compile()
        _MODULE_CACHE["bf16"] = nc
    return _MODULE_CACHE["bf16"]


def kernel(x, noise, ln1_g, ln1_b, ln2_g, ln2_b, w_qkv, b_qkv, w_proj, b_proj,
           w_fc1, b_fc1, w_fc2, b_fc2, block_dtype="f32r", **run_kw):
    from concourse import bass_utils

    x = np.ascontiguousarray(np.asarray(x, np.float32))
    noise = np.ascontiguousarray(np.asarray(noise, np.float32))
    B = x.shape[0]
    n_cores = B // B2
    wt = retile_weights(
        dict(ln1_g=ln1_g, ln1_b=ln1_b, ln2_g=ln2_g, ln2_b=ln2_b,
             w_qkv=w_qkv, b_qkv=b_qkv, w_proj=w_proj, b_proj=b_proj,
             w_fc1=w_fc1, b_fc1=b_fc1, w_fc2=w_fc2, b_fc2=b_fc2))

    nc = _get_module(block_dtype)
    in_maps = []
    for c in range(n_cores):
        m = dict(x=x[B2 * c:B2 * (c + 1)], noise=noise[B2 * c:B2 * (c + 1)])
        m.update(wt)
        in_maps.append(m)
    res = bass_utils.run_bass_kernel_spmd(
        nc, in_maps, core_ids=list(range(n_cores)), **run_kw)
    out = np.concatenate([res.results[c]["out"] for c in range(n_cores)], axis=0)
    if run_kw.get("trace"):
        return out, res
    return out



# revision 46
# speedup vs baseline: 1.0331x; 1.0305x over previous
"""DiT-SiTo block kernel builder for one NeuronCore (2 samples per core).

Index conventions (per sample):
  tokens t in [0,1024); window w in [0,256); slot s4 in {0..3}
  src index s in [0,768): s = 3*w + j (reference order)
  window-chunk layout: w = 128*c + p  (c in {0,1}, p = partition)
  gathered src rows: (p, cc) with cc = 3*c + j  ->  s = 3*(128*c+p) + j
  keep positions r in [0,512): r < 256 -> dst of window w=r; else kept src
  block token chunks: chunk c holds positions r in [128c, 128c+128), p = r%128
"""

from contextlib import ExitStack

import numpy as np

import concourse.bass as bass
import concourse.mybir as mybir
import concourse.tile as tile
from concourse.bass import IndirectOffsetOnAxis
from concourse import library_config
from concourse.masks import make_identity

I16 = mybir.dt.int16
F32 = mybir.dt.float32
F32R = mybir.dt.float32r
BF16 = mybir.dt.bfloat16
I32 = mybir.dt.int32
AF = mybir.ActivationFunctionType
ALU = mybir.AluOpType
AX = mybir.AxisListType

B2 = 2
N = 1024
D = 1152
DC = D // 128          # 9
H = 16
DH = 72
NW = 256
NS = 768
T = 512
TC = T // 128          # 4
D4 = 4608
BIG = 1.0e4
RSQ_DH = float(1.0 / np.sqrt(DH))


def host_constants():
    w = np.arange(NW)
    winbase = (64 * (w >> 4) + 2 * (w & 15)).astype(np.float32)
    iota256 = np.arange(NW, dtype=np.float32)
    jrow = np.arange(3, dtype=np.float32)
    # ltmask[cc, p, jj] = 1.0 iff jj < s(p, cc);  cc = 3*c + j
    cc = np.arange(6)
    c, j = cc // 3, cc % 3
    s = 3 * (128 * c[:, None] + np.arange(128)[None, :]) + j[:, None]   # [6,128]
    ltm = (np.arange(NS)[None, None, :] < s[:, :, None]).astype(np.float32)
    return winbase, iota256, jrow, np.ascontiguousarray(ltm)


def to_bf16(a):
    import ml_dtypes
    return np.ascontiguousarray(np.asarray(a, np.float32).astype(
        ml_dtypes.bfloat16))


def retile_weights(inp):
    """Host-side: fold LN affine into the following matmul, retile weights.

    All block weights go out partition-major and bf16 so every DMA load is
    [128, big-contiguous] (fat descriptors):
      wqk3  [128, 18*DC*128]  (p, mc, dc, col)  stationary chunks
      wv3   [128, DC*1152]    (p, dc, col)      moving rows
      wp3   [128, DC*1152]    (p, dc, col)      moving rows
      wf13  [36, 128, DC*128] (mf, p, dc, col)  stationary chunks
      wf23  [36, 128, 1152]   (kk, p, col)      moving rows
    """
    f32 = np.float32
    g1, b1 = np.asarray(inp["ln1_g"], f32), np.asarray(inp["ln1_b"], f32)
    g2, b2 = np.asarray(inp["ln2_g"], f32), np.asarray(inp["ln2_b"], f32)
    wqkv = np.asarray(inp["w_qkv"], f32); bqkv = np.asarray(inp["b_qkv"], f32)
    wfc1 = np.asarray(inp["w_fc1"], f32); bfc1 = np.asarray(inp["b_fc1"], f32)
    wqkv_f = g1[:, None] * wqkv
    bqkv_f = bqkv + b1 @ wqkv
    wfc1_f = g2[:, None] * wfc1
    bfc1_f = bfc1 + b2 @ wfc1

    # column order: [q heads 0-7 | k heads 0-7 | q heads 8-15 | k heads 8-15]
    perm = np.concatenate([
        np.arange(576), D + np.arange(576),
        576 + np.arange(576), D + 576 + np.arange(576)])
    wqk = wqkv_f[:, perm]                                      # [1152, 2304]
    wqk3 = wqk.reshape(DC, 128, 18, 128).transpose(1, 2, 0, 3).reshape(
        128, 18 * DC * 128)
    wv = wqkv_f[:, 2 * D:]
    wv3 = wv.reshape(DC, 128, D).transpose(1, 0, 2).reshape(128, DC * D)
    wp = np.asarray(inp["w_proj"], f32)
    wp3 = wp.reshape(DC, 128, D).transpose(1, 0, 2).reshape(128, DC * D)
    wf13 = wfc1_f.reshape(DC, 128, 36, 128).transpose(2, 1, 0, 3).reshape(
        36, 128, DC * 128)
    wf23 = np.asarray(inp["w_fc2"], f32).reshape(36, 128, D)
    bqk = np.ascontiguousarray(bqkv_f[perm].reshape(18, 128).T)     # [128, 18]
    return dict(
        wqk3=to_bf16(wqk3), wv3=to_bf16(wv3), wp3=to_bf16(wp3),
        wf13=to_bf16(wf13), wf23=to_bf16(wf23),
        bqk=bqk.astype(f32),
        bv_row=to_bf16(bqkv_f[None, 2 * D:]),
        bproj_row=to_bf16(np.asarray(inp["b_proj"], f32)[None, :]),
        bfc1=np.ascontiguousarray(bfc1_f.reshape(36, 128).T).astype(f32),
        bfc2_row=to_bf16(np.asarray(inp["b_fc2"], f32)[None, :]),
    )


def make_in_map(x_pair, noise_pair, weights):
    m = dict(x=np.ascontiguousarray(x_pair, np.float32),
             noise=np.ascontiguousarray(noise_pair, np.float32))
    m.update(weights)
    return m


def newton_recip(nc, pool, x, tag, iters=2):
    """r ~= 1/x to fp32 accuracy. x: [p,1] tile slice."""
    p = x.shape[0]
    r = pool.tile([p, 1], F32, tag=tag + "_r")
    t = pool.tile([p, 1], F32, tag=tag + "_t")
    nc.vector.reciprocal(r[:], x[:])
    for _ in range(iters):
        nc.vector.scalar_tensor_tensor(
            t[:], x[:], -1.0, r[:], op0=ALU.mult, op1=ALU.mult)
        nc.vector.tensor_scalar_add(t[:], t[:], 2.0)
        nc.vector.tensor_mul(r[:], r[:], t[:])
    return r


def build(nc, cfg=None):
    cfg = dict(cfg or {})
    BD = BF16
    dbg = cfg.get("debug", False)
    stop_after = cfg.get("stop_after", None)   # "index" to skip the block

    x_in = nc.dram_tensor("x", (B2, N, D), F32, kind="ExternalInput")
    noise_in = nc.dram_tensor("noise", (B2, NW, 4), F32, kind="ExternalInput")
    wqk3 = nc.dram_tensor("wqk3", (128, 18 * DC * 128), BD, kind="ExternalInput")
    wv3 = nc.dram_tensor("wv3", (128, DC * D), BD, kind="ExternalInput")
    wp3 = nc.dram_tensor("wp3", (128, DC * D), BD, kind="ExternalInput")
    wf13 = nc.dram_tensor("wf13", (36, 128, DC * 128), BD, kind="ExternalInput")
    wf23 = nc.dram_tensor("wf23", (36, 128, D), BD, kind="ExternalInput")
    bqk = nc.dram_tensor("bqk", (128, 18), F32, kind="ExternalInput")
    bv_row = nc.dram_tensor("bv_row", (1, D), BD, kind="ExternalInput")
    bproj_row = nc.dram_tensor("bproj_row", (1, D), BD, kind="ExternalInput")
    bfc1 = nc.dram_tensor("bfc1", (128, 36), F32, kind="ExternalInput")
    bfc2_row = nc.dram_tensor("bfc2_row", (1, D), BD, kind="ExternalInput")

    out = nc.dram_tensor("out", (B2, N, D), F32, kind="ExternalOutput")
    aden_d = nc.dram_tensor("aden_d", (B2, H, T), BD, kind="Internal")
    aden_raw = nc.dram_tensor("aden_raw", (B2, H, T), F32, kind="Internal")

    wb_np, iota_np, jrow_np, ltm_np = host_constants()
    winbase = nc.inline_tensor(wb_np, name="winbase")
    iota128 = nc.inline_tensor(np.arange(128, dtype=np.float32), name="iota128")
    iota256 = nc.inline_tensor(iota_np, name="iota256")
    jrow = nc.inline_tensor(jrow_np, name="jrow")
    ltmask = nc.inline_tensor(ltm_np, name="ltmask")

    okind = "ExternalOutput" if dbg else "Internal"
    xn_d = [nc.dram_tensor(f"xn_d{b}", (N, D), F32, kind=okind) for b in range(B2)]
    ktmp_d = [nc.dram_tensor(f"ktmp_d{b}", (NS,), I32, kind="Internal")
              for b in range(B2)]
    g_d = [nc.dram_tensor(f"g_d{b}", (N,), I32, kind=okind) for b in range(B2)]
    keep_d = [nc.dram_tensor(f"keep_d{b}", (T,), I32, kind=okind)
              for b in range(B2)]
    bo_d = [nc.dram_tensor(f"bo_d{b}", (T, D), BD, kind=okind)
            for b in range(B2)]
    mrow_d = [nc.dram_tensor(f"mrow_d{b}", (NS,), F32, kind="Internal")
              for b in range(B2)]
    sidx_d = [nc.dram_tensor(f"sidx_d{b}", (NS,), I16, kind="Internal")
              for b in range(B2)]
    didx_d = [nc.dram_tensor(f"didx_d{b}", (NW,), I16, kind="Internal")
              for b in range(B2)]
    kidx_d = [nc.dram_tensor(f"kidx_d{b}", (T,), I16, kind="Internal")
              for b in range(B2)]
    gidx_d = [nc.dram_tensor(f"gidx_d{b}", (N,), I16, kind="Internal")
              for b in range(B2)]
    ninv_d = [nc.dram_tensor(f"ninv_d{b}", (N,), F32, kind="Internal")
              for b in range(B2)]
    mk_d = [nc.dram_tensor(f"mk_d{b}", (NS,), F32, kind="Internal")
            for b in range(B2)]
    tok_d = [nc.dram_tensor(f"tok_d{b}", (N,), F32, kind="Internal")
             for b in range(B2)]
    val_d = [nc.dram_tensor(f"val_d{b}", (N,), F32, kind="Internal")
             for b in range(B2)]
    kperm_d = [nc.dram_tensor(f"kperm_d{b}", (T,), I32, kind="Internal")
               for b in range(B2)]
    if dbg:
        dbg_sc = nc.dram_tensor("dbg_sc", (B2, 128, 2, 4), F32,
                                kind="ExternalOutput")
        dbg_ms = nc.dram_tensor("dbg_ms", (B2, NS), F32, kind="ExternalOutput")
        dbg_best = nc.dram_tensor("dbg_best", (B2, NS), F32, kind="ExternalOutput")
        dbg_rank = nc.dram_tensor("dbg_rank", (B2, NS), F32, kind="ExternalOutput")

    ctx = ExitStack()
    tc = ctx.enter_context(tile.TileContext(nc))

    consts = ctx.enter_context(tc.tile_pool(name="consts", bufs=1))
    nc.gpsimd.load_library(library_config.mlp)
    ident = consts.tile([128, 128], F32)
    io128 = consts.tile([128, 1], F32)
    nc.sync.dma_start(io128[:], bass.AP(
        tensor=iota128, offset=0, ap=[[1, 128], [1, 1]]))
    make_identity(nc, ident)
    if BD != F32:
        identb = consts.tile([128, 128], BD)
        nc.vector.tensor_copy(identb[:], ident[:])
    else:
        identb = ident
    iotaBIG = consts.tile([128, NW], F32)
    nc.sync.dma_start(iotaBIG[:], bass.AP(
        tensor=iota256, offset=0, ap=[[0, 128], [1, NW]]))
    nc.vector.tensor_scalar_add(iotaBIG[:], iotaBIG[:], BIG)
    jb = consts.tile([128, 3], F32)
    nc.sync.dma_start(jb[:], bass.AP(tensor=jrow, offset=0, ap=[[0, 128], [1, 3]]))
    wbt = consts.tile([128, 2], F32)
    for c in range(2):
        nc.sync.dma_start(
            wbt[:, c:c + 1],
            bass.AP(tensor=winbase, offset=128 * c, ap=[[1, 128], [1, 1]]))
    witer = consts.tile([1, NW], F32)
    nc.sync.dma_start(witer[:], iota256[None, :])

    # Preload the V weight during the (latency-bound) index phase; wqk is
    # too big to coexist with the index pools and loads at block start.
    wv_pool = ctx.enter_context(tc.tile_pool(name="wv_pool", bufs=1))
    wvt = wv_pool.tile([128, DC, D], BD)

    ipools = ExitStack()
    psumS = ipools.enter_context(tc.tile_pool(name="psumS", bufs=3, space="PSUM"))
    psumT = ipools.enter_context(tc.tile_pool(name="psumT", bufs=3, space="PSUM"))

    # ============ index pipeline (phase-major, samples interleaved) ============
    # cross-phase per-sample tiles live in pX; phase-local scratch rotates
    # via bufs=2 pools so sample 1's work overlaps sample 0's DMA waits.
    pX = ipools.enter_context(tc.tile_pool(name="pX", bufs=1))
    st = {}; dt = {}; msc = {}; bst = {}; rnk = {}
    sidx_w = {}; didx_w = {}; srow_s = {}; drow_w = {}; brow = {}; rrow = {}

    # ---- A+B fused: norms, window scores, and xn all from slot tiles ----
    # Window-layout slot rows are exactly token rows of x, so sumsq over a
    # slot row reproduces the token norm bit-for-bit; xn is then written via
    # the inverse slot AP. One read of x, no chunk-layout pass at all.
    def slot_ap(b, s4, c):
        sy, sx = s4 >> 1, s4 & 1
        return bass.AP(
            tensor=x_in, offset=(b * N + 64 * 8 * c + 32 * sy + sx) * D,
            ap=[[64 * D, 8], [2 * D, 16], [1, D]])

    def slot_ap_xn(b, s4, c):
        sy, sx = s4 >> 1, s4 & 1
        return bass.AP(
            tensor=xn_d[b], offset=(64 * 8 * c + 32 * sy + sx) * D,
            ap=[[64 * D, 8], [2 * D, 16], [1, D]])

    PAIRS = [(0, 1), (0, 2), (0, 3), (1, 2), (1, 3), (2, 3)]
    PIDX = {p: i for i, p in enumerate(PAIRS)}

    with (
        tc.tile_pool(name="pB", bufs=1) as pB,
        tc.tile_pool(name="pBs", bufs=2) as pBs,
        tc.tile_pool(name="pR", bufs=2) as pR,
        tc.tile_pool(name="pXN", bufs=6) as pXN,
    ):
        slt = {}
        for b in range(B2):
            for s4 in range(4):
                for c in range(2):
                    sl = pB.tile([128, D], F32, tag=f"sl{b}_{s4}_{c}",
                                 name=f"sl{b}_{s4}_{c}")
                    nc.sync.dma_start(sl[:], slot_ap(b, s4, c))
                    slt[b, s4, c] = sl
        for dc in range(DC):
            nc.sync.dma_start(wvt[:, dc, :], wv3[:, dc * D:(dc + 1) * D])

        rslt = {}
        for b in range(B2):
            # per-token sumsq in window layout (scalar engine)
            nsq = pB.tile([128, 2, 4], F32, tag=f"nsq{b}", name=f"nsq{b}")
            for s4 in range(4):
                for c in range(2):
                    sq = pBs.tile([128, D], BF16, tag="sqscr")
                    nc.scalar.activation(
                        sq[:], slt[b, s4, c][:], AF.Square,
                        accum_out=nsq[:, c, s4:s4 + 1])
            # rsl = 1/(norm + 1e-6), same Newton chain as the reference path
            s08 = pB.tile([128, 2, 4], F32, tag=f"s08{b}", name=f"s08{b}")
            nc.scalar.activation(s08[:], nsq[:], AF.Sqrt)
            rs8 = pB.tile([128, 2, 4], F32, tag=f"rs8{b}", name=f"rs8{b}")
            t8 = pB.tile([128, 2, 4], F32, tag=f"t8{b}", name=f"t8{b}")
            nc.vector.reciprocal(rs8[:], s08[:])
            for _ in range(2):
                nc.vector.scalar_tensor_tensor(
                    t8[:], s08[:], -1.0, rs8[:], op0=ALU.mult, op1=ALU.mult)
                nc.vector.tensor_scalar_add(t8[:], t8[:], 2.0)
                nc.vector.tensor_mul(rs8[:], rs8[:], t8[:])
            q8 = pB.tile([128, 2, 4], F32, tag=f"q8{b}", name=f"q8{b}")
            nc.vector.tensor_mul(q8[:], nsq[:], rs8[:])
            nc.vector.tensor_add(q8[:], q8[:], s08[:])
            nc.vector.tensor_scalar(
                q8[:], q8[:], 0.5, 1e-6, op0=ALU.mult, op1=ALU.add)
            iv = pB.tile([128, 2, 4], F32, tag=f"inv{b}", name=f"inv{b}")
            nc.vector.reciprocal(iv[:], q8[:])
            for _ in range(2):
                nc.vector.scalar_tensor_tensor(
                    t8[:], q8[:], -1.0, iv[:], op0=ALU.mult, op1=ALU.mult)
                nc.vector.tensor_scalar_add(t8[:], t8[:], 2.0)
                nc.vector.tensor_mul(iv[:], iv[:], t8[:])
            rslt[b] = iv

        for b in range(B2):
            # xn_d rows first: C-phase gathers and the B->C pool handoff
            # both wait on these, so they must clear the vector queue early
            for s4 in range(4):
                for c in range(2):
                    sln = pXN.tile([128, D], F32, tag="sln")
                    nc.vector.tensor_scalar_mul(
                        sln[:], slt[b, s4, c][:], rslt[b][:, c, s4:s4 + 1])
                    nc.sync.dma_start(slot_ap_xn(b, s4, c), sln[:])
            dd = pB.tile([128, 2, 6], F32, tag=f"dots{b}", name=f"dots{b}")
            for c in range(2):
                for pi, (sa, sb) in enumerate(PAIRS):
                    scr = pBs.tile([128, D], F32, tag="scr")
                    nc.vector.scalar_tensor_tensor(
                        scr[:], slt[b, sa, c][:], 1.0, slt[b, sb, c][:],
                        op0=ALU.bypass, op1=ALU.mult,
                        accum_out=dd[:, c, pi:pi + 1])
            nt = pB.tile([128, 2, 4], F32, tag=f"nt{b}", name=f"nt{b}")
            for c in range(2):
                nc.sync.dma_start(
                    nt[:, c, :],
                    bass.AP(tensor=noise_in, offset=(b * NW + 128 * c) * 4,
                            ap=[[4, 128], [1, 4]]))
            nc.vector.tensor_scalar_mul(nt[:], nt[:], 0.1)
            sc = pB.tile([128, 2, 4], F32, tag=f"scsc{b}", name=f"scsc{b}")
            rsl = rslt[b]
            for s4 in range(4):
                others = [s for s in range(4) if s != s4]
                acc = pBs.tile([128, 2], F32, tag="acc")
                t2 = pBs.tile([128, 2], F32, tag="t2")
                o0 = others[0]
                nc.vector.tensor_mul(
                    acc[:], dd[:, :, PIDX[min(s4, o0), max(s4, o0)]],
                    rsl[:, :, o0])
                for o in others[1:]:
                    nc.vector.tensor_mul(
                        t2[:], dd[:, :, PIDX[min(s4, o), max(s4, o)]],
                        rsl[:, :, o])
                    nc.vector.tensor_add(acc[:], acc[:], t2[:])
                nc.vector.tensor_mul(acc[:], acc[:], rsl[:, :, s4])
                nc.vector.scalar_tensor_tensor(
                    sc[:, :, s4], acc[:], 0.25, nt[:, :, s4],
                    op0=ALU.mult, op1=ALU.add)
            if dbg:
                nc.sync.dma_start(dbg_sc[b], sc[:])

            # argmax over the 4 slots, first max wins:
            # dl = (1-e0) * (1 + (1-e1) * (2 - e2))
            mm = pBs.tile([128, 2], F32, tag="mm")
            m23 = pBs.tile([128, 2], F32, tag="m23")
            nc.vector.tensor_tensor(mm[:], sc[:, :, 0], sc[:, :, 1], op=ALU.max)
            nc.vector.tensor_tensor(m23[:], sc[:, :, 2], sc[:, :, 3], op=ALU.max)
            nc.vector.tensor_tensor(mm[:], mm[:], m23[:], op=ALU.max)
            e0 = pBs.tile([128, 2], F32, tag="e0")
            e1 = pBs.tile([128, 2], F32, tag="e1")
            e2 = pBs.tile([128, 2], F32, tag="e2")
            nc.vector.tensor_tensor(e0[:], sc[:, :, 0], mm[:], op=ALU.is_equal)
            nc.vector.tensor_tensor(e1[:], sc[:, :, 1], mm[:], op=ALU.is_equal)
            nc.vector.tensor_tensor(e2[:], sc[:, :, 2], mm[:], op=ALU.is_equal)
            u2 = pBs.tile([128, 2], F32, tag="u2")
            nc.vector.tensor_scalar(
                u2[:], e2[:], -1.0, 2.0, op0=ALU.mult, op1=ALU.add)
            v1 = pBs.tile([128, 2], F32, tag="v1")
            nc.vector.scalar_tensor_tensor(
                v1[:], e1[:], -1.0, u2[:], op0=ALU.mult, op1=ALU.mult)
            u1 = pBs.tile([128, 2], F32, tag="u1")
            nc.vector.tensor_add(u1[:], v1[:], u2[:])
            nc.vector.tensor_scalar_add(u1[:], u1[:], 1.0)
            v0 = pBs.tile([128, 2], F32, tag="v0")
            nc.vector.scalar_tensor_tensor(
                v0[:], e0[:], -1.0, u1[:], op0=ALU.mult, op1=ALU.mult)
            dl = pBs.tile([128, 2], F32, tag="dl")
            nc.vector.tensor_add(dl[:], v0[:], u1[:])

            # dst token = winbase + 32*(dl>>1) + (dl&1)
            syt = pBs.tile([128, 2], F32, tag="syt")
            nc.vector.tensor_scalar(syt[:], dl[:], 2.0, None, op0=ALU.is_ge)
            sxt = pBs.tile([128, 2], F32, tag="sxt")
            nc.vector.scalar_tensor_tensor(
                sxt[:], syt[:], -2.0, dl[:], op0=ALU.mult, op1=ALU.add)
            dtb = pX.tile([128, 2], F32, tag=f"dt{b}")
            nc.vector.scalar_tensor_tensor(
                dtb[:], syt[:], 32.0, sxt[:], op0=ALU.mult, op1=ALU.add)
            nc.vector.tensor_add(dtb[:], dtb[:], wbt[:])
            dt[b] = dtb

            # src tokens [128, 2, 3]
            stb_ = pX.tile([128, 2, 3], F32, tag=f"st{b}")
            for c in range(2):
                ge = pBs.tile([128, 3], F32, tag="ge")
                nc.vector.tensor_scalar(
                    ge[:], jb[:], dl[:, c:c + 1], None, op0=ALU.is_ge)
                sl3 = pBs.tile([128, 3], F32, tag="sl3")
                nc.vector.tensor_add(sl3[:], ge[:], jb[:])
                sy2 = pBs.tile([128, 3], F32, tag="sy2")
                nc.vector.tensor_scalar(
                    sy2[:], sl3[:], 2.0, None, op0=ALU.is_ge)
                sx2 = pBs.tile([128, 3], F32, tag="sx2")
                nc.vector.scalar_tensor_tensor(
                    sx2[:], sy2[:], -2.0, sl3[:], op0=ALU.mult, op1=ALU.add)
                nc.vector.scalar_tensor_tensor(
                    stb_[:, c, :], sy2[:], 32.0, sx2[:],
                    op0=ALU.mult, op1=ALU.add)
                nc.vector.tensor_scalar_add(
                    stb_[:, c, :], stb_[:, c, :], wbt[:, c:c + 1])
            st[b] = stb_

            # ---- R: token rows + idx staging (interleaved with B) ----
            srow = pX.tile([1, NS], F32, tag=f"srow_s{b}")
            for c in range(2):
                nc.sync.dma_start(
                    srow[:, 384 * c:384 * (c + 1)].rearrange(
                        "a (p j) -> a p j", p=128),
                    st[b][:, c, :])
            srow_s[b] = srow
            drow = pX.tile([1, NW], F32, tag=f"drow_w{b}")
            for c in range(2):
                nc.sync.dma_start(
                    drow[:, 128 * c:128 * (c + 1)], dt[b][:, c:c + 1])
            drow_w[b] = drow
            # i-order idx rows for dma_gather (i = 128*chunk + p)
            sgrow = pR.tile([1, NS], F32, tag="sgrow")
            for h2 in range(2):
                for j3 in range(3):
                    nc.sync.dma_start(
                        sgrow[:, 384 * h2 + 128 * j3:384 * h2 + 128 * (j3 + 1)]
                        .rearrange("a (k o) -> a k o", o=1),
                        st[b][:, h2, j3:j3 + 1])
            sgi = pR.tile([1, NS], I16, tag="sgi")
            nc.vector.tensor_copy(sgi[:], sgrow[:])
            nc.sync.dma_start(sidx_d[b][None, :], sgi[:])
            sw = pX.tile([128, 48], I16, tag=f"sidx_w{b}")
            for g8 in range(8):
                nc.sync.dma_start(sw[16 * g8:16 * (g8 + 1), :], bass.AP(
                    tensor=sidx_d[b], offset=0, ap=[[1, 16], [16, 48]]))
            sidx_w[b] = sw
            dgi = pR.tile([1, NW], I16, tag="dgi")
            nc.vector.tensor_copy(dgi[:], drow[:])
            nc.sync.dma_start(didx_d[b][None, :], dgi[:])
            dw = pX.tile([128, 16], I16, tag=f"didx_w{b}")
            for g8 in range(8):
                nc.sync.dma_start(dw[16 * g8:16 * (g8 + 1), :], bass.AP(
                    tensor=didx_d[b], offset=0, ap=[[1, 16], [16, 16]]))
            didx_w[b] = dw


    # ---- C: gather xn rows, transpose, sim, best + exact stable rank ----
    with (
        tc.tile_pool(name="pC1", bufs=1) as pC,
        tc.tile_pool(name="pC2", bufs=2) as pC2,
    ):
        for b in range(B2):
            mscb = pX.tile([128, 6], F32, tag=f"msc{b}")
            bstb = pX.tile([128, 6], F32, tag=f"bst{b}")
            rnkb = pX.tile([128, 6], F32, tag=f"rnk{b}")
            xnd = pC.tile([128, 2, D], F32, tag=f"xnd{b}")
            nc.gpsimd.dma_gather(
                out_ap=xnd[:], in_ap=xn_d[b][:], idxs_ap=didx_w[b][:],
                num_idxs=NW, num_idxs_reg=NW, elem_size=D)
            xndT = pC.tile([128, DC, NW], F32, tag=f"xndT{b}")
            for c in range(2):
                for dc in range(DC):
                    pt = psumT.tile([128, 128], F32, tag="pt")
                    nc.tensor.transpose(
                        pt[:], xnd[:, c, 128 * dc:128 * (dc + 1)], ident[:])
                    if dc % 2 == 0:
                        nc.scalar.copy(
                            xndT[:, dc, 128 * c:128 * (c + 1)], pt[:])
                    else:
                        nc.vector.tensor_copy(
                            xndT[:, dc, 128 * c:128 * (c + 1)], pt[:])
            for half in range(2):
                xns = pC.tile([128, 3, D], F32, tag=f"xns{half}")
                nc.gpsimd.dma_gather(
                    out_ap=xns[:], in_ap=xn_d[b][:],
                    idxs_ap=sidx_w[b][:, 24 * half:24 * (half + 1)],
                    num_idxs=384, num_idxs_reg=384, elem_size=D)
                xnsT = pC.tile([128, 3, DC, 128], F32, tag=f"xnsT{half}")
                for c3 in range(3):
                    for dc in range(DC):
                        pt = psumT.tile([128, 128], F32, tag="pt")
                        nc.tensor.transpose(
                            pt[:], xns[:, c3, 128 * dc:128 * (dc + 1)],
                            ident[:])
                        if dc % 2 == 0:
                            nc.scalar.copy(xnsT[:, c3, dc, :], pt[:])
                        else:
                            nc.vector.tensor_copy(xnsT[:, c3, dc, :], pt[:])
                for c3 in range(3):
                    cc6 = 3 * half + c3
                    ps = psumS.tile([128, NW], F32, tag="ps")
                    for dc in range(DC):
                        nc.tensor.matmul(
                            ps[:], xnsT[:, c3, dc, :], xndT[:, dc, :],
                            start=(dc == 0), stop=(dc == DC - 1))
                    nc.vector.reduce_max(
                        mscb[:, cc6:cc6 + 1], ps[:], axis=AX.X)
                    eqt = pC.tile([128, NW], F32, tag="eqt")
                    nc.vector.tensor_scalar(
                        eqt[:], ps[:], mscb[:, cc6:cc6 + 1], None,
                        op0=ALU.is_equal)
                    mskt = pC.tile([128, NW], F32, tag="mskt")
                    nc.vector.scalar_tensor_tensor(
                        mskt[:], eqt[:], -BIG, iotaBIG[:],
                        op0=ALU.mult, op1=ALU.add)
                    nc.vector.tensor_reduce(
                        bstb[:, cc6:cc6 + 1], mskt[:], axis=AX.X, op=ALU.min)
            msc[b], bst[b], rnk[b] = mscb, bstb, rnkb

        for b in range(B2):
            # maxsim broadcast via DRAM bounce (s order)
            for cc6 in range(6):
                c, j = cc6 // 3, cc6 % 3
                nc.sync.dma_start(
                    bass.AP(tensor=mrow_d[b], offset=384 * c + j,
                            ap=[[3, 128], [1, 1]]),
                    msc[b][:, cc6:cc6 + 1])
            mbc = pC2.tile([128, NS], F32, tag="mbc")
            nc.sync.dma_start(
                mbc[:],
                bass.AP(tensor=mrow_d[b], offset=0, ap=[[0, 128], [1, NS]]))

            # exact stable rank
            gcnt = pC2.tile([128, 1], F32, tag="gcnt")
            ecnt = pC2.tile([128, 1], F32, tag="ecnt")
            for lh in range(2):
                ltm = pC2.tile([128, 3, NS], F32, tag="ltm")
                for c3 in range(3):
                    nc.sync.dma_start(ltm[:, c3, :], ltmask[3 * lh + c3])
                for c3 in range(3):
                    cc6 = 3 * lh + c3
                    sc1 = pC2.tile([128, NS], F32, tag="sc1")
                    nc.vector.scalar_tensor_tensor(
                        sc1[:], mbc[:], msc[b][:, cc6:cc6 + 1], mbc[:],
                        op0=ALU.is_gt, op1=ALU.bypass, accum_out=gcnt[:])
                    sc2 = pC2.tile([128, NS], F32, tag="sc2")
                    nc.vector.scalar_tensor_tensor(
                        sc2[:], mbc[:], msc[b][:, cc6:cc6 + 1], ltm[:, c3, :],
                        op0=ALU.is_equal, op1=ALU.mult, accum_out=ecnt[:])
                    nc.vector.tensor_add(
                        rnk[b][:, cc6:cc6 + 1], gcnt[:], ecnt[:])

            # best + rank rows in s order
            browb = pX.tile([1, NS], F32, tag=f"brow{b}")
            rrowb = pX.tile([1, NS], F32, tag=f"rrow{b}")
            for cc6 in range(6):
                c, j = cc6 // 3, cc6 % 3
                dst_b = browb[:].rearrange(
                    "a (c p j) -> a p c j", c=2, p=128)[:, :, c, j]
                nc.sync.dma_start(dst_b, bst[b][:, cc6:cc6 + 1])
                dst_r = rrowb[:].rearrange(
                    "a (c p j) -> a p c j", c=2, p=128)[:, :, c, j]
                nc.sync.dma_start(dst_r, rnk[b][:, cc6:cc6 + 1])
            brow[b], rrow[b] = browb, rrowb
            if dbg:
                nc.sync.dma_start(dbg_ms[b][None, :], mbc[0:1, :])
                nc.sync.dma_start(dbg_best[b][None, :], browb[:])
                nc.sync.dma_start(dbg_rank[b][None, :], rrowb[:])

    # ---- K: masks, prefix sum, one-hot compaction, g construction ----
    with tc.tile_pool(name="pK", bufs=2) as pK:
        for b in range(B2):
            kpm = pK.tile([1, NS], F32, tag="kpm")
            nc.vector.tensor_scalar(
                kpm[:], rrow[b][:], 512.0, None, op0=ALU.is_ge)
            kex = pK.tile([1, NS], F32, tag="kex")
            nc.vector.tensor_tensor_scan(
                kex[:], kpm[:], kpm[:], 0.0, op0=ALU.add, op1=ALU.bypass)
            nc.vector.tensor_sub(kex[:], kex[:], kpm[:])
            # v_src = best + kpm*(256 + kex - best)
            tq = pK.tile([1, NS], F32, tag="tmp768")
            nc.vector.tensor_sub(tq[:], kex[:], brow[b][:])
            nc.vector.scalar_tensor_tensor(
                tq[:], tq[:], 256.0, kpm[:], op0=ALU.add, op1=ALU.mult)
            vsr = pK.tile([1, NS], F32, tag="vsr")
            nc.vector.tensor_add(vsr[:], tq[:], brow[b][:])

            # f32 token row [dst_w | src_s] and value row [witer | vsr]
            trow = pK.tile([1, N], F32, tag="trow")
            nc.scalar.copy(trow[:, :NW], drow_w[b][:])
            nc.scalar.copy(trow[:, NW:], srow_s[b][:])
            vrow = pK.tile([1, N], F32, tag="vrow")
            nc.scalar.copy(vrow[:, :NW], witer[:])
            nc.scalar.copy(vrow[:, NW:], vsr[:])
            # masked keep-rank row: kpm*(kex+1) - 1  (pruned -> -1)
            mk = pK.tile([1, NS], F32, tag="mk")
            nc.vector.scalar_tensor_tensor(
                mk[:], kex[:], 1.0, kpm[:], op0=ALU.add, op1=ALU.mult)
            nc.vector.tensor_scalar_add(mk[:], mk[:], -1.0)
            nc.sync.dma_start(mk_d[b][None, :], mk[:])
            nc.sync.dma_start(tok_d[b][None, :], trow[:])
            nc.sync.dma_start(val_d[b][None, :], vrow[:])
            mkb = pK.tile([128, NS], F32, tag="mkb")
            nc.sync.dma_start(mkb[:], bass.AP(
                tensor=mk_d[b], offset=0, ap=[[0, 128], [1, NS]]))
            stb = pK.tile([128, NS], F32, tag="stb")
            nc.sync.dma_start(stb[:], bass.AP(
                tensor=tok_d[b], offset=NW, ap=[[0, 128], [1, NS]]))
            tkb = pK.tile([128, N], F32, tag="tkb")
            nc.sync.dma_start(tkb[:], bass.AP(
                tensor=tok_d[b], offset=0, ap=[[0, 128], [1, N]]))
            vlb = pK.tile([128, N], F32, tag="vlb")
            nc.sync.dma_start(vlb[:], bass.AP(
                tensor=val_d[b], offset=0, ap=[[0, 128], [1, N]]))

            krow = pK.tile([1, T], F32, tag="krow")
            nc.scalar.copy(krow[:, :NW], drow_w[b][:])
            eqk = pK.tile([128, NS], F32, tag="eqk")
            for c2 in range(2):
                rtg = pK.tile([128, 1], F32, tag="rtg")
                nc.vector.tensor_scalar_add(rtg[:], io128[:], float(128 * c2))
                kv = pK.tile([128, 1], F32, tag="kv")
                nc.vector.scalar_tensor_tensor(
                    eqk[:], mkb[:], rtg[:], stb[:],
                    op0=ALU.is_equal, op1=ALU.mult, accum_out=kv[:])
                seg = krow[:, NW + 128 * c2:NW + 128 * (c2 + 1)]
                nc.sync.dma_start(seg.rearrange("a (k o) -> a k o", o=1), kv[:])

            grow = pK.tile([1, N], F32, tag="grow")
            eqg = pK.tile([128, N], F32, tag="eqg")
            for c8 in range(8):
                ttg = pK.tile([128, 1], F32, tag="ttg")
                nc.vector.tensor_scalar_add(ttg[:], io128[:], float(128 * c8))
                gv = pK.tile([128, 1], F32, tag="gv")
                nc.vector.scalar_tensor_tensor(
                    eqg[:], tkb[:], ttg[:], vlb[:],
                    op0=ALU.is_equal, op1=ALU.mult, accum_out=gv[:])
                seg = grow[:, 128 * c8:128 * (c8 + 1)]
                nc.sync.dma_start(seg.rearrange("a (k o) -> a k o", o=1), gv[:])

            # int16 gather-idx staging
            ki16 = pK.tile([1, T], I16, tag="ki16")
            nc.vector.tensor_copy(ki16[:], krow[:])
            nc.sync.dma_start(kidx_d[b][None, :], ki16[:])
            gi16 = pK.tile([1, N], I16, tag="gi16")
            nc.vector.tensor_copy(gi16[:], grow[:])
            nc.sync.dma_start(gidx_d[b][None, :], gi16[:])
            if dbg:
                ki32 = pK.tile([1, T], I32, tag="ki32")
                nc.vector.tensor_copy(ki32[:], krow[:])
                nc.sync.dma_start(keep_d[b][None, :], ki32[:])
                gi32 = pK.tile([1, N], I32, tag="gi32")
                nc.vector.tensor_copy(gi32[:], grow[:])
                nc.sync.dma_start(g_d[b][None, :], gi32[:])
    ipools.close()

    if stop_after == "index":
        ctx.close()
        return dict(nc=nc)

    # =================== DiT block ===================
    build_block(nc, tc, ctx, cfg, dict(
        identb=identb, ident=ident, x_in=x_in,
        wqk3=wqk3, wvt=wvt, wp3=wp3, wf13=wf13, wf23=wf23,
        bqk=bqk, bv_row=bv_row, bproj_row=bproj_row, bfc1=bfc1,
        bfc2_row=bfc2_row, bo_d=bo_d, kidx_d=kidx_d, aden_d=aden_d,
        aden_raw=aden_raw, gidx_d=gidx_d, out=out, dbg=dbg))

    ctx.close()
    return dict(nc=nc)


def layer_norm(nc, pool, small, xin, yout, eps=1e-6):
    """Row LN: yout = (x - mu) * rsqrt(var + eps). xin fp32 [128, D]."""
    mu = small.tile([128, 1], F32, tag="ln_mu")
    nc.vector.tensor_reduce(mu[:], xin, axis=AX.X, op=ALU.add)
    nc.vector.tensor_scalar_mul(mu[:], mu[:], 1.0 / D)
    xc = pool.tile([128, D], F32, tag="ln_xc")
    nc.vector.tensor_scalar(xc[:], xin, mu[:], None, op0=ALU.subtract)
    sq = pool.tile([128, D], F32, tag="ln_sq")
    var = small.tile([128, 1], F32, tag="ln_var")
    nc.vector.scalar_tensor_tensor(
        sq[:], xc[:], 1.0, xc[:], op0=ALU.bypass, op1=ALU.mult, accum_out=var[:])
    nc.vector.tensor_scalar(
        var[:], var[:], 1.0 / D, eps, op0=ALU.mult, op1=ALU.add)
    sd = small.tile([128, 1], F32, tag="ln_sd")
    nc.scalar.activation(sd[:], var[:], AF.Sqrt)
    rstd = small.tile([128, 1], F32, tag="ln_rstd")
    nc.vector.reciprocal(rstd[:], sd[:])
    nc.vector.tensor_scalar_mul(yout, xc[:], rstd[:])


def build_block(nc, tc, ctx, cfg, env):
    BD = BF16
    stop_after = cfg.get("stop_after", None)
    identb = env["identb"]
    x_in = env["x_in"]
    wqk3, wvt = env["wqk3"], env["wvt"]
    wp3, wf13, wf23 = env["wp3"], env["wf13"], env["wf23"]
    bo_d, kidx_d = env["bo_d"], env["kidx_d"]
    aden_d, aden_raw = env["aden_d"], env["aden_raw"]
    gidx_d, out = env["gidx_d"], env["out"]

    # PSUM banks: psA 2 + psB 2 on ctx; psT (LN phases) and psPO/psD
    # (attention) are scoped so attention gets 2 bufs each: max 8 banks
    bcp = ctx.enter_context(tc.tile_pool(name="bcp", bufs=1))
    psA = ctx.enter_context(tc.tile_pool(name="psA", bufs=2, space="PSUM"))
    psB = ctx.enter_context(tc.tile_pool(name="psB", bufs=2, space="PSUM"))
    small = ctx.enter_context(tc.tile_pool(name="bsmall", bufs=4))

    bvb = bcp.tile([128, D], BD)
    nc.sync.dma_start(bvb[:], bass.AP(
        tensor=env["bv_row"], offset=0, ap=[[0, 128], [1, D]]))
    bpb = bcp.tile([128, D], BD)
    nc.sync.dma_start(bpb[:], bass.AP(
        tensor=env["bproj_row"], offset=0, ap=[[0, 128], [1, D]]))
    bf2b = bcp.tile([128, D], BD)
    nc.sync.dma_start(bf2b[:], bass.AP(
        tensor=env["bfc2_row"], offset=0, ap=[[0, 128], [1, D]]))
    bqkt = bcp.tile([128, 18], F32)
    nc.sync.dma_start(bqkt[:], env["bqk"][:, :])
    bf1t = bcp.tile([128, 36], F32)
    nc.sync.dma_start(bf1t[:], env["bfc1"][:, :])
    # all-ones stationary column (softmax denominators via matmul)
    onesb = bcp.tile([128, 1], BD)
    nc.vector.tensor_scalar(onesb[:], bqkt[:, 0:1], 0.0, 1.0,
                            op0=ALU.mult, op1=ALU.add)
    kidx_t = [None, None]
    for b in range(B2):
        kpt = bcp.tile([128, 32], I16, tag=f"kidx2_{b}", name=f"kidx2_{b}")
        for g8 in range(8):
            nc.sync.dma_start(kpt[16 * g8:16 * (g8 + 1), :], bass.AP(
                tensor=kidx_d[b], offset=0, ap=[[1, 16], [16, 32]]))
        kidx_t[b] = kpt

    def transpose_to(psT, dst_sl, y, dc):
        pt = psT.tile([128, 128], BD, tag="bt")
        nc.tensor.transpose(pt[:], y[:, 128 * dc:128 * (dc + 1)], identb[:])
        if dc % 2 == 0:
            nc.scalar.copy(dst_sl, pt[:])
        else:
            nc.vector.tensor_copy(dst_sl, pt[:])

    # x1 accumulator f32; LN1 gathers write straight into it (x rows),
    # proj then adds b_proj + attention delta, MLP adds the rest.
    p_x1 = ctx.enter_context(tc.tile_pool(name="p_x1", bufs=1))
    x1 = p_x1.tile([128, B2, TC, D], F32)

    with tc.tile_pool(name="p_ot", bufs=1) as p_ot:
        OT = [p_ot.tile([128, DC, T], BD, tag=f"OT{b}", name=f"OT{b}")
              for b in range(B2)]
        with tc.tile_pool(name="p_yt", bufs=1) as p_yt:
            YT = p_yt.tile([128, DC, 2 * T], BD)
            # ---- LN1 -> YT ----
            with (
                tc.tile_pool(name="p_ln1", bufs=2) as p_ln,
                tc.tile_pool(name="psT1", bufs=2, space="PSUM") as psT1,
            ):
                for ct in range(8):
                    b, c4 = divmod(ct, TC)
                    nc.gpsimd.dma_gather(
                        out_ap=x1[:, b, c4:c4 + 1, :], in_ap=x_in[b],
                        idxs_ap=kidx_t[b][:, 8 * c4:8 * (c4 + 1)],
                        num_idxs=128, num_idxs_reg=128, elem_size=D)
                    y = p_ln.tile([128, D], BD, tag="y")
                    layer_norm(nc, p_ln, small, x1[:, b, c4, :], y[:])
                    for dc in range(DC):
                        transpose_to(psT1, YT[:, dc, 128 * ct:128 * (ct + 1)],
                                     y, dc)

            with tc.tile_pool(name="p_v", bufs=1) as p_v:
                V = p_v.tile([128, 2 * TC, D], BD)
                # ---- V (token-major) ----
                for ct in range(8):
                    for ns in range(3):
                        pv = psB.tile([128, 384], F32, tag="b")
                        for dc in range(DC):
                            nc.tensor.matmul(
                                pv[:], YT[:, dc, 128 * ct:128 * (ct + 1)],
                                wvt[:, dc, 384 * ns:384 * (ns + 1)],
                                start=(dc == 0), stop=(dc == DC - 1))
                        nc.vector.scalar_tensor_tensor(
                            V[:, ct, 384 * ns:384 * (ns + 1)], pv[:], 1.0,
                            bvb[:, 384 * ns:384 * (ns + 1)],
                            op0=ALU.bypass, op1=ALU.add)

                if stop_after == "v":
                    return
                # ---- attention: transposed scores, no max-subtract,
                #      denominators via ones-matmul, normalize at oh ----
                with (
                    tc.tile_pool(name="p_wqk2", bufs=1) as p_wqk2,
                    tc.tile_pool(name="psPO", bufs=2, space="PSUM") as psPO,
                    tc.tile_pool(name="psD", bufs=2, space="PSUM") as psD,
                ):
                    for hg in range(2):
                        wqh = p_wqk2.tile([128, DC, DC, 128], BD, tag="wqh")
                        for mcl in range(DC):
                            mc = DC * hg + mcl
                            nc.sync.dma_start(
                                wqh[:, mcl, :, :],
                                wqk3[:, mc * D:(mc + 1) * D])
                        for b in range(B2):
                            with (
                                tc.tile_pool(name="p_qk", bufs=1) as p_qk,
                                tc.tile_pool(name="p_att", bufs=3) as p_att,
                                tc.tile_pool(name="p_et", bufs=2) as p_et,
                            ):
                                QKT = p_qk.tile([128, DC, T], BD,
                                                name=f"QKT{b}{hg}")
                                for mcl in range(DC):
                                    mc = DC * hg + mcl
                                    pq = psA.tile([128, T], F32, tag="a")
                                    for dc in range(DC):
                                        nc.tensor.matmul(
                                            pq[:], wqh[:, mcl, dc, :],
                                            YT[:, dc, T * b:T * (b + 1)],
                                            start=(dc == 0),
                                            stop=(dc == DC - 1))
                                    if mcl % 2 == 0:
                                        nc.scalar.activation(
                                            QKT[:, mcl, :], pq[:],
                                            AF.Identity,
                                            bias=bqkt[:, mc:mc + 1])
                                    else:
                                        nc.vector.tensor_scalar(
                                            QKT[:, mcl, :], pq[:],
                                            bqkt[:, mc:mc + 1], None,
                                            op0=ALU.add)
                                for hl in range(8):
                                    h = 8 * hg + hl
                                    qh = p_att.tile([DH, T], BD, tag="qh")
                                    kh = p_att.tile([DH, T], BD, tag="kh")
                                    for (dst, base) in ((qh, DH * hl),
                                                        (kh, 576 + DH * hl)):
                                        r0 = base
                                        while r0 < base + DH:
                                            mcl, p0 = divmod(r0, 128)
                                            take = min(128 - p0,
                                                       base + DH - r0)
                                            nc.sync.dma_start(
                                                dst[r0 - base:
                                                    r0 - base + take, :],
                                                QKT[p0:p0 + take, mcl, :])
                                            r0 += take
                                    # E^T chunks from transposed scores
                                    ET = p_et.tile([128, TC, T], BD, tag="ET")
                                    for kc in range(TC):
                                        ps = psA.tile([128, T], F32, tag="a")
                                        nc.tensor.matmul(
                                            ps[:],
                                            kh[:, 128 * kc:128 * (kc + 1)],
                                            qh[:], start=True, stop=True)
                                        nc.scalar.activation(
                                            ET[:, kc, :], ps[:], AF.Exp,
                                            scale=RSQ_DH)
                                    # o^T (unnormalized) and denominators
                                    po = psPO.tile([DH, T], F32, tag="po")
                                    pden = psD.tile([1, T], F32, tag="pden")
                                    for kc in range(TC):
                                        nc.tensor.matmul(
                                            po[:],
                                            V[:, TC * b + kc,
                                              DH * h:DH * (h + 1)],
                                            ET[:, kc, :],
                                            start=(kc == 0),
                                            stop=(kc == TC - 1))
                                        nc.tensor.matmul(
                                            pden[:], onesb[:], ET[:, kc, :],
                                            start=(kc == 0),
                                            stop=(kc == TC - 1))
                                    # reciprocal via [32,16] DRAM reshape
                                    # ([1,512] DVE recip is ~4cyc/elem serial)
                                    rraw = small.tile([1, T], F32, tag="rraw")
                                    nc.scalar.copy(rraw[:], pden[:])
                                    nc.sync.dma_start(
                                        aden_raw[b, h][None, :], rraw[:])
                                    r32 = small.tile([32, 16], F32, tag="r32")
                                    nc.sync.dma_start(r32[:], bass.AP(
                                        tensor=aden_raw,
                                        offset=(b * H + h) * T,
                                        ap=[[16, 32], [1, 16]]))
                                    rd32 = small.tile([32, 16], F32,
                                                      tag="rd32")
                                    nc.vector.reciprocal(rd32[:], r32[:])
                                    rdb = small.tile([32, 16], BD, tag="rdb")
                                    nc.vector.tensor_copy(rdb[:], rd32[:])
                                    nc.sync.dma_start(
                                        bass.AP(tensor=aden_d,
                                                offset=(b * H + h) * T,
                                                ap=[[16, 32], [1, 16]]),
                                        rdb[:])
                                    rbc = p_att.tile([DH, T], BD, tag="rbc")
                                    nc.sync.dma_start(rbc[:], bass.AP(
                                        tensor=aden_d, offset=(b * H + h) * T,
                                        ap=[[0, DH], [1, T]]))
                                    oh = p_att.tile([DH, T], BD, tag="oh")
                                    nc.vector.tensor_tensor(
                                        oh[:], po[:], rbc[:], op=ALU.mult)
                                    r0 = DH * h
                                    while r0 < DH * (h + 1):
                                        dc, p0 = divmod(r0, 128)
                                        take = min(128 - p0, DH * (h + 1) - r0)
                                        nc.sync.dma_start(
                                            OT[b][p0:p0 + take, dc, :],
                                            oh[r0 - DH * h:
                                               r0 - DH * h + take, :])
                                        r0 += take

        # ---- proj + residual -> x1, full 9-dc PSUM accumulation ----
        with tc.tile_pool(name="p_wp", bufs=1) as p_wp:
            wpt = p_wp.tile([128, DC, D], BD)
            for dc in range(DC):
                nc.sync.dma_start(wpt[:, dc, :], wp3[:, dc * D:(dc + 1) * D])
            for b in range(B2):
                for c4 in range(TC):
                    nc.vector.tensor_add(
                        x1[:, b, c4, :], x1[:, b, c4, :], bpb[:])
                    for ns in range(3):
                        pp = psB.tile([128, 384], F32, tag="b")
                        for dc in range(DC):
                            nc.tensor.matmul(
                                pp[:], OT[b][:, dc, 128 * c4:128 * (c4 + 1)],
                                wpt[:, dc, 384 * ns:384 * (ns + 1)],
                                start=(dc == 0), stop=(dc == DC - 1))
                        sl = x1[:, b, c4, 384 * ns:384 * (ns + 1)]
                        nc.vector.scalar_tensor_tensor(
                            sl, pp[:], 1.0, sl, op0=ALU.bypass, op1=ALU.add)

    if stop_after == "attn":
        return
    # ---- LN2 + MLP (per-sample supergroups of 18) + inline recover ----
    with tc.tile_pool(name="p_y2", bufs=1) as p_y2:
        Y2T = p_y2.tile([128, DC, 2 * T], BD)
        with (
            tc.tile_pool(name="p_ln2", bufs=2) as p_ln,
            tc.tile_pool(name="psT2", bufs=2, space="PSUM") as psT2,
        ):
            for ct in range(8):
                b, c4 = divmod(ct, TC)
                y = p_ln.tile([128, D], BD, tag="y")
                layer_norm(nc, p_ln, small, x1[:, b, c4, :], y[:])
                for dc in range(DC):
                    transpose_to(psT2, Y2T[:, dc, 128 * ct:128 * (ct + 1)],
                                 y, dc)
                nc.vector.tensor_add(
                    x1[:, b, c4, :], x1[:, b, c4, :], bf2b[:])

        def emit_recover(rb):
            # recover sample rb: gather block rows back to full token order,
            # split in halves so the out-write of half 0 overlaps half 1
            with tc.tile_pool(name="recp", bufs=2) as recp:
                gw = recp.tile([128, 64], I16, tag="gw")
                for g8 in range(8):
                    nc.sync.dma_start(gw[16 * g8:16 * (g8 + 1), :], bass.AP(
                        tensor=gidx_d[rb], offset=0, ap=[[1, 16], [16, 64]]))
                for hf in range(2):
                    og = recp.tile([128, 4, D], BD, tag="og")
                    nc.gpsimd.dma_gather(
                        out_ap=og[:], in_ap=bo_d[rb][:],
                        idxs_ap=gw[:, 32 * hf:32 * (hf + 1)],
                        num_idxs=T, num_idxs_reg=T, elem_size=D)
                    ogf = recp.tile([128, 4, D], F32, tag="ogf")
                    nc.vector.tensor_copy(ogf[:], og[:])
                    nc.sync.dma_start(
                        bass.AP(tensor=out, offset=(rb * N + hf * T) * D,
                                ap=[[D, 128], [128 * D, 4], [1, D]]),
                        ogf[:])

        for b in range(B2):
            for sg in range(2):
                with tc.tile_pool(name="p_ht", bufs=1) as p_ht:
                    HT = p_ht.tile([128, 18, T], BD, name=f"HT{b}{sg}")
                    with tc.tile_pool(name="p_wf1", bufs=3) as p_wf1:
                        for k18 in range(18):
                            mf = 18 * sg + k18
                            wt = p_wf1.tile([128, DC, 128], BD, tag="wf1")
                            nc.sync.dma_start(wt[:], wf13[mf])
                            pf = psA.tile([128, T], F32, tag="a")
                            for dc in range(DC):
                                nc.tensor.matmul(
                                    pf[:], wt[:, dc, :],
                                    Y2T[:, dc, T * b:T * (b + 1)],
                                    start=(dc == 0), stop=(dc == DC - 1))
                            nc.scalar.activation(
                                HT[:, k18, :], pf[:],
                                AF.Gelu_apprx_tanh, bias=bf1t[:, mf:mf + 1])
                    with tc.tile_pool(name="p_wf2", bufs=1) as p_wf2:
                        wf2 = [p_wf2.tile([128, D], BD, tag=f"wf2_{i}",
                                          name=f"wf2t{b}{sg}{i}")
                               for i in range(18)]
                        for i in range(18):
                            nc.sync.dma_start(wf2[i][:], wf23[18 * sg + i])
                        for c4 in range(TC):
                            for ns in range(3):
                                pg = psB.tile([128, 384], F32, tag="b")
                                for i in range(18):
                                    nc.tensor.matmul(
                                        pg[:],
                                        HT[:, i, 128 * c4:128 * (c4 + 1)],
                                        wf2[i][:, 384 * ns:384 * (ns + 1)],
                                        start=(i == 0), stop=(i == 17))
                                sl = x1[:, b, c4, 384 * ns:384 * (ns + 1)]
                                nc.vector.scalar_tensor_tensor(
                                    sl, pg[:], 1.0, sl,
                                    op0=ALU.bypass, op1=ALU.add)

            # ---- write block output rows (DRAM row = 128*c4 + p) ----
            for c4 in range(TC):
                xob = small.tile([128, D], BD, tag="xob")
                nc.vector.tensor_copy(xob[:], x1[:, b, c4, :])
                nc.sync.dma_start(
                    bass.AP(tensor=bo_d[b], offset=c4 * 128 * D,
                            ap=[[D, 128], [1, D]]),
                    xob[:])
            emit_recover(b)


# ======================================================================
# kernel() entry point: full inputs -> full output on 8 NeuronCores
# ======================================================================

_MODULE_CACHE = {}


def _get_module(block_dtype_name):
    # block_dtype_name kept for interface compat; the block is always bf16
    if "bf16" not in _MODULE_CACHE:
        from concourse import bacc
        nc = bacc.Bacc(None, target_bir_lowering=False)
        build(nc, {})
        nc.compile()
        _MODULE_CACHE["bf16"] = nc
    return _MODULE_CACHE["bf16"]


def kernel(x, noise, ln1_g, ln1_b, ln2_g, ln2_b, w_qkv, b_qkv, w_proj, b_proj,
           w_fc1, b_fc1, w_fc2, b_fc2, block_dtype="f32r", **run_kw):
    from concourse import bass_utils

    x = np.ascontiguousarray(np.asarray(x, np.float32))
    noise = np.ascontiguousarray(np.asarray(noise, np.float32))
    B = x.shape[0]
    n_cores = B // B2
    wt = retile_weights(
        dict(ln1_g=ln1_g, ln1_b=ln1_b, ln2_g=ln2_g, ln2_b=ln2_b,
             w_qkv=w_qkv, b_qkv=b_qkv, w_proj=w_proj, b_proj=b_proj,
             w_fc1=w_fc1, b_fc1=b_fc1, w_fc2=w_fc2, b_fc2=b_fc2))

    nc = _get_module(block_dtype)
    in_maps = []
    for c in range(n_cores):
        m = dict(x=x[B2 * c:B2 * (c + 1)], noise=noise[B2 * c:B2 * (c + 1)])
        m.update(wt)
        in_maps.append(m)
    res = bass_utils.run_bass_kernel_spmd(
        nc, in_maps, core_ids=list(range(n_cores)), **run_kw)
    out = np.concatenate([res.results[c]["out"] for c in range(n_cores)], axis=0)
    if run_kw.get("trace"):
        return out, res
    return out



# revision 48
# speedup vs baseline: 1.0392x; 1.0060x over previous
"""DiT-SiTo block kernel builder for one NeuronCore (2 samples per core).

Index conventions (per sample):
  tokens t in [0,1024); window w in [0,256); slot s4 in {0..3}
  src index s in [0,768): s = 3*w + j (reference order)
  window-chunk layout: w = 128*c + p  (c in {0,1}, p = partition)
  gathered src rows: (p, cc) with cc = 3*c + j  ->  s = 3*(128*c+p) + j
  keep positions r in [0,512): r < 256 -> dst of window w=r; else kept src
  block token chunks: chunk c holds positions r in [128c, 128c+128), p = r%128
"""

from contextlib import ExitStack

import numpy as np

import concourse.bass as bass
import concourse.mybir as mybir
import concourse.tile as tile
from concourse.bass import IndirectOffsetOnAxis
from concourse import library_config
from concourse.masks import make_identity

I16 = mybir.dt.int16
F32 = mybir.dt.float32
F32R = mybir.dt.float32r
BF16 = mybir.dt.bfloat16
I32 = mybir.dt.int32
AF = mybir.ActivationFunctionType
ALU = mybir.AluOpType
AX = mybir.AxisListType

B2 = 2
N = 1024
D = 1152
DC = D // 128          # 9
H = 16
DH = 72
NW = 256
NS = 768
T = 512
TC = T // 128          # 4
D4 = 4608
BIG = 1.0e4
RSQ_DH = float(1.0 / np.sqrt(DH))


def host_constants():
    w = np.arange(NW)
    winbase = (64 * (w >> 4) + 2 * (w & 15)).astype(np.float32)
    iota256 = np.arange(NW, dtype=np.float32)
    jrow = np.arange(3, dtype=np.float32)
    # ltmask[cc, p, jj] = 1.0 iff jj < s(p, cc);  cc = 3*c + j
    cc = np.arange(6)
    c, j = cc // 3, cc % 3
    s = 3 * (128 * c[:, None] + np.arange(128)[None, :]) + j[:, None]   # [6,128]
    ltm = (np.arange(NS)[None, None, :] < s[:, :, None]).astype(np.float32)
    return winbase, iota256, jrow, np.ascontiguousarray(ltm)


def to_bf16(a):
    import ml_dtypes
    return np.ascontiguousarray(np.asarray(a, np.float32).astype(
        ml_dtypes.bfloat16))


def retile_weights(inp):
    """Host-side: fold LN affine into the following matmul, retile weights.

    All block weights go out partition-major and bf16 so every DMA load is
    [128, big-contiguous] (fat descriptors):
      wqk3  [128, 18*DC*128]  (p, mc, dc, col)  stationary chunks
      wv3   [128, DC*1152]    (p, dc, col)      moving rows
      wp3   [128, DC*1152]    (p, dc, col)      moving rows
      wf13  [36, 128, DC*128] (mf, p, dc, col)  stationary chunks
      wf23  [36, 128, 1152]   (kk, p, col)      moving rows
    """
    f32 = np.float32
    g1, b1 = np.asarray(inp["ln1_g"], f32), np.asarray(inp["ln1_b"], f32)
    g2, b2 = np.asarray(inp["ln2_g"], f32), np.asarray(inp["ln2_b"], f32)
    wqkv = np.asarray(inp["w_qkv"], f32); bqkv = np.asarray(inp["b_qkv"], f32)
    wfc1 = np.asarray(inp["w_fc1"], f32); bfc1 = np.asarray(inp["b_fc1"], f32)
    wqkv_f = g1[:, None] * wqkv
    bqkv_f = bqkv + b1 @ wqkv
    wfc1_f = g2[:, None] * wfc1
    bfc1_f = bfc1 + b2 @ wfc1

    # column order: [q heads 0-7 | k heads 0-7 | q heads 8-15 | k heads 8-15]
    perm = np.concatenate([
        np.arange(576), D + np.arange(576),
        576 + np.arange(576), D + 576 + np.arange(576)])
    wqk = wqkv_f[:, perm]                                      # [1152, 2304]
    wqk3 = wqk.reshape(DC, 128, 18, 128).transpose(1, 2, 0, 3).reshape(
        128, 18 * DC * 128)
    wv = wqkv_f[:, 2 * D:]
    wv3 = wv.reshape(DC, 128, D).transpose(1, 0, 2).reshape(128, DC * D)
    wp = np.asarray(inp["w_proj"], f32)
    wp3 = wp.reshape(DC, 128, D).transpose(1, 0, 2).reshape(128, DC * D)
    wf13 = wfc1_f.reshape(DC, 128, 36, 128).transpose(2, 1, 0, 3).reshape(
        36, 128, DC * 128)
    wf23 = np.asarray(inp["w_fc2"], f32).reshape(36, 128, D)
    bqk = np.ascontiguousarray(bqkv_f[perm].reshape(18, 128).T)     # [128, 18]
    return dict(
        wqk3=to_bf16(wqk3), wv3=to_bf16(wv3), wp3=to_bf16(wp3),
        wf13=to_bf16(wf13), wf23=to_bf16(wf23),
        bqk=bqk.astype(f32),
        bv_row=to_bf16(bqkv_f[None, 2 * D:]),
        bproj_row=to_bf16(np.asarray(inp["b_proj"], f32)[None, :]),
        bfc1=np.ascontiguousarray(bfc1_f.reshape(36, 128).T).astype(f32),
        bfc2_row=to_bf16(np.asarray(inp["b_fc2"], f32)[None, :]),
    )


def make_in_map(x_pair, noise_pair, weights):
    m = dict(x=np.ascontiguousarray(x_pair, np.float32),
             noise=np.ascontiguousarray(noise_pair, np.float32))
    m.update(weights)
    return m


def newton_recip(nc, pool, x, tag, iters=2):
    """r ~= 1/x to fp32 accuracy. x: [p,1] tile slice."""
    p = x.shape[0]
    r = pool.tile([p, 1], F32, tag=tag + "_r")
    t = pool.tile([p, 1], F32, tag=tag + "_t")
    nc.vector.reciprocal(r[:], x[:])
    for _ in range(iters):
        nc.vector.scalar_tensor_tensor(
            t[:], x[:], -1.0, r[:], op0=ALU.mult, op1=ALU.mult)
        nc.vector.tensor_scalar_add(t[:], t[:], 2.0)
        nc.vector.tensor_mul(r[:], r[:], t[:])
    return r


def build(nc, cfg=None):
    cfg = dict(cfg or {})
    BD = BF16
    dbg = cfg.get("debug", False)
    stop_after = cfg.get("stop_after", None)   # "index" to skip the block

    x_in = nc.dram_tensor("x", (B2, N, D), F32, kind="ExternalInput")
    noise_in = nc.dram_tensor("noise", (B2, NW, 4), F32, kind="ExternalInput")
    wqk3 = nc.dram_tensor("wqk3", (128, 18 * DC * 128), BD, kind="ExternalInput")
    wv3 = nc.dram_tensor("wv3", (128, DC * D), BD, kind="ExternalInput")
    wp3 = nc.dram_tensor("wp3", (128, DC * D), BD, kind="ExternalInput")
    wf13 = nc.dram_tensor("wf13", (36, 128, DC * 128), BD, kind="ExternalInput")
    wf23 = nc.dram_tensor("wf23", (36, 128, D), BD, kind="ExternalInput")
    bqk = nc.dram_tensor("bqk", (128, 18), F32, kind="ExternalInput")
    bv_row = nc.dram_tensor("bv_row", (1, D), BD, kind="ExternalInput")
    bproj_row = nc.dram_tensor("bproj_row", (1, D), BD, kind="ExternalInput")
    bfc1 = nc.dram_tensor("bfc1", (128, 36), F32, kind="ExternalInput")
    bfc2_row = nc.dram_tensor("bfc2_row", (1, D), BD, kind="ExternalInput")

    out = nc.dram_tensor("out", (B2, N, D), F32, kind="ExternalOutput")
    aden_d = nc.dram_tensor("aden_d", (B2, H, T), BD, kind="Internal")
    aden_raw = nc.dram_tensor("aden_raw", (B2, H, T), F32, kind="Internal")

    wb_np, iota_np, jrow_np, ltm_np = host_constants()
    winbase = nc.inline_tensor(wb_np, name="winbase")
    iota128 = nc.inline_tensor(np.arange(128, dtype=np.float32), name="iota128")
    iota256 = nc.inline_tensor(iota_np, name="iota256")
    jrow = nc.inline_tensor(jrow_np, name="jrow")
    ltmask = nc.inline_tensor(ltm_np, name="ltmask")

    okind = "ExternalOutput" if dbg else "Internal"
    xn_d = [nc.dram_tensor(f"xn_d{b}", (N, D), F32, kind=okind) for b in range(B2)]
    ktmp_d = [nc.dram_tensor(f"ktmp_d{b}", (NS,), I32, kind="Internal")
              for b in range(B2)]
    g_d = [nc.dram_tensor(f"g_d{b}", (N,), I32, kind=okind) for b in range(B2)]
    keep_d = [nc.dram_tensor(f"keep_d{b}", (T,), I32, kind=okind)
              for b in range(B2)]
    bo_d = [nc.dram_tensor(f"bo_d{b}", (T, D), BD, kind=okind)
            for b in range(B2)]
    mrow_d = [nc.dram_tensor(f"mrow_d{b}", (NS,), F32, kind="Internal")
              for b in range(B2)]
    sidx_d = [nc.dram_tensor(f"sidx_d{b}", (NS,), I16, kind="Internal")
              for b in range(B2)]
    didx_d = [nc.dram_tensor(f"didx_d{b}", (NW,), I16, kind="Internal")
              for b in range(B2)]
    kidx_d = [nc.dram_tensor(f"kidx_d{b}", (T,), I16, kind="Internal")
              for b in range(B2)]
    gidx_d = [nc.dram_tensor(f"gidx_d{b}", (N,), I16, kind="Internal")
              for b in range(B2)]
    ninv_d = [nc.dram_tensor(f"ninv_d{b}", (N,), F32, kind="Internal")
              for b in range(B2)]
    mk_d = [nc.dram_tensor(f"mk_d{b}", (NS,), F32, kind="Internal")
            for b in range(B2)]
    tok_d = [nc.dram_tensor(f"tok_d{b}", (N,), F32, kind="Internal")
             for b in range(B2)]
    val_d = [nc.dram_tensor(f"val_d{b}", (N,), F32, kind="Internal")
             for b in range(B2)]
    kperm_d = [nc.dram_tensor(f"kperm_d{b}", (T,), I32, kind="Internal")
               for b in range(B2)]
    if dbg:
        dbg_sc = nc.dram_tensor("dbg_sc", (B2, 128, 2, 4), F32,
                                kind="ExternalOutput")
        dbg_ms = nc.dram_tensor("dbg_ms", (B2, NS), F32, kind="ExternalOutput")
        dbg_best = nc.dram_tensor("dbg_best", (B2, NS), F32, kind="ExternalOutput")
        dbg_rank = nc.dram_tensor("dbg_rank", (B2, NS), F32, kind="ExternalOutput")

    ctx = ExitStack()
    tc = ctx.enter_context(tile.TileContext(nc))

    consts = ctx.enter_context(tc.tile_pool(name="consts", bufs=1))
    nc.gpsimd.load_library(library_config.mlp)
    ident = consts.tile([128, 128], F32)
    io128 = consts.tile([128, 1], F32)
    nc.sync.dma_start(io128[:], bass.AP(
        tensor=iota128, offset=0, ap=[[1, 128], [1, 1]]))
    make_identity(nc, ident)
    if BD != F32:
        identb = consts.tile([128, 128], BD)
        nc.vector.tensor_copy(identb[:], ident[:])
    else:
        identb = ident
    iotaBIG = consts.tile([128, NW], F32)
    nc.sync.dma_start(iotaBIG[:], bass.AP(
        tensor=iota256, offset=0, ap=[[0, 128], [1, NW]]))
    nc.vector.tensor_scalar_add(iotaBIG[:], iotaBIG[:], BIG)
    jb = consts.tile([128, 3], F32)
    nc.sync.dma_start(jb[:], bass.AP(tensor=jrow, offset=0, ap=[[0, 128], [1, 3]]))
    wbt = consts.tile([128, 2], F32)
    for c in range(2):
        nc.sync.dma_start(
            wbt[:, c:c + 1],
            bass.AP(tensor=winbase, offset=128 * c, ap=[[1, 128], [1, 1]]))
    witer = consts.tile([1, NW], F32)
    nc.sync.dma_start(witer[:], iota256[None, :])

    # Preload the V weight during the (latency-bound) index phase; wqk is
    # too big to coexist with the index pools and loads at block start.
    wv_pool = ctx.enter_context(tc.tile_pool(name="wv_pool", bufs=1))
    wvt = wv_pool.tile([128, DC, D], BD)

    ipools = ExitStack()
    psumS = ipools.enter_context(tc.tile_pool(name="psumS", bufs=3, space="PSUM"))
    psumT = ipools.enter_context(tc.tile_pool(name="psumT", bufs=3, space="PSUM"))

    # ============ index pipeline (phase-major, samples interleaved) ============
    # cross-phase per-sample tiles live in pX; phase-local scratch rotates
    # via bufs=2 pools so sample 1's work overlaps sample 0's DMA waits.
    pX = ipools.enter_context(tc.tile_pool(name="pX", bufs=1))
    st = {}; dt = {}; msc = {}; bst = {}; rnk = {}
    sidx_w = {}; didx_w = {}; srow_s = {}; drow_w = {}; brow = {}; rrow = {}

    # ---- A+B fused: norms, window scores, and xn all from slot tiles ----
    # Window-layout slot rows are exactly token rows of x, so sumsq over a
    # slot row reproduces the token norm bit-for-bit; xn is then written via
    # the inverse slot AP. One read of x, no chunk-layout pass at all.
    def slot_ap(b, s4, c):
        sy, sx = s4 >> 1, s4 & 1
        return bass.AP(
            tensor=x_in, offset=(b * N + 64 * 8 * c + 32 * sy + sx) * D,
            ap=[[64 * D, 8], [2 * D, 16], [1, D]])

    def slot_ap_xn(b, s4, c):
        sy, sx = s4 >> 1, s4 & 1
        return bass.AP(
            tensor=xn_d[b], offset=(64 * 8 * c + 32 * sy + sx) * D,
            ap=[[64 * D, 8], [2 * D, 16], [1, D]])

    PAIRS = [(0, 1), (0, 2), (0, 3), (1, 2), (1, 3), (2, 3)]
    PIDX = {p: i for i, p in enumerate(PAIRS)}

    with (
        tc.tile_pool(name="pB", bufs=1) as pB,
        tc.tile_pool(name="pBs", bufs=2) as pBs,
        tc.tile_pool(name="pR", bufs=2) as pR,
        tc.tile_pool(name="pXN", bufs=6) as pXN,
    ):
        slt = {}
        for b in range(B2):
            for s4 in range(4):
                for c in range(2):
                    sl = pB.tile([128, D], F32, tag=f"sl{b}_{s4}_{c}",
                                 name=f"sl{b}_{s4}_{c}")
                    nc.sync.dma_start(sl[:], slot_ap(b, s4, c))
                    slt[b, s4, c] = sl
        for dc in range(DC):
            nc.sync.dma_start(wvt[:, dc, :], wv3[:, dc * D:(dc + 1) * D])

        rslt = {}
        for b in range(B2):
            # per-token sumsq in window layout (scalar engine)
            nsq = pB.tile([128, 2, 4], F32, tag=f"nsq{b}", name=f"nsq{b}")
            for s4 in range(4):
                for c in range(2):
                    sq = pBs.tile([128, D], BF16, tag="sqscr")
                    nc.scalar.activation(
                        sq[:], slt[b, s4, c][:], AF.Square,
                        accum_out=nsq[:, c, s4:s4 + 1])
            # rsl = 1/(norm + 1e-6), same Newton chain as the reference path
            s08 = pB.tile([128, 2, 4], F32, tag=f"s08{b}", name=f"s08{b}")
            nc.scalar.activation(s08[:], nsq[:], AF.Sqrt)
            rs8 = pB.tile([128, 2, 4], F32, tag=f"rs8{b}", name=f"rs8{b}")
            t8 = pB.tile([128, 2, 4], F32, tag=f"t8{b}", name=f"t8{b}")
            nc.vector.reciprocal(rs8[:], s08[:])
            for _ in range(2):
                nc.vector.scalar_tensor_tensor(
                    t8[:], s08[:], -1.0, rs8[:], op0=ALU.mult, op1=ALU.mult)
                nc.vector.tensor_scalar_add(t8[:], t8[:], 2.0)
                nc.vector.tensor_mul(rs8[:], rs8[:], t8[:])
            q8 = pB.tile([128, 2, 4], F32, tag=f"q8{b}", name=f"q8{b}")
            nc.vector.tensor_mul(q8[:], nsq[:], rs8[:])
            nc.vector.tensor_add(q8[:], q8[:], s08[:])
            nc.vector.tensor_scalar(
                q8[:], q8[:], 0.5, 1e-6, op0=ALU.mult, op1=ALU.add)
            iv = pB.tile([128, 2, 4], F32, tag=f"inv{b}", name=f"inv{b}")
            nc.vector.reciprocal(iv[:], q8[:])
            for _ in range(2):
                nc.vector.scalar_tensor_tensor(
                    t8[:], q8[:], -1.0, iv[:], op0=ALU.mult, op1=ALU.mult)
                nc.vector.tensor_scalar_add(t8[:], t8[:], 2.0)
                nc.vector.tensor_mul(iv[:], iv[:], t8[:])
            rslt[b] = iv

        for b in range(B2):
            # xn_d rows first: C-phase gathers and the B->C pool handoff
            # both wait on these, so they must clear the vector queue early
            for s4 in range(4):
                for c in range(2):
                    sln = pXN.tile([128, D], F32, tag="sln")
                    nc.vector.tensor_scalar_mul(
                        sln[:], slt[b, s4, c][:], rslt[b][:, c, s4:s4 + 1])
                    nc.sync.dma_start(slot_ap_xn(b, s4, c), sln[:])
            dd = pB.tile([128, 2, 6], F32, tag=f"dots{b}", name=f"dots{b}")
            for c in range(2):
                for pi, (sa, sb) in enumerate(PAIRS):
                    scr = pBs.tile([128, D], F32, tag="scr")
                    nc.vector.scalar_tensor_tensor(
                        scr[:], slt[b, sa, c][:], 1.0, slt[b, sb, c][:],
                        op0=ALU.bypass, op1=ALU.mult,
                        accum_out=dd[:, c, pi:pi + 1])
            nt = pB.tile([128, 2, 4], F32, tag=f"nt{b}", name=f"nt{b}")
            for c in range(2):
                nc.sync.dma_start(
                    nt[:, c, :],
                    bass.AP(tensor=noise_in, offset=(b * NW + 128 * c) * 4,
                            ap=[[4, 128], [1, 4]]))
            nc.vector.tensor_scalar_mul(nt[:], nt[:], 0.1)
            sc = pB.tile([128, 2, 4], F32, tag=f"scsc{b}", name=f"scsc{b}")
            rsl = rslt[b]
            for s4 in range(4):
                others = [s for s in range(4) if s != s4]
                acc = pBs.tile([128, 2], F32, tag="acc")
                t2 = pBs.tile([128, 2], F32, tag="t2")
                o0 = others[0]
                nc.vector.tensor_mul(
                    acc[:], dd[:, :, PIDX[min(s4, o0), max(s4, o0)]],
                    rsl[:, :, o0])
                for o in others[1:]:
                    nc.vector.tensor_mul(
                        t2[:], dd[:, :, PIDX[min(s4, o), max(s4, o)]],
                        rsl[:, :, o])
                    nc.vector.tensor_add(acc[:], acc[:], t2[:])
                nc.vector.tensor_mul(acc[:], acc[:], rsl[:, :, s4])
                nc.vector.scalar_tensor_tensor(
                    sc[:, :, s4], acc[:], 0.25, nt[:, :, s4],
                    op0=ALU.mult, op1=ALU.add)
            if dbg:
                nc.sync.dma_start(dbg_sc[b], sc[:])

            # argmax over the 4 slots, first max wins:
            # dl = (1-e0) * (1 + (1-e1) * (2 - e2))
            mm = pBs.tile([128, 2], F32, tag="mm")
            m23 = pBs.tile([128, 2], F32, tag="m23")
            nc.vector.tensor_tensor(mm[:], sc[:, :, 0], sc[:, :, 1], op=ALU.max)
            nc.vector.tensor_tensor(m23[:], sc[:, :, 2], sc[:, :, 3], op=ALU.max)
            nc.vector.tensor_tensor(mm[:], mm[:], m23[:], op=ALU.max)
            e0 = pBs.tile([128, 2], F32, tag="e0")
            e1 = pBs.tile([128, 2], F32, tag="e1")
            e2 = pBs.tile([128, 2], F32, tag="e2")
            nc.vector.tensor_tensor(e0[:], sc[:, :, 0], mm[:], op=ALU.is_equal)
            nc.vector.tensor_tensor(e1[:], sc[:, :, 1], mm[:], op=ALU.is_equal)
            nc.vector.tensor_tensor(e2[:], sc[:, :, 2], mm[:], op=ALU.is_equal)
            u2 = pBs.tile([128, 2], F32, tag="u2")
            nc.vector.tensor_scalar(
                u2[:], e2[:], -1.0, 2.0, op0=ALU.mult, op1=ALU.add)
            v1 = pBs.tile([128, 2], F32, tag="v1")
            nc.vector.scalar_tensor_tensor(
                v1[:], e1[:], -1.0, u2[:], op0=ALU.mult, op1=ALU.mult)
            u1 = pBs.tile([128, 2], F32, tag="u1")
            nc.vector.tensor_add(u1[:], v1[:], u2[:])
            nc.vector.tensor_scalar_add(u1[:], u1[:], 1.0)
            v0 = pBs.tile([128, 2], F32, tag="v0")
            nc.vector.scalar_tensor_tensor(
                v0[:], e0[:], -1.0, u1[:], op0=ALU.mult, op1=ALU.mult)
            dl = pBs.tile([128, 2], F32, tag="dl")
            nc.vector.tensor_add(dl[:], v0[:], u1[:])

            # dst token = winbase + 32*(dl>>1) + (dl&1)
            syt = pBs.tile([128, 2], F32, tag="syt")
            nc.vector.tensor_scalar(syt[:], dl[:], 2.0, None, op0=ALU.is_ge)
            sxt = pBs.tile([128, 2], F32, tag="sxt")
            nc.vector.scalar_tensor_tensor(
                sxt[:], syt[:], -2.0, dl[:], op0=ALU.mult, op1=ALU.add)
            dtb = pX.tile([128, 2], F32, tag=f"dt{b}")
            nc.vector.scalar_tensor_tensor(
                dtb[:], syt[:], 32.0, sxt[:], op0=ALU.mult, op1=ALU.add)
            nc.vector.tensor_add(dtb[:], dtb[:], wbt[:])
            dt[b] = dtb

            # src tokens [128, 2, 3]
            stb_ = pX.tile([128, 2, 3], F32, tag=f"st{b}")
            for c in range(2):
                ge = pBs.tile([128, 3], F32, tag="ge")
                nc.vector.tensor_scalar(
                    ge[:], jb[:], dl[:, c:c + 1], None, op0=ALU.is_ge)
                sl3 = pBs.tile([128, 3], F32, tag="sl3")
                nc.vector.tensor_add(sl3[:], ge[:], jb[:])
                sy2 = pBs.tile([128, 3], F32, tag="sy2")
                nc.vector.tensor_scalar(
                    sy2[:], sl3[:], 2.0, None, op0=ALU.is_ge)
                sx2 = pBs.tile([128, 3], F32, tag="sx2")
                nc.vector.scalar_tensor_tensor(
                    sx2[:], sy2[:], -2.0, sl3[:], op0=ALU.mult, op1=ALU.add)
                nc.vector.scalar_tensor_tensor(
                    stb_[:, c, :], sy2[:], 32.0, sx2[:],
                    op0=ALU.mult, op1=ALU.add)
                nc.vector.tensor_scalar_add(
                    stb_[:, c, :], stb_[:, c, :], wbt[:, c:c + 1])
            st[b] = stb_

            # ---- R: token rows + idx staging (interleaved with B) ----
            srow = pX.tile([1, NS], F32, tag=f"srow_s{b}")
            for c in range(2):
                nc.sync.dma_start(
                    srow[:, 384 * c:384 * (c + 1)].rearrange(
                        "a (p j) -> a p j", p=128),
                    st[b][:, c, :])
            srow_s[b] = srow
            drow = pX.tile([1, NW], F32, tag=f"drow_w{b}")
            for c in range(2):
                nc.sync.dma_start(
                    drow[:, 128 * c:128 * (c + 1)], dt[b][:, c:c + 1])
            drow_w[b] = drow
            # i-order idx rows for dma_gather (i = 128*chunk + p)
            sgrow = pR.tile([1, NS], F32, tag="sgrow")
            for h2 in range(2):
                for j3 in range(3):
                    nc.sync.dma_start(
                        sgrow[:, 384 * h2 + 128 * j3:384 * h2 + 128 * (j3 + 1)]
                        .rearrange("a (k o) -> a k o", o=1),
                        st[b][:, h2, j3:j3 + 1])
            sgi = pR.tile([1, NS], I16, tag="sgi")
            nc.vector.tensor_copy(sgi[:], sgrow[:])
            nc.sync.dma_start(sidx_d[b][None, :], sgi[:])
            sw = pX.tile([128, 48], I16, tag=f"sidx_w{b}")
            for g8 in range(8):
                nc.sync.dma_start(sw[16 * g8:16 * (g8 + 1), :], bass.AP(
                    tensor=sidx_d[b], offset=0, ap=[[1, 16], [16, 48]]))
            sidx_w[b] = sw
            dgi = pR.tile([1, NW], I16, tag="dgi")
            nc.vector.tensor_copy(dgi[:], drow[:])
            nc.sync.dma_start(didx_d[b][None, :], dgi[:])
            dw = pX.tile([128, 16], I16, tag=f"didx_w{b}")
            for g8 in range(8):
                nc.sync.dma_start(dw[16 * g8:16 * (g8 + 1), :], bass.AP(
                    tensor=didx_d[b], offset=0, ap=[[1, 16], [16, 16]]))
            didx_w[b] = dw


    # ---- C: gather xn rows, transpose, sim, best + exact stable rank ----
    with (
        tc.tile_pool(name="pC1", bufs=1) as pC,
        tc.tile_pool(name="pC2", bufs=2) as pC2,
    ):
        for b in range(B2):
            mscb = pX.tile([128, 6], F32, tag=f"msc{b}")
            bstb = pX.tile([128, 6], F32, tag=f"bst{b}")
            rnkb = pX.tile([128, 6], F32, tag=f"rnk{b}")
            xnd = pC.tile([128, 2, D], F32, tag=f"xnd{b}")
            nc.gpsimd.dma_gather(
                out_ap=xnd[:], in_ap=xn_d[b][:], idxs_ap=didx_w[b][:],
                num_idxs=NW, num_idxs_reg=NW, elem_size=D)
            xndT = pC.tile([128, DC, NW], F32, tag=f"xndT{b}")
            for c in range(2):
                for dc in range(DC):
                    pt = psumT.tile([128, 128], F32, tag="pt")
                    nc.tensor.transpose(
                        pt[:], xnd[:, c, 128 * dc:128 * (dc + 1)], ident[:])
                    if dc % 2 == 0:
                        nc.scalar.copy(
                            xndT[:, dc, 128 * c:128 * (c + 1)], pt[:])
                    else:
                        nc.vector.tensor_copy(
                            xndT[:, dc, 128 * c:128 * (c + 1)], pt[:])
            for half in range(2):
                xns = pC.tile([128, 3, D], F32, tag=f"xns{half}")
                nc.gpsimd.dma_gather(
                    out_ap=xns[:], in_ap=xn_d[b][:],
                    idxs_ap=sidx_w[b][:, 24 * half:24 * (half + 1)],
                    num_idxs=384, num_idxs_reg=384, elem_size=D)
                xnsT = pC.tile([128, 3, DC, 128], F32, tag=f"xnsT{half}")
                for c3 in range(3):
                    for dc in range(DC):
                        pt = psumT.tile([128, 128], F32, tag="pt")
                        nc.tensor.transpose(
                            pt[:], xns[:, c3, 128 * dc:128 * (dc + 1)],
                            ident[:])
                        if dc % 2 == 0:
                            nc.scalar.copy(xnsT[:, c3, dc, :], pt[:])
                        else:
                            nc.vector.tensor_copy(xnsT[:, c3, dc, :], pt[:])
                for c3 in range(3):
                    cc6 = 3 * half + c3
                    ps = psumS.tile([128, NW], F32, tag="ps")
                    for dc in range(DC):
                        nc.tensor.matmul(
                            ps[:], xnsT[:, c3, dc, :], xndT[:, dc, :],
                            start=(dc == 0), stop=(dc == DC - 1))
                    nc.vector.reduce_max(
                        mscb[:, cc6:cc6 + 1], ps[:], axis=AX.X)
                    eqt = pC.tile([128, NW], F32, tag="eqt")
                    nc.vector.tensor_scalar(
                        eqt[:], ps[:], mscb[:, cc6:cc6 + 1], None,
                        op0=ALU.is_equal)
                    mskt = pC.tile([128, NW], F32, tag="mskt")
                    nc.vector.scalar_tensor_tensor(
                        mskt[:], eqt[:], -BIG, iotaBIG[:],
                        op0=ALU.mult, op1=ALU.add)
                    nc.vector.tensor_reduce(
                        bstb[:, cc6:cc6 + 1], mskt[:], axis=AX.X, op=ALU.min)
            msc[b], bst[b], rnk[b] = mscb, bstb, rnkb

        for b in range(B2):
            # maxsim broadcast via DRAM bounce (s order)
            for cc6 in range(6):
                c, j = cc6 // 3, cc6 % 3
                nc.sync.dma_start(
                    bass.AP(tensor=mrow_d[b], offset=384 * c + j,
                            ap=[[3, 128], [1, 1]]),
                    msc[b][:, cc6:cc6 + 1])
            mbc = pC2.tile([128, NS], F32, tag="mbc")
            nc.sync.dma_start(
                mbc[:],
                bass.AP(tensor=mrow_d[b], offset=0, ap=[[0, 128], [1, NS]]))

            # exact stable rank
            gcnt = pC2.tile([128, 1], F32, tag="gcnt")
            ecnt = pC2.tile([128, 1], F32, tag="ecnt")
            for lh in range(2):
                ltm = pC2.tile([128, 3, NS], F32, tag="ltm")
                for c3 in range(3):
                    nc.sync.dma_start(ltm[:, c3, :], ltmask[3 * lh + c3])
                for c3 in range(3):
                    cc6 = 3 * lh + c3
                    sc1 = pC2.tile([128, NS], F32, tag="sc1")
                    nc.vector.scalar_tensor_tensor(
                        sc1[:], mbc[:], msc[b][:, cc6:cc6 + 1], mbc[:],
                        op0=ALU.is_gt, op1=ALU.bypass, accum_out=gcnt[:])
                    sc2 = pC2.tile([128, NS], F32, tag="sc2")
                    nc.vector.scalar_tensor_tensor(
                        sc2[:], mbc[:], msc[b][:, cc6:cc6 + 1], ltm[:, c3, :],
                        op0=ALU.is_equal, op1=ALU.mult, accum_out=ecnt[:])
                    nc.vector.tensor_add(
                        rnk[b][:, cc6:cc6 + 1], gcnt[:], ecnt[:])

            # best + rank rows in s order
            browb = pX.tile([1, NS], F32, tag=f"brow{b}")
            rrowb = pX.tile([1, NS], F32, tag=f"rrow{b}")
            for cc6 in range(6):
                c, j = cc6 // 3, cc6 % 3
                dst_b = browb[:].rearrange(
                    "a (c p j) -> a p c j", c=2, p=128)[:, :, c, j]
                nc.sync.dma_start(dst_b, bst[b][:, cc6:cc6 + 1])
                dst_r = rrowb[:].rearrange(
                    "a (c p j) -> a p c j", c=2, p=128)[:, :, c, j]
                nc.sync.dma_start(dst_r, rnk[b][:, cc6:cc6 + 1])
            brow[b], rrow[b] = browb, rrowb
            if dbg:
                nc.sync.dma_start(dbg_ms[b][None, :], mbc[0:1, :])
                nc.sync.dma_start(dbg_best[b][None, :], browb[:])
                nc.sync.dma_start(dbg_rank[b][None, :], rrowb[:])

    # ---- K: masks, prefix sum, one-hot compaction, g construction ----
    with tc.tile_pool(name="pK", bufs=2) as pK:
        for b in range(B2):
            kpm = pK.tile([1, NS], F32, tag="kpm")
            nc.vector.tensor_scalar(
                kpm[:], rrow[b][:], 512.0, None, op0=ALU.is_ge)
            kex = pK.tile([1, NS], F32, tag="kex")
            nc.vector.tensor_tensor_scan(
                kex[:], kpm[:], kpm[:], 0.0, op0=ALU.add, op1=ALU.bypass)
            nc.vector.tensor_sub(kex[:], kex[:], kpm[:])
            # v_src = best + kpm*(256 + kex - best)
            tq = pK.tile([1, NS], F32, tag="tmp768")
            nc.vector.tensor_sub(tq[:], kex[:], brow[b][:])
            nc.vector.scalar_tensor_tensor(
                tq[:], tq[:], 256.0, kpm[:], op0=ALU.add, op1=ALU.mult)
            vsr = pK.tile([1, NS], F32, tag="vsr")
            nc.vector.tensor_add(vsr[:], tq[:], brow[b][:])

            # f32 token row [dst_w | src_s] and value row [witer | vsr]
            trow = pK.tile([1, N], F32, tag="trow")
            nc.scalar.copy(trow[:, :NW], drow_w[b][:])
            nc.scalar.copy(trow[:, NW:], srow_s[b][:])
            vrow = pK.tile([1, N], F32, tag="vrow")
            nc.scalar.copy(vrow[:, :NW], witer[:])
            nc.scalar.copy(vrow[:, NW:], vsr[:])
            # masked keep-rank row: kpm*(kex+1) - 1  (pruned -> -1)
            mk = pK.tile([1, NS], F32, tag="mk")
            nc.vector.scalar_tensor_tensor(
                mk[:], kex[:], 1.0, kpm[:], op0=ALU.add, op1=ALU.mult)
            nc.vector.tensor_scalar_add(mk[:], mk[:], -1.0)
            nc.sync.dma_start(mk_d[b][None, :], mk[:])
            nc.sync.dma_start(tok_d[b][None, :], trow[:])
            nc.sync.dma_start(val_d[b][None, :], vrow[:])
            mkb = pK.tile([128, NS], F32, tag="mkb")
            nc.sync.dma_start(mkb[:], bass.AP(
                tensor=mk_d[b], offset=0, ap=[[0, 128], [1, NS]]))
            stb = pK.tile([128, NS], F32, tag="stb")
            nc.sync.dma_start(stb[:], bass.AP(
                tensor=tok_d[b], offset=NW, ap=[[0, 128], [1, NS]]))
            tkb = pK.tile([128, N], F32, tag="tkb")
            nc.sync.dma_start(tkb[:], bass.AP(
                tensor=tok_d[b], offset=0, ap=[[0, 128], [1, N]]))
            vlb = pK.tile([128, N], F32, tag="vlb")
            nc.sync.dma_start(vlb[:], bass.AP(
                tensor=val_d[b], offset=0, ap=[[0, 128], [1, N]]))

            krow = pK.tile([1, T], F32, tag="krow")
            nc.scalar.copy(krow[:, :NW], drow_w[b][:])
            eqk = pK.tile([128, NS], F32, tag="eqk")
            for c2 in range(2):
                rtg = pK.tile([128, 1], F32, tag="rtg")
                nc.vector.tensor_scalar_add(rtg[:], io128[:], float(128 * c2))
                kv = pK.tile([128, 1], F32, tag="kv")
                nc.vector.scalar_tensor_tensor(
                    eqk[:], mkb[:], rtg[:], stb[:],
                    op0=ALU.is_equal, op1=ALU.mult, accum_out=kv[:])
                seg = krow[:, NW + 128 * c2:NW + 128 * (c2 + 1)]
                nc.sync.dma_start(seg.rearrange("a (k o) -> a k o", o=1), kv[:])

            grow = pK.tile([1, N], F32, tag="grow")
            eqg = pK.tile([128, N], F32, tag="eqg")
            for c8 in range(8):
                ttg = pK.tile([128, 1], F32, tag="ttg")
                nc.vector.tensor_scalar_add(ttg[:], io128[:], float(128 * c8))
                gv = pK.tile([128, 1], F32, tag="gv")
                nc.vector.scalar_tensor_tensor(
                    eqg[:], tkb[:], ttg[:], vlb[:],
                    op0=ALU.is_equal, op1=ALU.mult, accum_out=gv[:])
                seg = grow[:, 128 * c8:128 * (c8 + 1)]
                nc.sync.dma_start(seg.rearrange("a (k o) -> a k o", o=1), gv[:])

            # int16 gather-idx staging
            ki16 = pK.tile([1, T], I16, tag="ki16")
            nc.vector.tensor_copy(ki16[:], krow[:])
            nc.sync.dma_start(kidx_d[b][None, :], ki16[:])
            gi16 = pK.tile([1, N], I16, tag="gi16")
            nc.vector.tensor_copy(gi16[:], grow[:])
            nc.sync.dma_start(gidx_d[b][None, :], gi16[:])
            if dbg:
                ki32 = pK.tile([1, T], I32, tag="ki32")
                nc.vector.tensor_copy(ki32[:], krow[:])
                nc.sync.dma_start(keep_d[b][None, :], ki32[:])
                gi32 = pK.tile([1, N], I32, tag="gi32")
                nc.vector.tensor_copy(gi32[:], grow[:])
                nc.sync.dma_start(g_d[b][None, :], gi32[:])
    ipools.close()

    if stop_after == "index":
        ctx.close()
        return dict(nc=nc)

    # =================== DiT block ===================
    build_block(nc, tc, ctx, cfg, dict(
        identb=identb, ident=ident, x_in=x_in,
        wqk3=wqk3, wvt=wvt, wp3=wp3, wf13=wf13, wf23=wf23,
        bqk=bqk, bv_row=bv_row, bproj_row=bproj_row, bfc1=bfc1,
        bfc2_row=bfc2_row, bo_d=bo_d, kidx_d=kidx_d, aden_d=aden_d,
        aden_raw=aden_raw, gidx_d=gidx_d, out=out, dbg=dbg))

    ctx.close()
    return dict(nc=nc)


def layer_norm(nc, pool, small, xin, yout, eps=1e-6):
    """Row LN: yout = (x - mu) * rsqrt(var + eps). xin fp32 [128, D]."""
    mu = small.tile([128, 1], F32, tag="ln_mu")
    nc.vector.tensor_reduce(mu[:], xin, axis=AX.X, op=ALU.add)
    nc.vector.tensor_scalar_mul(mu[:], mu[:], 1.0 / D)
    xc = pool.tile([128, D], F32, tag="ln_xc")
    nc.vector.tensor_scalar(xc[:], xin, mu[:], None, op0=ALU.subtract)
    sq = pool.tile([128, D], F32, tag="ln_sq")
    var = small.tile([128, 1], F32, tag="ln_var")
    nc.vector.scalar_tensor_tensor(
        sq[:], xc[:], 1.0, xc[:], op0=ALU.bypass, op1=ALU.mult, accum_out=var[:])
    nc.vector.tensor_scalar(
        var[:], var[:], 1.0 / D, eps, op0=ALU.mult, op1=ALU.add)
    sd = small.tile([128, 1], F32, tag="ln_sd")
    nc.scalar.activation(sd[:], var[:], AF.Sqrt)
    rstd = small.tile([128, 1], F32, tag="ln_rstd")
    nc.vector.reciprocal(rstd[:], sd[:])
    nc.vector.tensor_scalar_mul(yout, xc[:], rstd[:])


def build_block(nc, tc, ctx, cfg, env):
    BD = BF16
    stop_after = cfg.get("stop_after", None)
    identb = env["identb"]
    x_in = env["x_in"]
    wqk3, wvt = env["wqk3"], env["wvt"]
    wp3, wf13, wf23 = env["wp3"], env["wf13"], env["wf23"]
    bo_d, kidx_d = env["bo_d"], env["kidx_d"]
    aden_d, aden_raw = env["aden_d"], env["aden_raw"]
    gidx_d, out = env["gidx_d"], env["out"]

    # PSUM banks: psA 2 + psB 2 on ctx; psT (LN phases) and psPO/psD
    # (attention) are scoped so attention gets 2 bufs each: max 8 banks
    bcp = ctx.enter_context(tc.tile_pool(name="bcp", bufs=1))
    psA = ctx.enter_context(tc.tile_pool(name="psA", bufs=2, space="PSUM"))
    psB = ctx.enter_context(tc.tile_pool(name="psB", bufs=2, space="PSUM"))
    small = ctx.enter_context(tc.tile_pool(name="bsmall", bufs=4))

    bvb = bcp.tile([128, D], BD)
    nc.sync.dma_start(bvb[:], bass.AP(
        tensor=env["bv_row"], offset=0, ap=[[0, 128], [1, D]]))
    bpb = bcp.tile([128, D], BD)
    nc.sync.dma_start(bpb[:], bass.AP(
        tensor=env["bproj_row"], offset=0, ap=[[0, 128], [1, D]]))
    bf2b = bcp.tile([128, D], BD)
    nc.sync.dma_start(bf2b[:], bass.AP(
        tensor=env["bfc2_row"], offset=0, ap=[[0, 128], [1, D]]))
    bqkt = bcp.tile([128, 18], F32)
    nc.sync.dma_start(bqkt[:], env["bqk"][:, :])
    bf1t = bcp.tile([128, 36], F32)
    nc.sync.dma_start(bf1t[:], env["bfc1"][:, :])
    # all-ones stationary column (softmax denominators via matmul)
    onesb = bcp.tile([128, 1], BD)
    nc.vector.tensor_scalar(onesb[:], bqkt[:, 0:1], 0.0, 1.0,
                            op0=ALU.mult, op1=ALU.add)
    kidx_t = [None, None]
    for b in range(B2):
        kpt = bcp.tile([128, 32], I16, tag=f"kidx2_{b}", name=f"kidx2_{b}")
        for g8 in range(8):
            nc.sync.dma_start(kpt[16 * g8:16 * (g8 + 1), :], bass.AP(
                tensor=kidx_d[b], offset=0, ap=[[1, 16], [16, 32]]))
        kidx_t[b] = kpt

    def transpose_to(psT, dst_sl, y, dc):
        pt = psT.tile([128, 128], BD, tag="bt")
        nc.tensor.transpose(pt[:], y[:, 128 * dc:128 * (dc + 1)], identb[:])
        if dc % 2 == 0:
            nc.scalar.copy(dst_sl, pt[:])
        else:
            nc.vector.tensor_copy(dst_sl, pt[:])

    # x1 accumulator f32; LN1 gathers write straight into it (x rows),
    # proj then adds b_proj + attention delta, MLP adds the rest.
    p_x1 = ctx.enter_context(tc.tile_pool(name="p_x1", bufs=1))
    x1 = p_x1.tile([128, B2, TC, D], F32)

    with tc.tile_pool(name="p_ot", bufs=1) as p_ot:
        OT = [p_ot.tile([128, DC, T], BD, tag=f"OT{b}", name=f"OT{b}")
              for b in range(B2)]
        with tc.tile_pool(name="p_yt", bufs=1) as p_yt:
            YT = p_yt.tile([128, DC, 2 * T], BD)
            # ---- LN1 -> YT ----
            with (
                tc.tile_pool(name="p_ln1", bufs=2) as p_ln,
                tc.tile_pool(name="psT1", bufs=2, space="PSUM") as psT1,
            ):
                for ct in range(8):
                    b, c4 = divmod(ct, TC)
                    nc.gpsimd.dma_gather(
                        out_ap=x1[:, b, c4:c4 + 1, :], in_ap=x_in[b],
                        idxs_ap=kidx_t[b][:, 8 * c4:8 * (c4 + 1)],
                        num_idxs=128, num_idxs_reg=128, elem_size=D)
                    y = p_ln.tile([128, D], BD, tag="y")
                    layer_norm(nc, p_ln, small, x1[:, b, c4, :], y[:])
                    for dc in range(DC):
                        transpose_to(psT1, YT[:, dc, 128 * ct:128 * (ct + 1)],
                                     y, dc)

            with tc.tile_pool(name="p_v", bufs=1) as p_v:
                V = p_v.tile([128, 2 * TC, D], BD)
                # ---- V (token-major) ----
                for ct in range(8):
                    for ns in range(3):
                        pv = psB.tile([128, 384], F32, tag="b")
                        for dc in range(DC):
                            nc.tensor.matmul(
                                pv[:], YT[:, dc, 128 * ct:128 * (ct + 1)],
                                wvt[:, dc, 384 * ns:384 * (ns + 1)],
                                start=(dc == 0), stop=(dc == DC - 1))
                        nc.vector.scalar_tensor_tensor(
                            V[:, ct, 384 * ns:384 * (ns + 1)], pv[:], 1.0,
                            bvb[:, 384 * ns:384 * (ns + 1)],
                            op0=ALU.bypass, op1=ALU.add)

                if stop_after == "v":
                    return
                # ---- attention: transposed scores, no max-subtract,
                #      denominators via ones-matmul, normalize at oh ----
                with (
                    tc.tile_pool(name="p_wqk2", bufs=1) as p_wqk2,
                    tc.tile_pool(name="psPO", bufs=2, space="PSUM") as psPO,
                    tc.tile_pool(name="psD", bufs=2, space="PSUM") as psD,
                ):
                    for hg in range(2):
                        wqh = p_wqk2.tile([128, DC, DC, 128], BD, tag="wqh")
                        for mcl in range(DC):
                            mc = DC * hg + mcl
                            nc.sync.dma_start(
                                wqh[:, mcl, :, :],
                                wqk3[:, mc * D:(mc + 1) * D])
                        for b in range(B2):
                            with (
                                tc.tile_pool(name="p_qk", bufs=1) as p_qk,
                                tc.tile_pool(name="p_att", bufs=3) as p_att,
                                tc.tile_pool(name="p_et", bufs=2) as p_et,
                            ):
                                QKT = p_qk.tile([128, DC, T], BD,
                                                name=f"QKT{b}{hg}")
                                for mcl in range(DC):
                                    mc = DC * hg + mcl
                                    pq = psA.tile([128, T], F32, tag="a")
                                    for dc in range(DC):
                                        nc.tensor.matmul(
                                            pq[:], wqh[:, mcl, dc, :],
                                            YT[:, dc, T * b:T * (b + 1)],
                                            start=(dc == 0),
                                            stop=(dc == DC - 1))
                                    if mcl % 2 == 0:
                                        nc.scalar.activation(
                                            QKT[:, mcl, :], pq[:],
                                            AF.Identity,
                                            bias=bqkt[:, mc:mc + 1])
                                    else:
                                        nc.vector.tensor_scalar(
                                            QKT[:, mcl, :], pq[:],
                                            bqkt[:, mc:mc + 1], None,
                                            op0=ALU.add)
                                for hl in range(8):
                                    h = 8 * hg + hl
                                    qh = p_att.tile([DH, T], BD, tag="qh")
                                    kh = p_att.tile([DH, T], BD, tag="kh")
                                    for (dst, base) in ((qh, DH * hl),
                                                        (kh, 576 + DH * hl)):
                                        r0 = base
                                        while r0 < base + DH:
                                            mcl, p0 = divmod(r0, 128)
                                            take = min(128 - p0,
                                                       base + DH - r0)
                                            nc.sync.dma_start(
                                                dst[r0 - base:
                                                    r0 - base + take, :],
                                                QKT[p0:p0 + take, mcl, :])
                                            r0 += take
                                    # E^T chunks from transposed scores
                                    ET = p_et.tile([128, TC, T], BD, tag="ET")
                                    for kc in range(TC):
                                        ps = psA.tile([128, T], F32, tag="a")
                                        nc.tensor.matmul(
                                            ps[:],
                                            kh[:, 128 * kc:128 * (kc + 1)],
                                            qh[:], start=True, stop=True)
                                        nc.scalar.activation(
                                            ET[:, kc, :], ps[:], AF.Exp,
                                            scale=RSQ_DH)
                                    # o^T (unnormalized) and denominators
                                    po = psPO.tile([DH, T], F32, tag="po")
                                    pden = psD.tile([1, T], F32, tag="pden")
                                    for kc in range(TC):
                                        nc.tensor.matmul(
                                            po[:],
                                            V[:, TC * b + kc,
                                              DH * h:DH * (h + 1)],
                                            ET[:, kc, :],
                                            start=(kc == 0),
                                            stop=(kc == TC - 1))
                                        nc.tensor.matmul(
                                            pden[:], onesb[:], ET[:, kc, :],
                                            start=(kc == 0),
                                            stop=(kc == TC - 1))
                                    # reciprocal via [32,16] DRAM reshape
                                    # ([1,512] DVE recip is ~4cyc/elem serial)
                                    rraw = small.tile([1, T], F32, tag="rraw")
                                    nc.scalar.copy(rraw[:], pden[:])
                                    nc.sync.dma_start(
                                        aden_raw[b, h][None, :], rraw[:])
                                    r32 = small.tile([32, 16], F32, tag="r32")
                                    nc.sync.dma_start(r32[:], bass.AP(
                                        tensor=aden_raw,
                                        offset=(b * H + h) * T,
                                        ap=[[16, 32], [1, 16]]))
                                    rd32 = small.tile([32, 16], F32,
                                                      tag="rd32")
                                    nc.vector.reciprocal(rd32[:], r32[:])
                                    rdb = small.tile([32, 16], BD, tag="rdb")
                                    nc.vector.tensor_copy(rdb[:], rd32[:])
                                    nc.sync.dma_start(
                                        bass.AP(tensor=aden_d,
                                                offset=(b * H + h) * T,
                                                ap=[[16, 32], [1, 16]]),
                                        rdb[:])
                                    rbc = p_att.tile([DH, T], BD, tag="rbc")
                                    nc.sync.dma_start(rbc[:], bass.AP(
                                        tensor=aden_d, offset=(b * H + h) * T,
                                        ap=[[0, DH], [1, T]]))
                                    oh = p_att.tile([DH, T], BD, tag="oh")
                                    nc.vector.tensor_tensor(
                                        oh[:], po[:], rbc[:], op=ALU.mult)
                                    r0 = DH * h
                                    while r0 < DH * (h + 1):
                                        dc, p0 = divmod(r0, 128)
                                        take = min(128 - p0, DH * (h + 1) - r0)
                                        nc.sync.dma_start(
                                            OT[b][p0:p0 + take, dc, :],
                                            oh[r0 - DH * h:
                                               r0 - DH * h + take, :])
                                        r0 += take

        # ---- proj + residual -> x1, full 9-dc PSUM accumulation ----
        with tc.tile_pool(name="p_wp", bufs=1) as p_wp:
            wpt = p_wp.tile([128, DC, D], BD)
            for dc in range(DC):
                nc.sync.dma_start(wpt[:, dc, :], wp3[:, dc * D:(dc + 1) * D])
            for b in range(B2):
                for c4 in range(TC):
                    nc.vector.tensor_add(
                        x1[:, b, c4, :], x1[:, b, c4, :], bpb[:])
                    for ns in range(3):
                        pp = psB.tile([128, 384], F32, tag="b")
                        for dc in range(DC):
                            nc.tensor.matmul(
                                pp[:], OT[b][:, dc, 128 * c4:128 * (c4 + 1)],
                                wpt[:, dc, 384 * ns:384 * (ns + 1)],
                                start=(dc == 0), stop=(dc == DC - 1))
                        sl = x1[:, b, c4, 384 * ns:384 * (ns + 1)]
                        nc.vector.scalar_tensor_tensor(
                            sl, pp[:], 1.0, sl, op0=ALU.bypass, op1=ALU.add)

    if stop_after == "attn":
        return
    # ---- LN2 + MLP (per-sample supergroups of 18) + inline recover ----
    with tc.tile_pool(name="p_y2", bufs=1) as p_y2:
        Y2T = p_y2.tile([128, DC, 2 * T], BD)
        with (
            tc.tile_pool(name="p_ln2", bufs=2) as p_ln,
            tc.tile_pool(name="psT2", bufs=2, space="PSUM") as psT2,
        ):
            for ct in range(8):
                b, c4 = divmod(ct, TC)
                y = p_ln.tile([128, D], BD, tag="y")
                layer_norm(nc, p_ln, small, x1[:, b, c4, :], y[:])
                for dc in range(DC):
                    transpose_to(psT2, Y2T[:, dc, 128 * ct:128 * (ct + 1)],
                                 y, dc)
                nc.vector.tensor_add(
                    x1[:, b, c4, :], x1[:, b, c4, :], bf2b[:])

        def emit_recover(rb):
            # recover sample rb: gather block rows back to full token order,
            # split in halves so the out-write of half 0 overlaps half 1
            with tc.tile_pool(name="recp", bufs=2) as recp:
                gw = recp.tile([128, 64], I16, tag="gw")
                for g8 in range(8):
                    nc.sync.dma_start(gw[16 * g8:16 * (g8 + 1), :], bass.AP(
                        tensor=gidx_d[rb], offset=0, ap=[[1, 16], [16, 64]]))
                for hf in range(2):
                    og = recp.tile([128, 4, D], BD, tag="og")
                    nc.gpsimd.dma_gather(
                        out_ap=og[:], in_ap=bo_d[rb][:],
                        idxs_ap=gw[:, 32 * hf:32 * (hf + 1)],
                        num_idxs=T, num_idxs_reg=T, elem_size=D)
                    ogf = recp.tile([128, 4, D], F32, tag="ogf")
                    nc.vector.tensor_copy(ogf[:], og[:])
                    nc.sync.dma_start(
                        bass.AP(tensor=out, offset=(rb * N + hf * T) * D,
                                ap=[[D, 128], [128 * D, 4], [1, D]]),
                        ogf[:])

        for b in range(B2):
            for sg in range(2):
                with tc.tile_pool(name="p_ht", bufs=1) as p_ht:
                    HT = p_ht.tile([128, 18, T], BD, name=f"HT{b}{sg}")
                    with tc.tile_pool(name="p_wf1", bufs=3) as p_wf1:
                        for k18 in range(18):
                            mf = 18 * sg + k18
                            wt = p_wf1.tile([128, DC, 128], BD, tag="wf1")
                            nc.sync.dma_start(wt[:], wf13[mf])
                            pf = psA.tile([128, T], F32, tag="a")
                            for dc in range(DC):
                                nc.tensor.matmul(
                                    pf[:], wt[:, dc, :],
                                    Y2T[:, dc, T * b:T * (b + 1)],
                                    start=(dc == 0), stop=(dc == DC - 1))
                            nc.scalar.activation(
                                HT[:, k18, :], pf[:],
                                AF.Gelu_apprx_tanh, bias=bf1t[:, mf:mf + 1])
                    with tc.tile_pool(name="p_wf2", bufs=1) as p_wf2:
                        wf2 = [p_wf2.tile([128, D], BD, tag=f"wf2_{i}",
                                          name=f"wf2t{b}{sg}{i}")
                               for i in range(18)]
                        for i in range(18):
                            nc.sync.dma_start(wf2[i][:], wf23[18 * sg + i])
                        for c4 in range(TC):
                            for ns in range(3):
                                pg = psB.tile([128, 384], F32, tag="b")
                                for i in range(18):
                                    nc.tensor.matmul(
                                        pg[:],
                                        HT[:, i, 128 * c4:128 * (c4 + 1)],
                                        wf2[i][:, 384 * ns:384 * (ns + 1)],
                                        start=(i == 0), stop=(i == 17))
                                sl = x1[:, b, c4, 384 * ns:384 * (ns + 1)]
                                nc.vector.scalar_tensor_tensor(
                                    sl, pg[:], 1.0, sl,
                                    op0=ALU.bypass, op1=ALU.add)

            # ---- write block output rows (DRAM row = 128*c4 + p) ----
            for c4 in range(TC):
                xob = small.tile([128, D], BD, tag="xob")
                nc.vector.tensor_copy(xob[:], x1[:, b, c4, :])
                nc.sync.dma_start(
                    bass.AP(tensor=bo_d[b], offset=c4 * 128 * D,
                            ap=[[D, 128], [1, D]]),
                    xob[:])
            emit_recover(b)


# ======================================================================
# kernel() entry point: full inputs -> full output on 8 NeuronCores
# ======================================================================

_MODULE_CACHE = {}


def _get_module(block_dtype_name):
    # block_dtype_name kept for interface compat; the block is always bf16
    if "bf16" not in _MODULE_CACHE:
        from concourse import bacc
        nc = bacc.Bacc(None, target_bir_lowering=False)
        build(nc, {})
        nc.compile()
        _MODULE_CACHE["bf16"] = nc
    return _MODULE_CACHE["bf16"]


def kernel(x, noise, ln1_g, ln1_b, ln2_g, ln2_b, w_qkv, b_qkv, w_proj, b_proj,
           w_fc1, b_fc1, w_fc2, b_fc2, block_dtype="f32r", **run_kw):
    from concourse import bass_utils

    x = np.ascontiguousarray(np.asarray(x, np.float32))
    noise = np.ascontiguousarray(np.asarray(noise, np.float32))
    B = x.shape[0]
    n_cores = B // B2
    wt = retile_weights(
        dict(ln1_g=ln1_g, ln1_b=ln1_b, ln2_g=ln2_g, ln2_b=ln2_b,
             w_qkv=w_qkv, b_qkv=b_qkv, w_proj=w_proj, b_proj=b_proj,
             w_fc1=w_fc1, b_fc1=b_fc1, w_fc2=w_fc2, b_fc2=b_fc2))

    nc = _get_module(block_dtype)
    in_maps = []
    for c in range(n_cores):
        m = dict(x=x[B2 * c:B2 * (c + 1)], noise=noise[B2 * c:B2 * (c + 1)])
        m.update(wt)
        in_maps.append(m)
    res = bass_utils.run_bass_kernel_spmd(
        nc, in_maps, core_ids=list(range(n_cores)), **run_kw)
    out = np.concatenate([res.results[c]["out"] for c in range(n_cores)], axis=0)
    if run_kw.get("trace"):
        return out, res
    return out

